# revision 1
# baseline (speedup 1.0000x reference)
"""Self-contained Trainium2 Bass kernel for the 2-layer GAT problem
(nn_GAT_26714696581831). 8-core SPMD: edges sorted by dst, 8 dst-range
shards; per-window one-hot matmul aggregation with dma_gather row fetches.

kernel(**inputs) takes the FULL unsharded inputs and returns the FULL
[50000, 2] output.
"""
import sys
sys.path.insert(0, '/opt/trn_rl_repo')
import numpy as np
import concourse.bass as bass
import concourse.mybir as mybir
import concourse.tile as tile
from concourse import library_config
from concourse.masks import make_identity
from concourse.bass_utils import run_bass_kernel_spmd

N_NODES = 50000
"""Workarounds for this walrus build, which rejects any instruction carrying
more than one sync-wait command: hoist extra waits onto same-engine NoOps
inserted immediately before the instruction."""


_ctr = [0]

def split_multi_waits(nc, max_waits=1):
    for fn in nc.m.functions:
        for bb in fn.blocks:
            insts = bb.instructions
            i = 0
            while i < len(insts):
                ins = insts[i]
                si = ins.sync_info
                if si is not None and si.on_wait and len(si.on_wait) > max_waits:
                    waits = list(si.on_wait)
                    keep = waits[-max_waits:]
                    hoist = waits[:-max_waits]
                    si.on_wait = keep
                    for w in hoist:
                        _ctr[0] += 1
                        n = mybir.InstNoOp(name=f"waitsplit-{_ctr[0]}", ins=[], outs=[])
                        n.engine = ins.engine
                        n.sync_info = mybir.SyncInfo(on_wait=[w], on_update=[])
                        insts.insert(i, n)
                        i += 1
                i += 1


def fix_library_reloads(nc):
    """bass_rust leaves InstPseudoReloadLibraryIndex.instr empty; this walrus
    rejects zero-length ISA instructions. Encode the 64-byte
    PSEUDO_LIBRARY_RELOAD_INDEX struct with the live ISA tables."""
    isa = nc.isa
    sn = 'NEURON_ISA_TPB_PSEUDO_LIBRARY_RELOAD_INDEX_STRUCT'
    e = isa.get_enum("NEURON_ISA_TPB_PSEUDO_OPCODE")
    val = e.NEURON_ISA_TPB_PSEUDO_OPCODE_PSEUDO_LIBRARY_RELOAD_INDEX.value
    for fn in nc.m.functions:
        for bb in fn.blocks:
            for ins in bb.instructions:
                if type(ins).__name__ == 'InstPseudoReloadLibraryIndex' and not ins.instr:
                    b = isa.asm({"header": {"opcode": 223, "inst_word_len": 16},
                                 "pseudo_opcode": val,
                                 "lib_index": ins.lib_index}, sn)
                    ins.instr = [int(x) for x in b]




WIN = 128                  # dst nodes per window
SPLIT = 32768              # int16 positive limit for gather indices


def preprocess(edge_index, n_nodes, ncores=8):
    src = np.asarray(edge_index[0], dtype=np.int64)
    dst = np.asarray(edge_index[1], dtype=np.int64)
    npc = n_nodes // ncores
    nwin = (npc + WIN - 1) // WIN

    order = np.argsort(dst, kind="stable")
    src_s = src[order]
    dst_s = dst[order]

    counts = np.bincount(dst_s // npc, minlength=ncores)
    core_slices = np.concatenate([[0], np.cumsum(counts)])

    nlow = np.zeros((ncores, nwin), dtype=np.int64)
    nhigh = np.zeros((ncores, nwin), dtype=np.int64)
    per_core_win_edges = []
    for c in range(ncores):
        s0, s1 = core_slices[c], core_slices[c + 1]
        csrc = src_s[s0:s1]
        cdst = dst_s[s0:s1]
        wloc = (cdst - c * npc) // WIN
        dloc = (cdst - c * npc) % WIN
        wins = []
        for w in range(nwin):
            m = wloc == w
            ws, wd = csrc[m], dloc[m]
            lo = ws < SPLIT
            wins.append((ws[lo], ws[~lo] - SPLIT, wd[lo], wd[~lo]))
            nlow[c, w] = lo.sum()
            nhigh[c, w] = (~lo).sum()
        per_core_win_edges.append(wins)

    nbw_low = ((nlow.max(axis=0) + 127) // 128).astype(int)
    nbw_high = ((nhigh.max(axis=0) + 127) // 128).astype(int)
    for w in range(nwin):
        if nbw_low[w] + nbw_high[w] == 0:
            nbw_low[w] = 1
    NB = int(nbw_low.sum() + nbw_high.sum())

    gidx_lin = np.zeros((ncores, NB * 128), dtype=np.int16)
    srcidx_lin = np.zeros((ncores, NB * 128), dtype=np.int32)
    dstidx_lin = np.zeros((ncores, NB * 128), dtype=np.int32)
    dstloc_lin = np.full((ncores, NB * 128), -1, dtype=np.int16)

    for c in range(ncores):
        b0 = 0
        for w in range(nwin):
            slo, shi, dlo, dhi = per_core_win_edges[c][w]
            o = b0 * 128
            gidx_lin[c, o:o + len(slo)] = slo.astype(np.int16)
            srcidx_lin[c, o:o + len(slo)] = slo
            dstidx_lin[c, o:o + len(dlo)] = dlo + w * WIN + c * npc
            dstloc_lin[c, o:o + len(dlo)] = dlo.astype(np.int16)
            b0 += int(nbw_low[w])
            o = b0 * 128
            gidx_lin[c, o:o + len(shi)] = shi.astype(np.int16)
            srcidx_lin[c, o:o + len(shi)] = shi + SPLIT
            dstidx_lin[c, o:o + len(dhi)] = dhi + w * WIN + c * npc
            dstloc_lin[c, o:o + len(dhi)] = dhi.astype(np.int16)
            b0 += int(nbw_high[w])
        assert b0 == NB

    def wrap16(lin):  # [NC, NB*128] -> [NC, 128, NB*8] dma_gather layout
        x = lin.reshape(ncores, NB * 8, 16).transpose(0, 2, 1)
        return np.ascontiguousarray(np.tile(x, (1, 8, 1)))

    # dst-local indices for the a_dst gather (per-core local table, < 32768)
    adidx_lin = np.empty((ncores, NB * 128), dtype=np.int16)
    for c in range(ncores):
        loc = dstidx_lin[c].astype(np.int64) - c * npc
        loc[dstloc_lin[c] < 0] = 0          # pad slots -> row 0
        adidx_lin[c] = loc.astype(np.int16)

    def slotlay(lin, dtype):  # [NC, NB*128] -> [NC, 128, NB] ([p,b] = slot b*128+p)
        return np.ascontiguousarray(lin.reshape(ncores, NB, 128).transpose(0, 2, 1)).astype(dtype)

    return dict(
        NB=NB, nwin=nwin, npc=npc, ncores=ncores,
        nbw_low=nbw_low, nbw_high=nbw_high,
        gidx=wrap16(gidx_lin),
        adidx=wrap16(adidx_lin),
        srcidx=slotlay(srcidx_lin, np.int32),
        dstidx=slotlay(dstidx_lin, np.int32),
        dstloc=slotlay(dstloc_lin, np.int16),
    )




F32 = mybir.dt.float32
BF16 = mybir.dt.bfloat16
I16 = mybir.dt.int16
U16 = mybir.dt.uint16
AF = mybir.ActivationFunctionType
OP = mybir.AluOpType

SPLIT = 32768
GCHUNK = 8   # blocks per dma_gather call (1024 idx: single-packet-safe)


def chunked_gather(nc, out_tile, in_ap, idx_sb, b0, nblk, elem, regs, boff=0):
    """Issue dma_gather in <=GCHUNK-block chunks writing out_tile[:, boff+i...]."""
    done = 0
    while done < nblk:
        step = min(GCHUNK, nblk - done)
        n = step * 128
        if n not in regs:
            regs[n] = nc.gpsimd.to_reg(n)
        nc.gpsimd.dma_gather(
            out_tile[:, boff + done:boff + done + step, :], in_ap,
            idx_sb[:, (b0 + done) * 8:(b0 + done + step) * 8],
            n, regs[n], elem)
        done += step


def build(pp, N, F_IN=128, HID=64, HEADS=4, OUT=2, neg_slope=0.2, stages='ABCDE', clevel=9, for_sim=False):
    NB = pp["NB"]
    NWIN = pp["nwin"]
    NPC = pp["npc"]
    HC1 = HEADS * HID          # 256
    HC2 = HEADS * OUT          # 8
    NBWmax = int(max(pp["nbw_low"][w] + pp["nbw_high"][w] for w in range(NWIN)))
    NCHUNK = (N + 127) // 128
    T1C = HC1 + 128            # 384 bf16 cols = 768B rows
    L1COL = HC1 + 2 * HEADS    # 264
    K1COL = HID + HEADS        # 68  (skip + W_ad fold)
    W2COL = HC2 + 2 * HEADS + OUT  # 18
    R2COL = HC2 + HEADS        # 12
    T2C = 128                  # u16 cols = 256B rows

    nc = bass.Bass("TRN2", target_bir_lowering=False, debug=False, num_devices=8)

    # ---- I/O ----
    xT = nc.dram_tensor("xT", [F_IN, N], F32, kind="ExternalInput")
    xTown = nc.dram_tensor("xTown", [F_IN, NPC], F32, kind="ExternalInput")
    W1s_d = nc.dram_tensor("W1s", [F_IN, HC1], F32, kind="ExternalInput")
    W1d_d = nc.dram_tensor("W1d", [F_IN, HC1], F32, kind="ExternalInput")
    a1s_d = nc.dram_tensor("a1s", [128, HC1], F32, kind="ExternalInput")
    a1d_d = nc.dram_tensor("a1d", [128, HC1], F32, kind="ExternalInput")
    Wl1_d = nc.dram_tensor("Wl1", [F_IN, HID], F32, kind="ExternalInput")
    b1_d = nc.dram_tensor("b1", [128, HID], F32, kind="ExternalInput")
    bl1_d = nc.dram_tensor("bl1", [128, HID], F32, kind="ExternalInput")
    W2s_d = nc.dram_tensor("W2s", [HID, HC2], F32, kind="ExternalInput")
    W2d_d = nc.dram_tensor("W2d", [HID, HC2], F32, kind="ExternalInput")
    a2s_d = nc.dram_tensor("a2s", [128, HC2], F32, kind="ExternalInput")
    a2d_d = nc.dram_tensor("a2d", [128, HC2], F32, kind="ExternalInput")
    Wl2_d = nc.dram_tensor("Wl2", [HID, OUT], F32, kind="ExternalInput")
    b2_d = nc.dram_tensor("b2", [128, OUT], F32, kind="ExternalInput")
    bl2_d = nc.dram_tensor("bl2", [128, OUT], F32, kind="ExternalInput")
    gidx_d = nc.dram_tensor("gidx", [128, NB * 8], I16, kind="ExternalInput")
    dlidx_d = nc.dram_tensor("dlidx", [128, NB * 8], I16, kind="ExternalInput")
    dstloc_d = nc.dram_tensor("dstloc", [128, NB], I16, kind="ExternalInput")
    out_d = nc.dram_tensor("out", [NPC, OUT], F32, kind="ExternalOutput")

    # internal DRAM
    table1 = nc.dram_tensor("table1", [N, T1C], U16)
    adtab1 = nc.dram_tensor("adtab1", [NWIN * 128, 128], U16)   # 256B rows
    t2local = nc.dram_tensor("t2local", [NPC, T2C], U16)
    ad2tab = nc.dram_tensor("ad2tab", [NWIN * 128, T2C], U16)
    table2 = nc.dram_tensor("table2", [N, T2C], U16, addr_space="Shared")

    with tile.TileContext(nc) as tc:
        with tc.tile_pool(name="const", bufs=1) as cpool, \
             tc.tile_pool(name="resident", bufs=1) as rpool:

            # ---- constants / weights prep ----
            W1aug = cpool.tile([F_IN, L1COL], F32)
            nc.sync.dma_start(out=W1aug[:, 0:HC1], in_=W1s_d[:, :])
            wtmp = cpool.tile([F_IN, HC1], F32, tag="wtmp")
            atile = cpool.tile([128, HC1], F32, tag="atile")
            nc.sync.dma_start(out=atile[:, :], in_=a1s_d[:, :])
            nc.vector.tensor_tensor(out=wtmp[:, :], in0=W1aug[:, 0:HC1],
                                    in1=atile[:, :], op=OP.mult)
            nc.vector.tensor_reduce(out=W1aug[:, HC1:HC1 + HEADS],
                                    in_=wtmp[:, :].rearrange("p (h c) -> p h c", h=HEADS),
                                    axis=mybir.AxisListType.X, op=OP.add)
            wtmp2 = cpool.tile([F_IN, HC1], F32, tag="wtmp2")
            atile2 = cpool.tile([128, HC1], F32, tag="atile2")
            nc.sync.dma_start(out=wtmp2[:, :], in_=W1d_d[:, :])
            nc.sync.dma_start(out=atile2[:, :], in_=a1d_d[:, :])
            nc.vector.tensor_tensor(out=wtmp2[:, :], in0=wtmp2[:, :],
                                    in1=atile2[:, :], op=OP.mult)
            nc.vector.tensor_reduce(out=W1aug[:, HC1 + HEADS:L1COL],
                                    in_=wtmp2[:, :].rearrange("p (h c) -> p h c", h=HEADS),
                                    axis=mybir.AxisListType.X, op=OP.add)

            # skip weights + a_dst fold for own nodes
            Wl1aug = cpool.tile([F_IN, K1COL], F32)
            nc.sync.dma_start(out=Wl1aug[:, 0:HID], in_=Wl1_d[:, :])
            nc.vector.tensor_copy(Wl1aug[:, HID:K1COL], W1aug[:, HC1 + HEADS:L1COL])

            W2aug = cpool.tile([HID, W2COL], F32)
            nc.sync.dma_start(out=W2aug[:, 0:HC2], in_=W2s_d[:, :])
            nc.sync.dma_start(out=W2aug[:, HC2 + 2 * HEADS:W2COL], in_=Wl2_d[:, :])
            w2tmp = cpool.tile([HID, HC2], F32, tag="w2tmp")
            a2tile = cpool.tile([128, HC2], F32, tag="a2tile")
            nc.sync.dma_start(out=a2tile[:, :], in_=a2s_d[:, :])
            nc.vector.tensor_tensor(out=w2tmp[:, :], in0=W2aug[:, 0:HC2],
                                    in1=a2tile[0:HID, :], op=OP.mult)
            nc.vector.tensor_reduce(out=W2aug[:, HC2:HC2 + HEADS],
                                    in_=w2tmp[:, :].rearrange("p (h c) -> p h c", h=HEADS),
                                    axis=mybir.AxisListType.X, op=OP.add)
            w2tmp2 = cpool.tile([HID, HC2], F32, tag="w2tmp2")
            a2tile2 = cpool.tile([128, HC2], F32, tag="a2tile2")
            nc.sync.dma_start(out=w2tmp2[:, :], in_=W2d_d[:, :])
            nc.sync.dma_start(out=a2tile2[:, :], in_=a2d_d[:, :])
            nc.vector.tensor_tensor(out=w2tmp2[:, :], in0=w2tmp2[:, :],
                                    in1=a2tile2[0:HID, :], op=OP.mult)
            nc.vector.tensor_reduce(out=W2aug[:, HC2 + HEADS:HC2 + 2 * HEADS],
                                    in_=w2tmp2[:, :].rearrange("p (h c) -> p h c", h=HEADS),
                                    axis=mybir.AxisListType.X, op=OP.add)

            bias1 = cpool.tile([128, HID], F32)
            nc.sync.dma_start(out=bias1[:, :], in_=b1_d[:, :])
            btmp = cpool.tile([128, HID], F32, tag="btmp")
            nc.sync.dma_start(out=btmp[:, :], in_=bl1_d[:, :])
            nc.vector.tensor_tensor(out=bias1[:, :], in0=bias1[:, :], in1=btmp[:, :], op=OP.add)
            bias2 = cpool.tile([128, OUT], F32)
            nc.sync.dma_start(out=bias2[:, :], in_=b2_d[:, :])
            btmp2 = cpool.tile([128, OUT], F32, tag="btmp2")
            nc.sync.dma_start(out=btmp2[:, :], in_=bl2_d[:, :])
            nc.vector.tensor_tensor(out=bias2[:, :], in0=bias2[:, :], in1=btmp2[:, :], op=OP.add)

            iota_t = cpool.tile([128, 128], I16)
            nc.gpsimd.iota(iota_t[:, :], pattern=[[1, 128]], base=0, channel_multiplier=0)
            ident = cpool.tile([128, 128], F32)
            make_identity(nc, ident[:, :])

            gidx_sb = rpool.tile([128, NB * 8], I16)
            nc.sync.dma_start(out=gidx_sb[:, :], in_=gidx_d[:, :])
            dlidx_sb = rpool.tile([128, NB * 8], I16)
            nc.sync.dma_start(out=dlidx_sb[:, :], in_=dlidx_d[:, :])
            dstloc_sb = rpool.tile([128, NB], I16)
            nc.sync.dma_start(out=dstloc_sb[:, :], in_=dstloc_d[:, :])

            # all standard-library gpsimd ops (iota/affine_select/memset) are
            # above; from here on the Q7 carveout holds the mlp library.
            nc.gpsimd.load_library(library_config.mlp)
            gregs = {}

            if 'C' in stages and clevel >= 5:
                hT = rpool.tile([HID, NWIN, 128], F32)
                skip2sb = rpool.tile([128, NWIN, OUT], F32)
                outsb = rpool.tile([128, NWIN, OUT], F32)

            # ---- stage B ----
            NCHUNK_ = NCHUNK if 'B' in stages else 0
            with tc.tile_pool(name="projps", bufs=2, space="PSUM") as ppp, \
                 tc.tile_pool(name="projsb", bufs=3) as psb:
                for i in range(NCHUNK_):
                    o = i * 128
                    cn = min(128, N - o)
                    xb = psb.tile([F_IN, 128], F32, tag="xb")
                    nc.sync.dma_start(out=xb[:, 0:cn], in_=xT[:, o:o + cn])
                    ps = ppp.tile([128, L1COL], F32, space="PSUM")
                    nc.tensor.matmul(out=ps[0:cn, :], lhsT=xb[:, 0:cn], rhs=W1aug[:, :],
                                     start=True, stop=True)
                    st = psb.tile([128, L1COL], F32, tag="st")
                    nc.any.tensor_copy(st[0:cn, :], ps[0:cn, :])
                    nc.gpsimd.dma_start(out=table1[o:o + cn, 0:HC1].bitcast(BF16), in_=st[0:cn, 0:HC1])
                    nc.sync.dma_start(out=table1[o:o + cn, HC1:HC1 + 16],
                                      in_=st[0:cn, HC1:L1COL].bitcast(U16))

            # ---- stage C ----
            NWIN_C = NWIN if 'C' in stages else 0
            with tc.tile_pool(name="winps", bufs=2, space="PSUM") as wps, \
                 tc.tile_pool(name="skps", bufs=2, space="PSUM") as kps, \
                 tc.tile_pool(name="trps", bufs=2, space="PSUM") as tps, \
                 tc.tile_pool(name="winsb", bufs=2) as wsb:
                b0 = 0
                for w in range(NWIN_C):
                    BL = int(pp["nbw_low"][w])
                    BH = int(pp["nbw_high"][w])
                    nb = BL + BH
                    cn_w = min(128, NPC - w * 128)
                    # skip matmul + a_dst of own nodes -> adtab1 window rows
                    xo = wsb.tile([F_IN, 128], F32, tag="xo")
                    nc.sync.dma_start(out=xo[:, 0:cn_w], in_=xTown[:, w * 128:w * 128 + cn_w])
                    psK = kps.tile([128, K1COL], F32, space="PSUM")
                    nc.tensor.matmul(out=psK[0:cn_w, :], lhsT=xo[:, 0:cn_w], rhs=Wl1aug[:, :],
                                     start=True, stop=True)
                    stK = wsb.tile([128, HEADS], F32, tag="stK")
                    nc.vector.tensor_copy(stK[0:cn_w, :], psK[0:cn_w, HID:K1COL])
                    nc.sync.dma_start(out=adtab1[w * 128:w * 128 + cn_w, 0:8],
                                      in_=stK[0:cn_w, :].bitcast(U16))
                    # gathers
                    G = wsb.tile([128, NBWmax, T1C], U16, tag="G")
                    if BL:
                        chunked_gather(nc, G, table1[0:min(SPLIT, N), :],
                                       gidx_sb, b0, BL, T1C, gregs)
                    if BH:
                        chunked_gather(nc, G, table1[SPLIT:N, :],
                                       gidx_sb, b0 + BL, BH, T1C, gregs, boff=BL)
                    AD = wsb.tile([128, NBWmax, 128], U16, tag="AD")
                    chunked_gather(nc, AD, adtab1[w * 128:w * 128 + 128, :],
                                   dlidx_sb, b0, nb, 128, gregs)
                    # ex = exp(lrelu(as + ad))
                    if clevel < 2:
                        b0 += nb
                        continue
                    ex = wsb.tile([128, NBWmax, HEADS], F32, tag="ex")
                    nc.vector.tensor_tensor(out=ex[:, 0:nb, :],
                                            in0=G[:, 0:nb, HC1:HC1 + 8].bitcast(F32),
                                            in1=AD[:, 0:nb, 0:8].bitcast(F32), op=OP.add)
                    lrt = wsb.tile([128, NBWmax, HEADS], F32, tag="lrt")
                    nc.vector.tensor_scalar_mul(lrt[:, 0:nb, :], ex[:, 0:nb, :], neg_slope)
                    nc.vector.tensor_tensor(out=ex[:, 0:nb, :], in0=ex[:, 0:nb, :],
                                            in1=lrt[:, 0:nb, :], op=OP.max)
                    nc.scalar.activation(out=ex[:, 0:nb, :], in_=ex[:, 0:nb, :], func=AF.Exp)
                    exb = wsb.tile([128, NBWmax, HEADS], BF16, tag="exb")
                    nc.vector.tensor_copy(exb[:, 0:nb, :], ex[:, 0:nb, :])
                    if clevel < 3:
                        b0 += nb
                        continue
                    Gp = wsb.tile([128, NBWmax, HC1 + HEADS], BF16, tag="Gp")
                    nc.vector.tensor_tensor(
                        out=Gp[:, 0:nb, 0:HC1].rearrange("p b (h c) -> p b h c", h=HEADS),
                        in0=G[:, 0:nb, 0:HC1].bitcast(BF16).rearrange("p b (h c) -> p b h c", h=HEADS),
                        in1=exb[:, 0:nb, :].unsqueeze(3).to_broadcast([128, nb, HEADS, HID]),
                        op=OP.mult)
                    nc.vector.tensor_copy(Gp[:, 0:nb, HC1:HC1 + HEADS], exb[:, 0:nb, :])
                    S = wsb.tile([128, NBWmax, 128], BF16, tag="S")
                    nc.vector.tensor_tensor(
                        out=S[:, 0:nb, :],
                        in0=dstloc_sb[:, b0:b0 + nb].unsqueeze(2).to_broadcast([128, nb, 128]),
                        in1=iota_t[:, :].unsqueeze(1).to_broadcast([128, nb, 128]),
                        op=OP.is_equal)
                    psW = wps.tile([128, HC1 + HEADS], F32, space="PSUM")
                    for j in range(nb):
                        nc.tensor.matmul(out=psW[:, :], lhsT=S[:, j, :], rhs=Gp[:, j, :],
                                         start=(j == 0), stop=(j == nb - 1))
                    if clevel < 4:
                        b0 += nb
                        continue
                    # extract
                    rec = wsb.tile([128, HEADS], F32, tag="rec")
                    nc.vector.tensor_scalar(out=rec[:, :], in0=psW[:, HC1:HC1 + HEADS],
                                            scalar1=1e-16, scalar2=None, op0=OP.add)
                    nc.vector.reciprocal(rec[:, :], rec[:, :])
                    nc.vector.tensor_scalar_mul(rec[:, :], rec[:, :], 1.0 / HEADS)
                    gat = wsb.tile([128, HC1], F32, tag="gat")
                    nc.vector.tensor_tensor(
                        out=gat[:, :].rearrange("p (h c) -> p h c", h=HEADS),
                        in0=psW[:, 0:HC1].rearrange("p (h c) -> p h c", h=HEADS),
                        in1=rec[:, :].unsqueeze(2).to_broadcast([128, HEADS, HID]),
                        op=OP.mult)
                    hred = wsb.tile([128, HID], F32, tag="hred")
                    nc.vector.tensor_reduce(
                        out=hred[:, :],
                        in_=gat[:, :].rearrange("p (h c) -> p c h", h=HEADS),
                        axis=mybir.AxisListType.X, op=OP.add)
                    nc.vector.tensor_tensor(out=hred[:, :], in0=hred[:, :],
                                            in1=psK[:, 0:HID], op=OP.add)
                    nc.vector.tensor_tensor(out=hred[:, :], in0=hred[:, :],
                                            in1=bias1[:, :], op=OP.add)
                    if clevel < 5:
                        b0 += nb
                        continue
                    hwin = wsb.tile([128, HID], F32, tag="hwin")
                    nc.scalar.activation(out=hwin[:, :], in_=hred[:, :], func=AF.Sigmoid)
                    psT = tps.tile([HID, 128], F32, space="PSUM")
                    nc.tensor.transpose(out=psT[:, :], in_=hwin[:, :], identity=ident[:, :])
                    nc.vector.tensor_copy(hT[:, w, :], psT[:, :])
                    b0 += nb

            # ---- stage D ----
            NWIN_D = NWIN if 'D' in stages else 0
            with tc.tile_pool(name="l2ps", bufs=2, space="PSUM") as lps, \
                 tc.tile_pool(name="l2sb", bufs=3) as lsb:
                for w in range(NWIN_D):
                    cn_w = min(128, NPC - w * 128)
                    psL = lps.tile([128, W2COL], F32, space="PSUM")
                    nc.tensor.matmul(out=psL[0:cn_w, :], lhsT=hT[:, w, 0:cn_w],
                                     rhs=W2aug[:, :], start=True, stop=True)
                    st2 = lsb.tile([128, W2COL], F32, tag="st2")
                    nc.any.tensor_copy(st2[0:cn_w, :], psL[0:cn_w, :])
                    nc.sync.dma_start(out=t2local[w * 128:w * 128 + cn_w, 0:2 * (HC2 + HEADS)],
                                      in_=st2[0:cn_w, 0:HC2 + HEADS].bitcast(U16))
                    nc.sync.dma_start(out=ad2tab[w * 128:w * 128 + cn_w, 0:2 * HEADS],
                                      in_=st2[0:cn_w, HC2 + HEADS:HC2 + 2 * HEADS].bitcast(U16))
                    nc.vector.tensor_copy(skip2sb[0:cn_w, w, :],
                                          st2[0:cn_w, HC2 + 2 * HEADS:W2COL])
            if 'D' in stages:
                nc.gpsimd.collective_compute(
                    "AllGather", OP.bypass, replica_groups=[list(range(8))],
                    ins=[t2local[:, :]], outs=[table2[:, :]])

            # ---- stage E ----
            NWIN_E = NWIN if 'E' in stages else 0
            with tc.tile_pool(name="w2ps", bufs=2, space="PSUM") as wps2, \
                 tc.tile_pool(name="w2sb", bufs=2) as w2sb:
                b0 = 0
                for w in range(NWIN_E):
                    BL = int(pp["nbw_low"][w])
                    BH = int(pp["nbw_high"][w])
                    nb = BL + BH
                    g2s = w2sb.tile([128, NBWmax, T2C], U16, tag="g2s")
                    if BL:
                        chunked_gather(nc, g2s, table2[0:min(SPLIT, N), :],
                                       gidx_sb, b0, BL, T2C, gregs)
                    if BH:
                        chunked_gather(nc, g2s, table2[SPLIT:N, :],
                                       gidx_sb, b0 + BL, BH, T2C, gregs, boff=BL)
                    g2d = w2sb.tile([128, NBWmax, T2C], U16, tag="g2d")
                    chunked_gather(nc, g2d, ad2tab[w * 128:w * 128 + 128, :],
                                   dlidx_sb, b0, nb, T2C, gregs)
                    ex2 = w2sb.tile([128, NBWmax, HEADS], F32, tag="ex2")
                    nc.vector.tensor_tensor(out=ex2[:, 0:nb, :],
                                            in0=g2s[:, 0:nb, 2 * HC2:2 * (HC2 + HEADS)].bitcast(F32),
                                            in1=g2d[:, 0:nb, 0:2 * HEADS].bitcast(F32), op=OP.add)
                    lrt2 = w2sb.tile([128, NBWmax, HEADS], F32, tag="lrt2")
                    nc.vector.tensor_scalar_mul(lrt2[:, 0:nb, :], ex2[:, 0:nb, :], neg_slope)
                    nc.vector.tensor_tensor(out=ex2[:, 0:nb, :], in0=ex2[:, 0:nb, :],
                                            in1=lrt2[:, 0:nb, :], op=OP.max)
                    nc.scalar.activation(out=ex2[:, 0:nb, :], in_=ex2[:, 0:nb, :], func=AF.Exp)
                    ex2b = w2sb.tile([128, NBWmax, HEADS], BF16, tag="ex2b")
                    nc.vector.tensor_copy(ex2b[:, 0:nb, :], ex2[:, 0:nb, :])
                    g2sb = w2sb.tile([128, NBWmax, HC2], BF16, tag="g2sb")
                    nc.vector.tensor_copy(g2sb[:, 0:nb, :], g2s[:, 0:nb, 0:2 * HC2].bitcast(F32))
                    R2 = w2sb.tile([128, NBWmax, R2COL], BF16, tag="R2")
                    nc.vector.tensor_tensor(
                        out=R2[:, 0:nb, 0:HC2].rearrange("p b (h c) -> p b h c", h=HEADS),
                        in0=g2sb[:, 0:nb, :].rearrange("p b (h c) -> p b h c", h=HEADS),
                        in1=ex2b[:, 0:nb, :].unsqueeze(3).to_broadcast([128, nb, HEADS, OUT]),
                        op=OP.mult)
                    nc.vector.tensor_copy(R2[:, 0:nb, HC2:R2COL], ex2b[:, 0:nb, :])
                    S2 = w2sb.tile([128, NBWmax, 128], BF16, tag="S2")
                    nc.vector.tensor_tensor(
                        out=S2[:, 0:nb, :],
                        in0=dstloc_sb[:, b0:b0 + nb].unsqueeze(2).to_broadcast([128, nb, 128]),
                        in1=iota_t[:, :].unsqueeze(1).to_broadcast([128, nb, 128]),
                        op=OP.is_equal)
                    psW2 = wps2.tile([128, R2COL], F32, space="PSUM")
                    for j in range(nb):
                        nc.tensor.matmul(out=psW2[:, :], lhsT=S2[:, j, :], rhs=R2[:, j, :],
                                         start=(j == 0), stop=(j == nb - 1))
                    rec2 = w2sb.tile([128, HEADS], F32, tag="rec2")
                    nc.vector.tensor_scalar(out=rec2[:, :], in0=psW2[:, HC2:R2COL],
                                            scalar1=1e-16, scalar2=None, op0=OP.add)
                    nc.vector.reciprocal(rec2[:, :], rec2[:, :])
                    nc.vector.tensor_scalar_mul(rec2[:, :], rec2[:, :], 1.0 / HEADS)
                    og = w2sb.tile([128, HC2], F32, tag="og")
                    nc.vector.tensor_tensor(
                        out=og[:, :].rearrange("p (h c) -> p h c", h=HEADS),
                        in0=psW2[:, 0:HC2].rearrange("p (h c) -> p h c", h=HEADS),
                        in1=rec2[:, :].unsqueeze(2).to_broadcast([128, HEADS, OUT]),
                        op=OP.mult)
                    ored = w2sb.tile([128, OUT], F32, tag="ored")
                    nc.vector.tensor_reduce(
                        out=ored[:, :],
                        in_=og[:, :].rearrange("p (h c) -> p c h", h=HEADS),
                        axis=mybir.AxisListType.X, op=OP.add)
                    nc.vector.tensor_tensor(out=ored[:, :], in0=ored[:, :],
                                            in1=skip2sb[:, w, :], op=OP.add)
                    nc.vector.tensor_tensor(out=outsb[:, w, :], in0=ored[:, :],
                                            in1=bias2[:, :], op=OP.add)
                    b0 += nb

            # ---- final output DMA ----
            wf = NPC // 128 if 'E' in stages else 0
            rem = NPC % 128 if 'E' in stages else 0
            if wf:
                nc.sync.dma_start(
                    out=out_d[0:wf * 128, :].rearrange("(w p) c -> p w c", p=128),
                    in_=outsb[:, 0:wf, :])
            if rem:
                nc.sync.dma_start(out=out_d[wf * 128:NPC, :], in_=outsb[0:rem, wf, :])

    fix_library_reloads(nc)
    if not for_sim:
        split_multi_waits(nc)
    return nc


def make_in_maps(pp, inputs, N, F_IN=128, HID=64, HEADS=4, OUT=2):
    NPC = pp["npc"]
    NB = pp["NB"]
    x = np.ascontiguousarray(np.asarray(inputs["x"], dtype=np.float32))
    xT = np.ascontiguousarray(x.T)
    f32 = lambda a, shp: np.ascontiguousarray(np.asarray(a, dtype=np.float32).reshape(shp))
    rep = lambda a, shp: np.tile(f32(a, shp), (128, 1))
    common = {
        "xT": xT,
        "W1s": f32(inputs["W1s"], (F_IN, HEADS * HID)),
        "W1d": f32(inputs["W1d"], (F_IN, HEADS * HID)),
        "a1s": rep(inputs["a1s"], (1, HEADS * HID)),
        "a1d": rep(inputs["a1d"], (1, HEADS * HID)),
        "Wl1": f32(inputs["Wl1"], (F_IN, HID)),
        "b1": rep(inputs["b1"], (1, HID)),
        "bl1": rep(inputs["bl1"], (1, HID)),
        "W2s": f32(inputs["W2s"], (HID, HEADS * OUT)),
        "W2d": f32(inputs["W2d"], (HID, HEADS * OUT)),
        "a2s": rep(inputs["a2s"], (1, HEADS * OUT)),
        "a2d": rep(inputs["a2d"], (1, HEADS * OUT)),
        "Wl2": f32(inputs["Wl2"], (HID, OUT)),
        "b2": rep(inputs["b2"], (1, OUT)),
        "bl2": rep(inputs["bl2"], (1, OUT)),
    }
    # dst-local gather indices (pads -> 0)
    dl = pp["dstloc"].copy()          # [NC, 128, NB]
    in_maps = []
    for c in range(8):
        m = dict(common)
        m["xTown"] = np.ascontiguousarray(xT[:, c * NPC:(c + 1) * NPC])
        m["gidx"] = pp["gidx"][c]
        m["dstloc"] = pp["dstloc"][c]
        lin = pp["dstloc"][c].T.reshape(-1).astype(np.int16)   # slot order
        lin = np.where(lin < 0, 0, lin)
        wrap = np.tile(lin.reshape(NB * 8, 16).T, (8, 1))
        m["dlidx"] = np.ascontiguousarray(wrap)
        in_maps.append(m)
    return in_maps


_BUILD_CACHE = {}
LAST_RESULTS = None


def kernel(**inputs):
    """Full inputs in, full [N, 2] float32 output out."""
    global LAST_RESULTS
    trace = bool(inputs.pop("_trace", False))
    pp = preprocess(inputs["edge_index"], N_NODES)
    key = (pp["NB"], tuple(pp["nbw_low"]), tuple(pp["nbw_high"]))
    if key not in _BUILD_CACHE:
        _BUILD_CACHE[key] = build(pp, N_NODES)
    nc = _BUILD_CACHE[key]
    in_maps = make_in_maps(pp, inputs, N_NODES)
    res = run_bass_kernel_spmd(nc, in_maps, list(range(8)), trace=trace)
    LAST_RESULTS = res
    out = np.concatenate([res.results[c]["out"] for c in range(8)], axis=0)
    return out.astype(np.float32)



# revision 2
# speedup vs baseline: 1.8244x; 1.8244x over previous
"""Self-contained Trainium2 Bass kernel for the 2-layer GAT problem
(nn_GAT_26714696581831). 8-core SPMD: edges sorted by dst, 8 dst-range
shards; per-window one-hot matmul aggregation with dma_gather row fetches
spread across 4 SWDGE queues (4 Q7 core pairs generate descriptors in
parallel).

kernel(**inputs) takes the FULL unsharded inputs and returns the FULL
[50000, 2] output.
"""
import sys
sys.path.insert(0, '/opt/trn_rl_repo')
import numpy as np
import concourse.bass as bass
import concourse.mybir as mybir
import concourse.tile as tile
from concourse import library_config
from concourse.masks import make_identity
from concourse.bass_utils import run_bass_kernel_spmd

N_NODES = 50000
"""Workarounds for this walrus build, which rejects any instruction carrying
more than one sync-wait command: hoist extra waits onto same-engine NoOps
inserted immediately before the instruction."""


_ctr = [0]

def split_multi_waits(nc, max_waits=1):
    for fn in nc.m.functions:
        for bb in fn.blocks:
            insts = bb.instructions
            i = 0
            while i < len(insts):
                ins = insts[i]
                si = ins.sync_info
                if si is not None and si.on_wait and len(si.on_wait) > max_waits:
                    waits = list(si.on_wait)
                    keep = waits[-max_waits:]
                    hoist = waits[:-max_waits]
                    si.on_wait = keep
                    for w in hoist:
                        _ctr[0] += 1
                        n = mybir.InstNoOp(name=f"waitsplit-{_ctr[0]}", ins=[], outs=[])
                        n.engine = ins.engine
                        n.sync_info = mybir.SyncInfo(on_wait=[w], on_update=[])
                        insts.insert(i, n)
                        i += 1
                i += 1


def fix_library_reloads(nc):
    """bass_rust leaves InstPseudoReloadLibraryIndex.instr empty; this walrus
    rejects zero-length ISA instructions. Encode the 64-byte
    PSEUDO_LIBRARY_RELOAD_INDEX struct with the live ISA tables."""
    isa = nc.isa
    sn = 'NEURON_ISA_TPB_PSEUDO_LIBRARY_RELOAD_INDEX_STRUCT'
    e = isa.get_enum("NEURON_ISA_TPB_PSEUDO_OPCODE")
    val = e.NEURON_ISA_TPB_PSEUDO_OPCODE_PSEUDO_LIBRARY_RELOAD_INDEX.value
    for fn in nc.m.functions:
        for bb in fn.blocks:
            for ins in bb.instructions:
                if type(ins).__name__ == 'InstPseudoReloadLibraryIndex' and not ins.instr:
                    b = isa.asm({"header": {"opcode": 223, "inst_word_len": 16},
                                 "pseudo_opcode": val,
                                 "lib_index": ins.lib_index}, sn)
                    ins.instr = [int(x) for x in b]




WIN = 128                  # dst nodes per window
SPLIT = 32768              # int16 positive limit for gather indices


def preprocess(edge_index, n_nodes, ncores=8):
    src = np.asarray(edge_index[0], dtype=np.int64)
    dst = np.asarray(edge_index[1], dtype=np.int64)
    npc = n_nodes // ncores
    nwin = (npc + WIN - 1) // WIN

    order = np.argsort(dst, kind="stable")
    src_s = src[order]
    dst_s = dst[order]

    counts = np.bincount(dst_s // npc, minlength=ncores)
    core_slices = np.concatenate([[0], np.cumsum(counts)])

    nlow = np.zeros((ncores, nwin), dtype=np.int64)
    nhigh = np.zeros((ncores, nwin), dtype=np.int64)
    per_core_win_edges = []
    for c in range(ncores):
        s0, s1 = core_slices[c], core_slices[c + 1]
        csrc = src_s[s0:s1]
        cdst = dst_s[s0:s1]
        wloc = (cdst - c * npc) // WIN
        dloc = (cdst - c * npc) % WIN
        wins = []
        for w in range(nwin):
            m = wloc == w
            ws, wd = csrc[m], dloc[m]
            lo = ws < SPLIT
            wins.append((ws[lo], ws[~lo] - SPLIT, wd[lo], wd[~lo]))
            nlow[c, w] = lo.sum()
            nhigh[c, w] = (~lo).sum()
        per_core_win_edges.append(wins)

    nbw_low = ((nlow.max(axis=0) + 127) // 128).astype(int)
    nbw_high = ((nhigh.max(axis=0) + 127) // 128).astype(int)
    for w in range(nwin):
        if nbw_low[w] + nbw_high[w] == 0:
            nbw_low[w] = 1
    NB = int(nbw_low.sum() + nbw_high.sum())

    gidx_lin = np.zeros((ncores, NB * 128), dtype=np.int16)
    srcidx_lin = np.zeros((ncores, NB * 128), dtype=np.int32)
    dstidx_lin = np.zeros((ncores, NB * 128), dtype=np.int32)
    dstloc_lin = np.full((ncores, NB * 128), -1, dtype=np.int16)

    for c in range(ncores):
        b0 = 0
        for w in range(nwin):
            slo, shi, dlo, dhi = per_core_win_edges[c][w]
            o = b0 * 128
            gidx_lin[c, o:o + len(slo)] = slo.astype(np.int16)
            srcidx_lin[c, o:o + len(slo)] = slo
            dstidx_lin[c, o:o + len(dlo)] = dlo + w * WIN + c * npc
            dstloc_lin[c, o:o + len(dlo)] = dlo.astype(np.int16)
            b0 += int(nbw_low[w])
            o = b0 * 128
            gidx_lin[c, o:o + len(shi)] = shi.astype(np.int16)
            srcidx_lin[c, o:o + len(shi)] = shi + SPLIT
            dstidx_lin[c, o:o + len(dhi)] = dhi + w * WIN + c * npc
            dstloc_lin[c, o:o + len(dhi)] = dhi.astype(np.int16)
            b0 += int(nbw_high[w])
        assert b0 == NB

    def wrap16(lin):  # [NC, NB*128] -> [NC, 128, NB*8] dma_gather layout
        x = lin.reshape(ncores, NB * 8, 16).transpose(0, 2, 1)
        return np.ascontiguousarray(np.tile(x, (1, 8, 1)))

    # dst-local indices for the a_dst gather (per-core local table, < 32768)
    adidx_lin = np.empty((ncores, NB * 128), dtype=np.int16)
    for c in range(ncores):
        loc = dstidx_lin[c].astype(np.int64) - c * npc
        loc[dstloc_lin[c] < 0] = 0          # pad slots -> row 0
        adidx_lin[c] = loc.astype(np.int16)

    def slotlay(lin, dtype):  # [NC, NB*128] -> [NC, 128, NB] ([p,b] = slot b*128+p)
        return np.ascontiguousarray(lin.reshape(ncores, NB, 128).transpose(0, 2, 1)).astype(dtype)

    return dict(
        NB=NB, nwin=nwin, npc=npc, ncores=ncores,
        nbw_low=nbw_low, nbw_high=nbw_high,
        gidx=wrap16(gidx_lin),
        adidx=wrap16(adidx_lin),
        srcidx=slotlay(srcidx_lin, np.int32),
        dstidx=slotlay(dstidx_lin, np.int32),
        dstloc=slotlay(dstloc_lin, np.int16),
    )




F32 = mybir.dt.float32
BF16 = mybir.dt.bfloat16
I16 = mybir.dt.int16
U16 = mybir.dt.uint16
AF = mybir.ActivationFunctionType
OP = mybir.AluOpType

SPLIT = 32768
GCHUNK = 8   # blocks per dma_gather call (1024 idx: single-packet-safe)
NQ = 4       # SWDGE queues: queue q's descriptors are generated by Q7 core
             # pair (2q, 2q+1); round-robin spreads desc-gen over all 8 cores


def chunked_gather(nc, out_tile, in_ap, idx_sb, b0, nblk, elem, regs, qrr, boff=0):
    """Issue dma_gather in <=GCHUNK-block chunks writing out_tile[:, boff+i...]."""
    done = 0
    while done < nblk:
        step = min(GCHUNK, nblk - done)
        n = step * 128
        if n not in regs:
            regs[n] = nc.gpsimd.to_reg(n)
        nc.gpsimd.dma_gather(
            out_tile[:, boff + done:boff + done + step, :], in_ap,
            idx_sb[:, (b0 + done) * 8:(b0 + done + step) * 8],
            n, regs[n], elem, queue_num=qrr[0] % NQ)
        qrr[0] += 1
        done += step


def build(pp, N, F_IN=128, HID=64, HEADS=4, OUT=2, neg_slope=0.2, stages='ABCDE'):
    NB = pp["NB"]
    NWIN = pp["nwin"]
    NPC = pp["npc"]
    HC1 = HEADS * HID          # 256
    HC2 = HEADS * OUT          # 8
    NBWmax = int(max(pp["nbw_low"][w] + pp["nbw_high"][w] for w in range(NWIN)))
    NCHUNK = (N + 127) // 128
    T1C = HC1 + 128            # 384 u16 cols = 768B rows
    L1COL = HC1 + HEADS        # 260: xs + a_src fold
    K1COL = HID + HEADS        # 68  (skip + W_ad fold)
    W2COL = HC2 + 2 * HEADS + OUT  # 18
    R2COL = HC2 + HEADS        # 12
    T2C = 128                  # u16 cols = 256B rows
    BBATCH = 4                 # stage-B chunks per DMA batch

    nc = bass.Bass("TRN2", target_bir_lowering=False, debug=False,
                   num_devices=8, num_swdge_queues=NQ)

    # ---- I/O ----
    xT = nc.dram_tensor("xT", [F_IN, N], F32, kind="ExternalInput")
    xTown = nc.dram_tensor("xTown", [F_IN, NPC], F32, kind="ExternalInput")
    W1aug_d = nc.dram_tensor("W1aug", [F_IN, L1COL], F32, kind="ExternalInput")
    Wl1aug_d = nc.dram_tensor("Wl1aug", [F_IN, K1COL], F32, kind="ExternalInput")
    W2aug_d = nc.dram_tensor("W2aug", [HID, W2COL], F32, kind="ExternalInput")
    bias1_d = nc.dram_tensor("bias1", [128, HID], F32, kind="ExternalInput")
    bias2_d = nc.dram_tensor("bias2", [128, OUT], F32, kind="ExternalInput")
    gidx_d = nc.dram_tensor("gidx", [128, NB * 8], I16, kind="ExternalInput")
    dlidx_d = nc.dram_tensor("dlidx", [128, NB * 8], I16, kind="ExternalInput")
    dstloc_d = nc.dram_tensor("dstloc", [128, NB], I16, kind="ExternalInput")
    out_d = nc.dram_tensor("out", [NPC, OUT], F32, kind="ExternalOutput")

    # internal DRAM
    table1 = nc.dram_tensor("table1", [N, T1C], U16)
    adtab1 = nc.dram_tensor("adtab1", [NWIN * 128, 128], U16)   # 256B rows
    t2local = nc.dram_tensor("t2local", [NPC, T2C], U16)
    ad2tab = nc.dram_tensor("ad2tab", [NWIN * 128, T2C], U16)
    table2 = nc.dram_tensor("table2", [N, T2C], U16, addr_space="Shared")

    with tile.TileContext(nc) as tc:
        with tc.tile_pool(name="const", bufs=1) as cpool, \
             tc.tile_pool(name="resident", bufs=1) as rpool:

            # ---- constants (all weight folding/permutation done on host) ----
            W1aug = cpool.tile([F_IN, L1COL], F32)
            nc.sync.dma_start(out=W1aug[:, :], in_=W1aug_d[:, :])
            Wl1aug = cpool.tile([F_IN, K1COL], F32)
            nc.sync.dma_start(out=Wl1aug[:, :], in_=Wl1aug_d[:, :])
            W2aug = cpool.tile([HID, W2COL], F32)
            nc.sync.dma_start(out=W2aug[:, :], in_=W2aug_d[:, :])
            bias1 = cpool.tile([128, HID], F32)
            nc.sync.dma_start(out=bias1[:, :], in_=bias1_d[:, :])
            bias2 = cpool.tile([128, OUT], F32)
            nc.sync.dma_start(out=bias2[:, :], in_=bias2_d[:, :])

            iota_t = cpool.tile([128, 128], I16)
            nc.gpsimd.iota(iota_t[:, :], pattern=[[1, 128]], base=0, channel_multiplier=0)
            ident = cpool.tile([128, 128], F32)
            make_identity(nc, ident[:, :])

            gidx_sb = rpool.tile([128, NB * 8], I16)
            nc.sync.dma_start(out=gidx_sb[:, :], in_=gidx_d[:, :])
            dlidx_sb = rpool.tile([128, NB * 8], I16)
            nc.sync.dma_start(out=dlidx_sb[:, :], in_=dlidx_d[:, :])
            dstloc_sb = rpool.tile([128, NB], I16)
            nc.sync.dma_start(out=dstloc_sb[:, :], in_=dstloc_d[:, :])

            # all standard-library gpsimd ops (iota/affine_select/memset) are
            # above; from here on the Q7 carveout holds the mlp library.
            nc.gpsimd.load_library(library_config.mlp)
            gregs = {}
            qrr = [0]

            if 'C' in stages:
                hT = rpool.tile([HID, NWIN, 128], F32)
                skip2sb = rpool.tile([128, NWIN, OUT], F32)
                outsb = rpool.tile([128, NWIN, OUT], F32)

            # ---- stage B: project all N nodes, table1 = [xs bf16 | a_src f32] ----
            NBAT = (NCHUNK + BBATCH - 1) // BBATCH if 'B' in stages else 0
            with tc.tile_pool(name="projps", bufs=2, space="PSUM") as ppp, \
                 tc.tile_pool(name="projsb", bufs=3) as psb:
                for i in range(NBAT):
                    o = i * 128 * BBATCH
                    cb = min(128 * BBATCH, N - o)          # rows this batch
                    nch = (cb + 127) // 128
                    xb = psb.tile([F_IN, BBATCH * 128], F32, tag="xb")
                    nc.sync.dma_start(out=xb[:, 0:cb], in_=xT[:, o:o + cb])
                    stbf = psb.tile([128, BBATCH, HC1], BF16, tag="stbf")
                    sts = psb.tile([128, BBATCH, 2 * HEADS], U16, tag="sts")
                    for j in range(nch):
                        cn = min(128, cb - j * 128)
                        ps = ppp.tile([128, L1COL], F32, space="PSUM")
                        nc.tensor.matmul(out=ps[0:cn, :],
                                         lhsT=xb[:, j * 128:j * 128 + cn],
                                         rhs=W1aug[:, :], start=True, stop=True)
                        nc.vector.tensor_copy(stbf[0:cn, j, :], ps[0:cn, 0:HC1])
                        nc.vector.tensor_copy(sts[0:cn, j, :].bitcast(F32),
                                              ps[0:cn, HC1:L1COL])
                    if cb == BBATCH * 128:
                        nc.scalar.dma_start(
                            out=table1[o:o + cb, 0:HC1].bitcast(BF16)
                                .rearrange("(b p) c -> p b c", p=128),
                            in_=stbf[:, :, :])
                        nc.sync.dma_start(
                            out=table1[o:o + cb, HC1:HC1 + 2 * HEADS]
                                .rearrange("(b p) c -> p b c", p=128),
                            in_=sts[:, :, :])
                    else:
                        for j in range(nch):
                            cn = min(128, cb - j * 128)
                            oj = o + j * 128
                            nc.scalar.dma_start(
                                out=table1[oj:oj + cn, 0:HC1].bitcast(BF16),
                                in_=stbf[0:cn, j, :])
                            nc.sync.dma_start(
                                out=table1[oj:oj + cn, HC1:HC1 + 2 * HEADS],
                                in_=sts[0:cn, j, :])

            # ---- stage C: layer-1 edge softmax + aggregation per dst window ----
            NWIN_C = NWIN if 'C' in stages else 0
            with tc.tile_pool(name="winps", bufs=2, space="PSUM") as wps, \
                 tc.tile_pool(name="skps", bufs=2, space="PSUM") as kps, \
                 tc.tile_pool(name="trps", bufs=2, space="PSUM") as tps, \
                 tc.tile_pool(name="winsb", bufs=2) as wsb:
                b0 = 0
                for w in range(NWIN_C):
                    BL = int(pp["nbw_low"][w])
                    BH = int(pp["nbw_high"][w])
                    nb = BL + BH
                    cn_w = min(128, NPC - w * 128)
                    # skip matmul + a_dst of own nodes -> adtab1 window rows
                    xo = wsb.tile([F_IN, 128], F32, tag="xo")
                    nc.sync.dma_start(out=xo[:, 0:cn_w], in_=xTown[:, w * 128:w * 128 + cn_w])
                    psK = kps.tile([128, K1COL], F32, space="PSUM")
                    nc.tensor.matmul(out=psK[0:cn_w, :], lhsT=xo[:, 0:cn_w], rhs=Wl1aug[:, :],
                                     start=True, stop=True)
                    stK = wsb.tile([128, HEADS], F32, tag="stK")
                    nc.vector.tensor_copy(stK[0:cn_w, :], psK[0:cn_w, HID:K1COL])
                    nc.sync.dma_start(out=adtab1[w * 128:w * 128 + cn_w, 0:8],
                                      in_=stK[0:cn_w, :].bitcast(U16))
                    # gathers
                    G = wsb.tile([128, NBWmax, T1C], U16, tag="G")
                    if BL:
                        chunked_gather(nc, G, table1[0:min(SPLIT, N), :],
                                       gidx_sb, b0, BL, T1C, gregs, qrr)
                    if BH:
                        chunked_gather(nc, G, table1[SPLIT:N, :],
                                       gidx_sb, b0 + BL, BH, T1C, gregs, qrr, boff=BL)
                    AD = wsb.tile([128, NBWmax, 128], U16, tag="AD")
                    chunked_gather(nc, AD, adtab1[w * 128:w * 128 + 128, :],
                                   dlidx_sb, b0, nb, 128, gregs, qrr)
                    # e = a_src + a_dst; ex = exp(lrelu(e)) = max(exp(e), exp(0.2e))
                    ex = wsb.tile([128, NBWmax, HEADS], F32, tag="ex")
                    nc.vector.tensor_tensor(out=ex[:, 0:nb, :],
                                            in0=G[:, 0:nb, HC1:HC1 + 8].bitcast(F32),
                                            in1=AD[:, 0:nb, 0:8].bitcast(F32), op=OP.add)
                    exa = wsb.tile([128, NBWmax, HEADS], BF16, tag="exa")
                    nc.scalar.activation(out=exa[:, 0:nb, :], in_=ex[:, 0:nb, :],
                                         func=AF.Exp)
                    exb = wsb.tile([128, NBWmax, HEADS], BF16, tag="exb")
                    nc.scalar.activation(out=exb[:, 0:nb, :], in_=ex[:, 0:nb, :],
                                         func=AF.Exp, scale=float(neg_slope))
                    nc.vector.tensor_tensor(out=exb[:, 0:nb, :], in0=exa[:, 0:nb, :],
                                            in1=exb[:, 0:nb, :], op=OP.max)
                    Gp = wsb.tile([128, NBWmax, HC1 + HEADS], BF16, tag="Gp")
                    nc.vector.tensor_tensor(
                        out=Gp[:, 0:nb, 0:HC1].rearrange("p b (h c) -> p b h c", h=HEADS),
                        in0=G[:, 0:nb, 0:HC1].bitcast(BF16).rearrange("p b (h c) -> p b h c", h=HEADS),
                        in1=exb[:, 0:nb, :].unsqueeze(3).to_broadcast([128, nb, HEADS, HID]),
                        op=OP.mult)
                    nc.vector.tensor_copy(Gp[:, 0:nb, HC1:HC1 + HEADS], exb[:, 0:nb, :])
                    S = wsb.tile([128, NBWmax, 128], BF16, tag="S")
                    nc.vector.tensor_tensor(
                        out=S[:, 0:nb, :],
                        in0=dstloc_sb[:, b0:b0 + nb].unsqueeze(2).to_broadcast([128, nb, 128]),
                        in1=iota_t[:, :].unsqueeze(1).to_broadcast([128, nb, 128]),
                        op=OP.is_equal)
                    psW = wps.tile([128, HC1 + HEADS], F32, space="PSUM")
                    for j in range(nb):
                        nc.tensor.matmul(out=psW[:, :], lhsT=S[:, j, :], rhs=Gp[:, j, :],
                                         start=(j == 0), stop=(j == nb - 1))
                    # extract
                    rec = wsb.tile([128, HEADS], F32, tag="rec")
                    nc.vector.tensor_scalar(out=rec[:, :], in0=psW[:, HC1:HC1 + HEADS],
                                            scalar1=1e-16, scalar2=None, op0=OP.add)
                    nc.vector.reciprocal(rec[:, :], rec[:, :])
                    nc.vector.tensor_scalar_mul(rec[:, :], rec[:, :], 1.0 / HEADS)
                    gat = wsb.tile([128, HC1], F32, tag="gat")
                    nc.vector.tensor_tensor(
                        out=gat[:, :].rearrange("p (h c) -> p h c", h=HEADS),
                        in0=psW[:, 0:HC1].rearrange("p (h c) -> p h c", h=HEADS),
                        in1=rec[:, :].unsqueeze(2).to_broadcast([128, HEADS, HID]),
                        op=OP.mult)
                    hred = wsb.tile([128, HID], F32, tag="hred")
                    nc.vector.tensor_reduce(
                        out=hred[:, :],
                        in_=gat[:, :].rearrange("p (h c) -> p c h", h=HEADS),
                        axis=mybir.AxisListType.X, op=OP.add)
                    nc.vector.tensor_tensor(out=hred[:, :], in0=hred[:, :],
                                            in1=psK[:, 0:HID], op=OP.add)
                    nc.vector.tensor_tensor(out=hred[:, :], in0=hred[:, :],
                                            in1=bias1[:, :], op=OP.add)
                    hwin = wsb.tile([128, HID], F32, tag="hwin")
                    nc.scalar.activation(out=hwin[:, :], in_=hred[:, :], func=AF.Sigmoid)
                    psT = tps.tile([HID, 128], F32, space="PSUM")
                    nc.tensor.transpose(out=psT[:, :], in_=hwin[:, :], identity=ident[:, :])
                    nc.vector.tensor_copy(hT[:, w, :], psT[:, :])
                    b0 += nb

            # ---- stage D: layer-2 projections of own nodes; z2 stored bf16 ----
            NWIN_D = NWIN if 'D' in stages else 0
            with tc.tile_pool(name="l2ps", bufs=2, space="PSUM") as lps, \
                 tc.tile_pool(name="l2sb", bufs=3) as lsb:
                for w in range(NWIN_D):
                    cn_w = min(128, NPC - w * 128)
                    psL = lps.tile([128, W2COL], F32, space="PSUM")
                    nc.tensor.matmul(out=psL[0:cn_w, :], lhsT=hT[:, w, 0:cn_w],
                                     rhs=W2aug[:, :], start=True, stop=True)
                    # t2 row: [0:8]=z2 bf16, [8:16]=a_src2 f32
                    t2st = lsb.tile([128, 16], U16, tag="t2st")
                    nc.vector.tensor_copy(t2st[0:cn_w, 0:HC2].bitcast(BF16),
                                          psL[0:cn_w, 0:HC2])
                    nc.vector.tensor_copy(t2st[0:cn_w, HC2:16].bitcast(F32),
                                          psL[0:cn_w, HC2:HC2 + HEADS])
                    nc.sync.dma_start(out=t2local[w * 128:w * 128 + cn_w, 0:16],
                                      in_=t2st[0:cn_w, :])
                    stad = lsb.tile([128, HEADS], F32, tag="stad")
                    nc.vector.tensor_copy(stad[0:cn_w, :],
                                          psL[0:cn_w, HC2 + HEADS:HC2 + 2 * HEADS])
                    nc.sync.dma_start(out=ad2tab[w * 128:w * 128 + cn_w, 0:2 * HEADS],
                                      in_=stad[0:cn_w, :].bitcast(U16))
                    nc.vector.tensor_copy(skip2sb[0:cn_w, w, :],
                                          psL[0:cn_w, HC2 + 2 * HEADS:W2COL])
            if 'D' in stages:
                nc.gpsimd.collective_compute(
                    "AllGather", OP.bypass, replica_groups=[list(range(8))],
                    ins=[t2local[:, :]], outs=[table2[:, :]])

            # ---- stage E: layer-2 edge softmax + aggregation ----
            NWIN_E = NWIN if 'E' in stages else 0
            with tc.tile_pool(name="w2ps", bufs=2, space="PSUM") as wps2, \
                 tc.tile_pool(name="w2sb", bufs=2) as w2sb:
                b0 = 0
                for w in range(NWIN_E):
                    BL = int(pp["nbw_low"][w])
                    BH = int(pp["nbw_high"][w])
                    nb = BL + BH
                    g2s = w2sb.tile([128, NBWmax, T2C], U16, tag="g2s")
                    if BL:
                        chunked_gather(nc, g2s, table2[0:min(SPLIT, N), :],
                                       gidx_sb, b0, BL, T2C, gregs, qrr)
                    if BH:
                        chunked_gather(nc, g2s, table2[SPLIT:N, :],
                                       gidx_sb, b0 + BL, BH, T2C, gregs, qrr, boff=BL)
                    g2d = w2sb.tile([128, NBWmax, T2C], U16, tag="g2d")
                    chunked_gather(nc, g2d, ad2tab[w * 128:w * 128 + 128, :],
                                   dlidx_sb, b0, nb, T2C, gregs, qrr)
                    ex2 = w2sb.tile([128, NBWmax, HEADS], F32, tag="ex2")
                    nc.vector.tensor_tensor(out=ex2[:, 0:nb, :],
                                            in0=g2s[:, 0:nb, HC2:16].bitcast(F32),
                                            in1=g2d[:, 0:nb, 0:8].bitcast(F32), op=OP.add)
                    ex2a = w2sb.tile([128, NBWmax, HEADS], BF16, tag="ex2a")
                    nc.scalar.activation(out=ex2a[:, 0:nb, :], in_=ex2[:, 0:nb, :],
                                         func=AF.Exp)
                    ex2b = w2sb.tile([128, NBWmax, HEADS], BF16, tag="ex2b")
                    nc.scalar.activation(out=ex2b[:, 0:nb, :], in_=ex2[:, 0:nb, :],
                                         func=AF.Exp, scale=float(neg_slope))
                    nc.vector.tensor_tensor(out=ex2b[:, 0:nb, :], in0=ex2a[:, 0:nb, :],
                                            in1=ex2b[:, 0:nb, :], op=OP.max)
                    R2 = w2sb.tile([128, NBWmax, R2COL], BF16, tag="R2")
                    nc.vector.tensor_tensor(
                        out=R2[:, 0:nb, 0:HC2].rearrange("p b (h c) -> p b h c", h=HEADS),
                        in0=g2s[:, 0:nb, 0:HC2].bitcast(BF16).rearrange("p b (h c) -> p b h c", h=HEADS),
                        in1=ex2b[:, 0:nb, :].unsqueeze(3).to_broadcast([128, nb, HEADS, OUT]),
                        op=OP.mult)
                    nc.vector.tensor_copy(R2[:, 0:nb, HC2:R2COL], ex2b[:, 0:nb, :])
                    S2 = w2sb.tile([128, NBWmax, 128], BF16, tag="S2")
                    nc.vector.tensor_tensor(
                        out=S2[:, 0:nb, :],
                        in0=dstloc_sb[:, b0:b0 + nb].unsqueeze(2).to_broadcast([128, nb, 128]),
                        in1=iota_t[:, :].unsqueeze(1).to_broadcast([128, nb, 128]),
                        op=OP.is_equal)
                    psW2 = wps2.tile([128, R2COL], F32, space="PSUM")
                    for j in range(nb):
                        nc.tensor.matmul(out=psW2[:, :], lhsT=S2[:, j, :], rhs=R2[:, j, :],
                                         start=(j == 0), stop=(j == nb - 1))
                    rec2 = w2sb.tile([128, HEADS], F32, tag="rec2")
                    nc.vector.tensor_scalar(out=rec2[:, :], in0=psW2[:, HC2:R2COL],
                                            scalar1=1e-16, scalar2=None, op0=OP.add)
                    nc.vector.reciprocal(rec2[:, :], rec2[:, :])
                    nc.vector.tensor_scalar_mul(rec2[:, :], rec2[:, :], 1.0 / HEADS)
                    og = w2sb.tile([128, HC2], F32, tag="og")
                    nc.vector.tensor_tensor(
                        out=og[:, :].rearrange("p (h c) -> p h c", h=HEADS),
                        in0=psW2[:, 0:HC2].rearrange("p (h c) -> p h c", h=HEADS),
                        in1=rec2[:, :].unsqueeze(2).to_broadcast([128, HEADS, OUT]),
                        op=OP.mult)
                    ored = w2sb.tile([128, OUT], F32, tag="ored")
                    nc.vector.tensor_reduce(
                        out=ored[:, :],
                        in_=og[:, :].rearrange("p (h c) -> p c h", h=HEADS),
                        axis=mybir.AxisListType.X, op=OP.add)
                    nc.vector.tensor_tensor(out=ored[:, :], in0=ored[:, :],
                                            in1=skip2sb[:, w, :], op=OP.add)
                    nc.vector.tensor_tensor(out=outsb[:, w, :], in0=ored[:, :],
                                            in1=bias2[:, :], op=OP.add)
                    b0 += nb

            # ---- final output DMA ----
            wf = NPC // 128 if 'E' in stages else 0
            rem = NPC % 128 if 'E' in stages else 0
            if wf:
                nc.sync.dma_start(
                    out=out_d[0:wf * 128, :].rearrange("(w p) c -> p w c", p=128),
                    in_=outsb[:, 0:wf, :])
            if rem:
                nc.sync.dma_start(out=out_d[wf * 128:NPC, :], in_=outsb[0:rem, wf, :])

    fix_library_reloads(nc)
    split_multi_waits(nc)
    return nc


def make_in_maps(pp, inputs, N, F_IN=128, HID=64, HEADS=4, OUT=2):
    NPC = pp["npc"]
    NB = pp["NB"]
    x = np.ascontiguousarray(np.asarray(inputs["x"], dtype=np.float32))
    xT = np.ascontiguousarray(x.T)
    f32 = lambda a, shp: np.ascontiguousarray(np.asarray(a, dtype=np.float32).reshape(shp))

    W1s = f32(inputs["W1s"], (F_IN, HEADS * HID))
    W1d = f32(inputs["W1d"], (F_IN, HEADS * HID))
    a1s = f32(inputs["a1s"], (HEADS, HID))
    a1d = f32(inputs["a1d"], (HEADS, HID))
    W2s = f32(inputs["W2s"], (HID, HEADS * OUT))
    W2d = f32(inputs["W2d"], (HID, HEADS * OUT))
    a2s = f32(inputs["a2s"], (HEADS, OUT))
    a2d = f32(inputs["a2d"], (HEADS, OUT))
    # host weight folding: a_src/a_dst projections as extra W columns
    fold1s = np.einsum('fhc,hc->fh', W1s.reshape(F_IN, HEADS, HID), a1s)
    fold1d = np.einsum('fhc,hc->fh', W1d.reshape(F_IN, HEADS, HID), a1d)
    fold2s = np.einsum('fhc,hc->fh', W2s.reshape(HID, HEADS, OUT), a2s)
    fold2d = np.einsum('fhc,hc->fh', W2d.reshape(HID, HEADS, OUT), a2d)
    W1aug = np.ascontiguousarray(np.concatenate([W1s, fold1s], axis=1))
    Wl1aug = np.ascontiguousarray(
        np.concatenate([f32(inputs["Wl1"], (F_IN, HID)), fold1d], axis=1))
    W2aug = np.ascontiguousarray(np.concatenate(
        [W2s, fold2s, fold2d, f32(inputs["Wl2"], (HID, OUT))], axis=1))
    bias1 = np.tile((f32(inputs["b1"], (1, HID)) + f32(inputs["bl1"], (1, HID))), (128, 1))
    bias2 = np.tile((f32(inputs["b2"], (1, OUT)) + f32(inputs["bl2"], (1, OUT))), (128, 1))

    common = {
        "xT": xT,
        "W1aug": W1aug,
        "Wl1aug": Wl1aug,
        "W2aug": W2aug,
        "bias1": np.ascontiguousarray(bias1),
        "bias2": np.ascontiguousarray(bias2),
    }
    in_maps = []
    for c in range(8):
        m = dict(common)
        m["xTown"] = np.ascontiguousarray(xT[:, c * NPC:(c + 1) * NPC])
        m["gidx"] = pp["gidx"][c]
        m["dstloc"] = pp["dstloc"][c]
        lin = pp["dstloc"][c].T.reshape(-1).astype(np.int16)   # slot order
        lin = np.where(lin < 0, 0, lin)
        wrap = np.tile(lin.reshape(NB * 8, 16).T, (8, 1))
        m["dlidx"] = np.ascontiguousarray(wrap)
        in_maps.append(m)
    return in_maps


_BUILD_CACHE = {}
LAST_RESULTS = None


def kernel(**inputs):
    """Full inputs in, full [N, 2] float32 output out."""
    global LAST_RESULTS
    trace = bool(inputs.pop("_trace", False))
    pp = preprocess(inputs["edge_index"], N_NODES)
    key = (pp["NB"], tuple(pp["nbw_low"]), tuple(pp["nbw_high"]))
    if key not in _BUILD_CACHE:
        _BUILD_CACHE[key] = build(pp, N_NODES)
    nc = _BUILD_CACHE[key]
    in_maps = make_in_maps(pp, inputs, N_NODES)
    res = run_bass_kernel_spmd(nc, in_maps, list(range(8)), trace=trace)
    LAST_RESULTS = res
    out = np.concatenate([res.results[c]["out"] for c in range(8)], axis=0)
    return out.astype(np.float32)


# revision 9
# speedup vs baseline: 2.1854x; 1.1979x over previous
"""Self-contained Trainium2 Bass kernel for the 2-layer GAT problem
(nn_GAT_26714696581831). 8-core SPMD: edges sorted by dst, 8 dst-range
shards; per-window one-hot matmul aggregation with dma_gather row fetches
spread across 4 SWDGE queues (4 Q7 core pairs generate descriptors in
parallel).

kernel(**inputs) takes the FULL unsharded inputs and returns the FULL
[50000, 2] output.
"""
import sys
sys.path.insert(0, '/opt/trn_rl_repo')
import numpy as np
import concourse.bass as bass
import concourse.mybir as mybir
import concourse.tile as tile
from concourse import library_config
from concourse.masks import make_identity
from concourse.bass_utils import run_bass_kernel_spmd

N_NODES = 50000
"""Workarounds for this walrus build, which rejects any instruction carrying
more than one sync-wait command: hoist extra waits onto same-engine NoOps
inserted immediately before the instruction."""


_ctr = [0]

def split_multi_waits(nc, max_waits=1):
    for fn in nc.m.functions:
        for bb in fn.blocks:
            insts = bb.instructions
            i = 0
            while i < len(insts):
                ins = insts[i]
                si = ins.sync_info
                if si is not None and si.on_wait and len(si.on_wait) > max_waits:
                    waits = list(si.on_wait)
                    keep = waits[-max_waits:]
                    hoist = waits[:-max_waits]
                    si.on_wait = keep
                    for w in hoist:
                        _ctr[0] += 1
                        n = mybir.InstNoOp(name=f"waitsplit-{_ctr[0]}", ins=[], outs=[])
                        n.engine = ins.engine
                        n.sync_info = mybir.SyncInfo(on_wait=[w], on_update=[])
                        insts.insert(i, n)
                        i += 1
                i += 1


def fix_library_reloads(nc):
    """bass_rust leaves InstPseudoReloadLibraryIndex.instr empty; this walrus
    rejects zero-length ISA instructions. Encode the 64-byte
    PSEUDO_LIBRARY_RELOAD_INDEX struct with the live ISA tables."""
    isa = nc.isa
    sn = 'NEURON_ISA_TPB_PSEUDO_LIBRARY_RELOAD_INDEX_STRUCT'
    e = isa.get_enum("NEURON_ISA_TPB_PSEUDO_OPCODE")
    val = e.NEURON_ISA_TPB_PSEUDO_OPCODE_PSEUDO_LIBRARY_RELOAD_INDEX.value
    for fn in nc.m.functions:
        for bb in fn.blocks:
            for ins in bb.instructions:
                if type(ins).__name__ == 'InstPseudoReloadLibraryIndex' and not ins.instr:
                    b = isa.asm({"header": {"opcode": 223, "inst_word_len": 16},
                                 "pseudo_opcode": val,
                                 "lib_index": ins.lib_index}, sn)
                    ins.instr = [int(x) for x in b]




WIN = 128                  # dst nodes per window
SPLIT = 32768              # int16 positive limit for gather indices


def preprocess(edge_index, n_nodes, ncores=8):
    src = np.asarray(edge_index[0], dtype=np.int64)
    dst = np.asarray(edge_index[1], dtype=np.int64)
    npc = n_nodes // ncores
    nwin = (npc + WIN - 1) // WIN

    order = np.argsort(dst, kind="stable")
    src_s = src[order]
    dst_s = dst[order]

    counts = np.bincount(dst_s // npc, minlength=ncores)
    core_slices = np.concatenate([[0], np.cumsum(counts)])

    nlow = np.zeros((ncores, nwin), dtype=np.int64)
    nhigh = np.zeros((ncores, nwin), dtype=np.int64)
    per_core_win_edges = []
    for c in range(ncores):
        s0, s1 = core_slices[c], core_slices[c + 1]
        csrc = src_s[s0:s1]
        cdst = dst_s[s0:s1]
        wloc = (cdst - c * npc) // WIN
        dloc = (cdst - c * npc) % WIN
        wins = []
        for w in range(nwin):
            m = wloc == w
            ws, wd = csrc[m], dloc[m]
            lo = ws < SPLIT
            wins.append((ws[lo], ws[~lo] - SPLIT, wd[lo], wd[~lo]))
            nlow[c, w] = lo.sum()
            nhigh[c, w] = (~lo).sum()
        per_core_win_edges.append(wins)

    nbw_low = ((nlow.max(axis=0) + 127) // 128).astype(int)
    nbw_high = ((nhigh.max(axis=0) + 127) // 128).astype(int)
    for w in range(nwin):
        if nbw_low[w] + nbw_high[w] == 0:
            nbw_low[w] = 1
    NB = int(nbw_low.sum() + nbw_high.sum())

    gidx_lin = np.zeros((ncores, NB * 128), dtype=np.int16)
    srcidx_lin = np.zeros((ncores, NB * 128), dtype=np.int32)
    dstidx_lin = np.zeros((ncores, NB * 128), dtype=np.int32)
    dstloc_lin = np.full((ncores, NB * 128), -1, dtype=np.int16)

    for c in range(ncores):
        b0 = 0
        for w in range(nwin):
            slo, shi, dlo, dhi = per_core_win_edges[c][w]
            o = b0 * 128
            gidx_lin[c, o:o + len(slo)] = slo.astype(np.int16)
            srcidx_lin[c, o:o + len(slo)] = slo
            dstidx_lin[c, o:o + len(dlo)] = dlo + w * WIN + c * npc
            dstloc_lin[c, o:o + len(dlo)] = dlo.astype(np.int16)
            b0 += int(nbw_low[w])
            o = b0 * 128
            gidx_lin[c, o:o + len(shi)] = shi.astype(np.int16)
            srcidx_lin[c, o:o + len(shi)] = shi + SPLIT
            dstidx_lin[c, o:o + len(dhi)] = dhi + w * WIN + c * npc
            dstloc_lin[c, o:o + len(dhi)] = dhi.astype(np.int16)
            b0 += int(nbw_high[w])
        assert b0 == NB

    def wrap16(lin):  # [NC, NB*128] -> [NC, 128, NB*8] dma_gather layout
        x = lin.reshape(ncores, NB * 8, 16).transpose(0, 2, 1)
        return np.ascontiguousarray(np.tile(x, (1, 8, 1)))

    # dst-local indices for the a_dst gather (per-core local table, < 32768)
    adidx_lin = np.empty((ncores, NB * 128), dtype=np.int16)
    for c in range(ncores):
        loc = dstidx_lin[c].astype(np.int64) - c * npc
        loc[dstloc_lin[c] < 0] = 0          # pad slots -> row 0
        adidx_lin[c] = loc.astype(np.int16)

    def slotlay(lin, dtype):  # [NC, NB*128] -> [NC, 128, NB] ([p,b] = slot b*128+p)
        return np.ascontiguousarray(lin.reshape(ncores, NB, 128).transpose(0, 2, 1)).astype(dtype)

    return dict(
        NB=NB, nwin=nwin, npc=npc, ncores=ncores,
        nbw_low=nbw_low, nbw_high=nbw_high,
        gidx=wrap16(gidx_lin),
        adidx=wrap16(adidx_lin),
        srcidx=slotlay(srcidx_lin, np.int32),
        dstidx=slotlay(dstidx_lin, np.int32),
        dstloc=slotlay(dstloc_lin, np.int16),
    )




F32 = mybir.dt.float32
BF16 = mybir.dt.bfloat16
I16 = mybir.dt.int16
U16 = mybir.dt.uint16
AF = mybir.ActivationFunctionType
OP = mybir.AluOpType

SPLIT = 32768
GCHUNK = 8   # blocks per dma_gather call (1024 idx: single-packet-safe)
NQ = 4       # SWDGE queues: queue q's descriptors are generated by Q7 core
             # pair (2q, 2q+1); round-robin spreads desc-gen over all 8 cores


def chunked_gather(nc, out_tile, in_ap, idx_sb, b0, nblk, elem, regs, qrr, boff=0):
    """Issue dma_gather in <=GCHUNK-block chunks writing out_tile[:, boff+i...]."""
    done = 0
    while done < nblk:
        step = min(GCHUNK, nblk - done)
        n = step * 128
        if n not in regs:
            regs[n] = nc.gpsimd.to_reg(n)
        nc.gpsimd.dma_gather(
            out_tile[:, boff + done:boff + done + step, :], in_ap,
            idx_sb[:, (b0 + done) * 8:(b0 + done + step) * 8],
            n, regs[n], elem, queue_num=qrr[0] % NQ)
        qrr[0] += 1
        done += step


def build(pp, N, F_IN=128, HID=64, HEADS=4, OUT=2, neg_slope=0.2, stages='ABCDE'):
    NB = pp["NB"]
    NWIN = pp["nwin"]
    NPC = pp["npc"]
    HC1 = HEADS * HID          # 256
    HC2 = HEADS * OUT          # 8
    NBWmax = int(max(pp["nbw_low"][w] + pp["nbw_high"][w] for w in range(NWIN)))
    NCHUNK = (N + 127) // 128
    T1C = HC1 + 128            # 384 u16 cols = 768B rows
    L1COL = HC1 + HEADS        # 260: xs + a_src fold
    K1COL = HID + HEADS        # 68  (skip + W_ad fold)
    W2COL = HC2 + 2 * HEADS + OUT  # 18
    R2COL = HC2 + HEADS        # 12
    T2C = 128                  # u16 cols = 256B rows
    T2W = 16                   # used u16 cols of a table2 row (z2 bf16 + a2s f32)
    BBATCH = 4                 # stage-B chunks per DMA batch
    WSPLIT = NWIN // 2         # C-windows per t2 chunk (chunked allgather)
    ROWA = WSPLIT * 128        # own rows in chunk A
    ROWB = NPC - ROWA          # own rows in chunk B

    nc = bass.Bass("TRN2", target_bir_lowering=False, debug=False,
                   num_devices=8, num_swdge_queues=NQ)

    # ---- I/O ----
    xT = nc.dram_tensor("xT", [F_IN, N], F32, kind="ExternalInput")
    xTown = nc.dram_tensor("xTown", [F_IN, NPC], F32, kind="ExternalInput")
    W1aug_d = nc.dram_tensor("W1aug", [F_IN, L1COL], F32, kind="ExternalInput")
    Wl1aug_d = nc.dram_tensor("Wl1aug", [F_IN, K1COL], F32, kind="ExternalInput")
    W2aug_d = nc.dram_tensor("W2aug", [HID, W2COL], F32, kind="ExternalInput")
    bias1_d = nc.dram_tensor("bias1", [128, HID], F32, kind="ExternalInput")
    bias2_d = nc.dram_tensor("bias2", [128, OUT], F32, kind="ExternalInput")
    gidx_d = nc.dram_tensor("gidx", [128, NB * 8], I16, kind="ExternalInput")
    dlidx_d = nc.dram_tensor("dlidx", [128, NB * 8], I16, kind="ExternalInput")
    dstloc_d = nc.dram_tensor("dstloc", [128, NB], I16, kind="ExternalInput")
    out_d = nc.dram_tensor("out", [NPC, OUT], F32, kind="ExternalOutput")

    # internal DRAM. table1 split lo/hi so low-src gathers can start while
    # stage B is still projecting the high node range.
    table1lo = nc.dram_tensor("table1lo", [SPLIT, T1C], U16)
    table1hi = nc.dram_tensor("table1hi", [N - SPLIT, T1C], U16)
    adtab1 = nc.dram_tensor("adtab1", [NWIN * 128, 128], U16)   # 256B rows
    # layer-2 tables: compact [*, 16] payload allgathered in two window
    # chunks (A fires mid-stage-C), then expanded into 256B gather rows.
    t2cA = nc.dram_tensor("t2cA", [ROWA, T2W], U16)
    t2cB = nc.dram_tensor("t2cB", [ROWB, T2W], U16)
    tab2cA = nc.dram_tensor("tab2cA", [8 * ROWA, T2W], U16, addr_space="Shared")
    tab2cB = nc.dram_tensor("tab2cB", [8 * ROWB, T2W], U16, addr_space="Shared")
    ad2A = nc.dram_tensor("ad2A", [WSPLIT * 128, T2C], U16)
    ad2B = nc.dram_tensor("ad2B", [(NWIN - WSPLIT) * 128, T2C], U16)
    table2 = nc.dram_tensor("table2", [N, T2C], U16)

    with tile.TileContext(nc) as tc:
        with tc.tile_pool(name="const", bufs=1) as cpool, \
             tc.tile_pool(name="resident", bufs=1) as rpool:

            # ---- constants (all weight folding/permutation done on host) ----
            W1aug = cpool.tile([F_IN, L1COL], F32)
            nc.sync.dma_start(out=W1aug[:, :], in_=W1aug_d[:, :])
            Wl1aug = cpool.tile([F_IN, K1COL], F32)
            nc.sync.dma_start(out=Wl1aug[:, :], in_=Wl1aug_d[:, :])
            W2aug = cpool.tile([HID, W2COL], F32)
            nc.sync.dma_start(out=W2aug[:, :], in_=W2aug_d[:, :])
            bias1 = cpool.tile([128, HID], F32)
            nc.sync.dma_start(out=bias1[:, :], in_=bias1_d[:, :])
            bias2 = cpool.tile([128, OUT], F32)
            nc.sync.dma_start(out=bias2[:, :], in_=bias2_d[:, :])

            iota_t = cpool.tile([128, 128], I16)
            nc.gpsimd.iota(iota_t[:, :], pattern=[[1, 128]], base=0, channel_multiplier=0)
            ident = cpool.tile([128, 128], F32)
            make_identity(nc, ident[:, :])

            gidx_sb = rpool.tile([128, NB * 8], I16)
            nc.sync.dma_start(out=gidx_sb[:, :], in_=gidx_d[:, :])
            dlidx_sb = rpool.tile([128, NB * 8], I16)
            nc.sync.dma_start(out=dlidx_sb[:, :], in_=dlidx_d[:, :])
            dstloc_sb = rpool.tile([128, NB], I16)
            nc.sync.dma_start(out=dstloc_sb[:, :], in_=dstloc_d[:, :])

            # all standard-library gpsimd ops (iota/affine_select/memset) are
            # above; from here on the Q7 carveout holds the mlp library.
            nc.gpsimd.load_library(library_config.mlp)
            gregs = {}
            qrr = [0]

            if 'C' in stages:
                skip2sb = rpool.tile([128, NWIN, OUT], F32)
                outsb = rpool.tile([128, NWIN, OUT], F32)

            # ---- stage B: project all N nodes, table1 = [xs bf16 | a_src f32] ----
            # low node range [0, SPLIT) first: its gathers unblock early.
            assert SPLIT % (128 * BBATCH) == 0
            NBAT = (NCHUNK + BBATCH - 1) // BBATCH if 'B' in stages else 0
            with tc.tile_pool(name="projps", bufs=2, space="PSUM") as ppp, \
                 tc.tile_pool(name="projsb", bufs=3) as psb:
                for i in range(NBAT):
                    o = i * 128 * BBATCH
                    cb = min(128 * BBATCH, N - o)          # rows this batch
                    nch = (cb + 127) // 128
                    tab, to = (table1lo, o) if o < SPLIT else (table1hi, o - SPLIT)
                    xb = psb.tile([F_IN, BBATCH * 128], F32, tag="xb")
                    nc.sync.dma_start(out=xb[:, 0:cb], in_=xT[:, o:o + cb])
                    stbf = psb.tile([128, BBATCH, HC1], BF16, tag="stbf")
                    sts = psb.tile([128, BBATCH, 2 * HEADS], U16, tag="sts")
                    for j in range(nch):
                        cn = min(128, cb - j * 128)
                        ps = ppp.tile([128, L1COL], F32, space="PSUM")
                        nc.tensor.matmul(out=ps[0:cn, :],
                                         lhsT=xb[:, j * 128:j * 128 + cn],
                                         rhs=W1aug[:, :], start=True, stop=True)
                        nc.scalar.activation(out=stbf[0:cn, j, :],
                                             in_=ps[0:cn, 0:HC1], func=AF.Copy)
                        nc.vector.tensor_copy(sts[0:cn, j, :].bitcast(F32),
                                              ps[0:cn, HC1:L1COL])
                    if cb == BBATCH * 128:
                        nc.scalar.dma_start(
                            out=tab[to:to + cb, 0:HC1].bitcast(BF16)
                                .rearrange("(b p) c -> p b c", p=128),
                            in_=stbf[:, :, :])
                        nc.sync.dma_start(
                            out=tab[to:to + cb, HC1:HC1 + 2 * HEADS]
                                .rearrange("(b p) c -> p b c", p=128),
                            in_=sts[:, :, :])
                    else:
                        for j in range(nch):
                            cn = min(128, cb - j * 128)
                            oj = to + j * 128
                            nc.scalar.dma_start(
                                out=tab[oj:oj + cn, 0:HC1].bitcast(BF16),
                                in_=stbf[0:cn, j, :])
                            nc.sync.dma_start(
                                out=tab[oj:oj + cn, HC1:HC1 + 2 * HEADS],
                                in_=sts[0:cn, j, :])

            # ---- stage C: layer-1 edge softmax + aggregation per dst window,
            # with the layer-2 projection (old stage D) fused per window ----
            NWIN_C = NWIN if 'C' in stages else 0
            with tc.tile_pool(name="winps", bufs=2, space="PSUM") as wps, \
                 tc.tile_pool(name="skps", bufs=2, space="PSUM") as kps, \
                 tc.tile_pool(name="trps", bufs=2, space="PSUM") as tps, \
                 tc.tile_pool(name="l2ps", bufs=2, space="PSUM") as lps, \
                 tc.tile_pool(name="gpool", bufs=3) as gpl, \
                 tc.tile_pool(name="spool", bufs=6) as spl, \
                 tc.tile_pool(name="winsb", bufs=3) as wsb:
                b0 = 0
                for w in range(NWIN_C):
                    BL = int(pp["nbw_low"][w])
                    BH = int(pp["nbw_high"][w])
                    nb = BL + BH
                    cn_w = min(128, NPC - w * 128)
                    # skip matmul + a_dst of own nodes -> adtab1 window rows
                    xo = wsb.tile([F_IN, 128], F32, tag="xo")
                    nc.sync.dma_start(out=xo[:, 0:cn_w], in_=xTown[:, w * 128:w * 128 + cn_w])
                    psK = kps.tile([128, K1COL], F32, space="PSUM")
                    nc.tensor.matmul(out=psK[0:cn_w, :], lhsT=xo[:, 0:cn_w], rhs=Wl1aug[:, :],
                                     start=True, stop=True)
                    stK = wsb.tile([128, HEADS], F32, tag="stK")
                    nc.vector.tensor_copy(stK[0:cn_w, :], psK[0:cn_w, HID:K1COL])
                    nc.sync.dma_start(out=adtab1[w * 128:w * 128 + cn_w, 0:8],
                                      in_=stK[0:cn_w, :].bitcast(U16))
                    # one-hot dst matrix (only dep: dstloc) - deep pool so it
                    # can be built ahead whenever the DVE has slack
                    S = spl.tile([128, NBWmax, 128], BF16, tag="S")
                    nc.vector.tensor_tensor(
                        out=S[:, 0:nb, :],
                        in0=dstloc_sb[:, b0:b0 + nb].unsqueeze(2).to_broadcast([128, nb, 128]),
                        in1=iota_t[:, :].unsqueeze(1).to_broadcast([128, nb, 128]),
                        op=OP.is_equal)
                    # gathers
                    G = gpl.tile([128, NBWmax, T1C], U16, tag="G")
                    if BL:
                        chunked_gather(nc, G, table1lo[:, :],
                                       gidx_sb, b0, BL, T1C, gregs, qrr)
                    if BH:
                        chunked_gather(nc, G, table1hi[:, :],
                                       gidx_sb, b0 + BL, BH, T1C, gregs, qrr, boff=BL)
                    AD = gpl.tile([128, NBWmax, 128], U16, tag="AD")
                    chunked_gather(nc, AD, adtab1[w * 128:w * 128 + 128, :],
                                   dlidx_sb, b0, nb, 128, gregs, qrr)
                    # e = a_src + a_dst; ex = exp(lrelu(e)) = max(exp(e), exp(0.2e))
                    ex = wsb.tile([128, NBWmax, HEADS], F32, tag="ex")
                    nc.vector.tensor_tensor(out=ex[:, 0:nb, :],
                                            in0=G[:, 0:nb, HC1:HC1 + 8].bitcast(F32),
                                            in1=AD[:, 0:nb, 0:8].bitcast(F32), op=OP.add)
                    exa = wsb.tile([128, NBWmax, HEADS], BF16, tag="exa")
                    nc.scalar.activation(out=exa[:, 0:nb, :], in_=ex[:, 0:nb, :],
                                         func=AF.Exp)
                    exb = wsb.tile([128, NBWmax, HEADS], BF16, tag="exb")
                    nc.scalar.activation(out=exb[:, 0:nb, :], in_=ex[:, 0:nb, :],
                                         func=AF.Exp, scale=float(neg_slope))
                    nc.vector.tensor_tensor(out=exb[:, 0:nb, :], in0=exa[:, 0:nb, :],
                                            in1=exb[:, 0:nb, :], op=OP.max)
                    Gp = gpl.tile([128, NBWmax, HC1 + HEADS], BF16, tag="Gp")
                    nc.vector.tensor_tensor(
                        out=Gp[:, 0:nb, 0:HC1].rearrange("p b (h c) -> p b h c", h=HEADS),
                        in0=G[:, 0:nb, 0:HC1].bitcast(BF16).rearrange("p b (h c) -> p b h c", h=HEADS),
                        in1=exb[:, 0:nb, :].unsqueeze(3).to_broadcast([128, nb, HEADS, HID]),
                        op=OP.mult)
                    nc.vector.tensor_copy(Gp[:, 0:nb, HC1:HC1 + HEADS], exb[:, 0:nb, :])
                    psW = wps.tile([128, HC1 + HEADS], F32, space="PSUM")
                    for j in range(nb):
                        nc.tensor.matmul(out=psW[:, :], lhsT=S[:, j, :], rhs=Gp[:, j, :],
                                         start=(j == 0), stop=(j == nb - 1))
                    # extract: h = sigmoid(gat/4sum + skip + bias)
                    rec = wsb.tile([128, HEADS], F32, tag="rec")
                    nc.vector.tensor_scalar(out=rec[:, :], in0=psW[:, HC1:HC1 + HEADS],
                                            scalar1=1e-16, scalar2=float(HEADS),
                                            op0=OP.add, op1=OP.mult)
                    nc.vector.reciprocal(rec[:, :], rec[:, :])
                    gat = wsb.tile([128, HC1], F32, tag="gat")
                    nc.vector.tensor_tensor(
                        out=gat[:, :].rearrange("p (h c) -> p h c", h=HEADS),
                        in0=psW[:, 0:HC1].rearrange("p (h c) -> p h c", h=HEADS),
                        in1=rec[:, :].unsqueeze(2).to_broadcast([128, HEADS, HID]),
                        op=OP.mult)
                    hred = wsb.tile([128, HID], F32, tag="hred")
                    nc.vector.tensor_reduce(
                        out=hred[:, :],
                        in_=gat[:, :].rearrange("p (h c) -> p c h", h=HEADS),
                        axis=mybir.AxisListType.X, op=OP.add)
                    nc.vector.tensor_tensor(out=hred[:, :], in0=hred[:, :],
                                            in1=psK[:, 0:HID], op=OP.add)
                    nc.vector.tensor_tensor(out=hred[:, :], in0=hred[:, :],
                                            in1=bias1[:, :], op=OP.add)
                    hwin = wsb.tile([128, HID], F32, tag="hwin")
                    nc.scalar.activation(out=hwin[:, :], in_=hred[:, :], func=AF.Sigmoid)
                    psT = tps.tile([HID, 128], F32, space="PSUM")
                    nc.tensor.transpose(out=psT[:, :], in_=hwin[:, :], identity=ident[:, :])
                    htw = wsb.tile([HID, 128], F32, tag="htw")
                    nc.vector.tensor_copy(htw[:, :], psT[:, :])
                    # fused layer-2 projection of this window's own nodes
                    psL = lps.tile([128, W2COL], F32, space="PSUM")
                    nc.tensor.matmul(out=psL[0:cn_w, :], lhsT=htw[:, 0:cn_w],
                                     rhs=W2aug[:, :], start=True, stop=True)
                    t2st = wsb.tile([128, T2W], U16, tag="t2st")
                    nc.vector.tensor_copy(t2st[0:cn_w, 0:HC2].bitcast(BF16),
                                          psL[0:cn_w, 0:HC2])
                    nc.vector.tensor_copy(t2st[0:cn_w, HC2:T2W].bitcast(F32),
                                          psL[0:cn_w, HC2:HC2 + HEADS])
                    if w < WSPLIT:
                        nc.sync.dma_start(out=t2cA[w * 128:w * 128 + cn_w, :],
                                          in_=t2st[0:cn_w, :])
                    else:
                        ob = (w - WSPLIT) * 128
                        nc.sync.dma_start(out=t2cB[ob:ob + cn_w, :],
                                          in_=t2st[0:cn_w, :])
                    stad = wsb.tile([128, HEADS], F32, tag="stad")
                    nc.vector.tensor_copy(stad[0:cn_w, :],
                                          psL[0:cn_w, HC2 + HEADS:HC2 + 2 * HEADS])
                    if w < WSPLIT:
                        nc.sync.dma_start(out=ad2A[w * 128:w * 128 + cn_w, 0:2 * HEADS],
                                          in_=stad[0:cn_w, :].bitcast(U16))
                    else:
                        ob = (w - WSPLIT) * 128
                        nc.sync.dma_start(out=ad2B[ob:ob + cn_w, 0:2 * HEADS],
                                          in_=stad[0:cn_w, :].bitcast(U16))
                    nc.vector.tensor_copy(skip2sb[0:cn_w, w, :],
                                          psL[0:cn_w, HC2 + 2 * HEADS:W2COL])
                    b0 += nb
                    if w == WSPLIT - 1 and 'D' in stages:
                        # chunk A complete on our core: allgather + expand it
                        # while the rest of stage C runs
                        nc.gpsimd.collective_compute(
                            "AllGather", OP.bypass, replica_groups=[list(range(8))],
                            ins=[t2cA[:, :]], outs=[tab2cA[:, :]])
                        nc.sync.dma_start(
                            out=table2[:, 0:T2W]
                                .rearrange("(c r) x -> c r x", c=8)[:, 0:ROWA, :],
                            in_=tab2cA[:, :].rearrange("(c r) x -> c r x", c=8))

            if 'D' in stages:
                nc.gpsimd.collective_compute(
                    "AllGather", OP.bypass, replica_groups=[list(range(8))],
                    ins=[t2cB[:, :]], outs=[tab2cB[:, :]])
                nc.sync.dma_start(
                    out=table2[:, 0:T2W]
                        .rearrange("(c r) x -> c r x", c=8)[:, ROWA:NPC, :],
                    in_=tab2cB[:, :].rearrange("(c r) x -> c r x", c=8))

            # ---- stage E: layer-2 edge softmax + aggregation ----
            NWIN_E = NWIN if 'E' in stages else 0
            with tc.tile_pool(name="w2ps", bufs=2, space="PSUM") as wps2, \
                 tc.tile_pool(name="s2pool", bufs=6) as spl2, \
                 tc.tile_pool(name="w2sb", bufs=3) as w2sb:
                b0 = 0
                for w in range(NWIN_E):
                    BL = int(pp["nbw_low"][w])
                    BH = int(pp["nbw_high"][w])
                    nb = BL + BH
                    # dst-side gather only needs C-phase data: can start
                    # during the chunk-B collective flight
                    g2d = w2sb.tile([128, NBWmax, T2C], U16, tag="g2d")
                    if w < WSPLIT:
                        chunked_gather(nc, g2d, ad2A[w * 128:w * 128 + 128, :],
                                       dlidx_sb, b0, nb, T2C, gregs, qrr)
                    else:
                        ob = (w - WSPLIT) * 128
                        chunked_gather(nc, g2d, ad2B[ob:ob + 128, :],
                                       dlidx_sb, b0, nb, T2C, gregs, qrr)
                    g2s = w2sb.tile([128, NBWmax, T2C], U16, tag="g2s")
                    if BL:
                        chunked_gather(nc, g2s, table2[0:min(SPLIT, N), :],
                                       gidx_sb, b0, BL, T2C, gregs, qrr)
                    if BH:
                        chunked_gather(nc, g2s, table2[SPLIT:N, :],
                                       gidx_sb, b0 + BL, BH, T2C, gregs, qrr, boff=BL)
                    ex2 = w2sb.tile([128, NBWmax, HEADS], F32, tag="ex2")
                    nc.vector.tensor_tensor(out=ex2[:, 0:nb, :],
                                            in0=g2s[:, 0:nb, HC2:16].bitcast(F32),
                                            in1=g2d[:, 0:nb, 0:8].bitcast(F32), op=OP.add)
                    ex2a = w2sb.tile([128, NBWmax, HEADS], BF16, tag="ex2a")
                    nc.scalar.activation(out=ex2a[:, 0:nb, :], in_=ex2[:, 0:nb, :],
                                         func=AF.Exp)
                    ex2b = w2sb.tile([128, NBWmax, HEADS], BF16, tag="ex2b")
                    nc.scalar.activation(out=ex2b[:, 0:nb, :], in_=ex2[:, 0:nb, :],
                                         func=AF.Exp, scale=float(neg_slope))
                    nc.vector.tensor_tensor(out=ex2b[:, 0:nb, :], in0=ex2a[:, 0:nb, :],
                                            in1=ex2b[:, 0:nb, :], op=OP.max)
                    R2 = w2sb.tile([128, NBWmax, R2COL], BF16, tag="R2")
                    nc.vector.tensor_tensor(
                        out=R2[:, 0:nb, 0:HC2].rearrange("p b (h c) -> p b h c", h=HEADS),
                        in0=g2s[:, 0:nb, 0:HC2].bitcast(BF16).rearrange("p b (h c) -> p b h c", h=HEADS),
                        in1=ex2b[:, 0:nb, :].unsqueeze(3).to_broadcast([128, nb, HEADS, OUT]),
                        op=OP.mult)
                    nc.vector.tensor_copy(R2[:, 0:nb, HC2:R2COL], ex2b[:, 0:nb, :])
                    S2 = spl2.tile([128, NBWmax, 128], BF16, tag="S2")
                    nc.vector.tensor_tensor(
                        out=S2[:, 0:nb, :],
                        in0=dstloc_sb[:, b0:b0 + nb].unsqueeze(2).to_broadcast([128, nb, 128]),
                        in1=iota_t[:, :].unsqueeze(1).to_broadcast([128, nb, 128]),
                        op=OP.is_equal)
                    psW2 = wps2.tile([128, R2COL], F32, space="PSUM")
                    for j in range(nb):
                        nc.tensor.matmul(out=psW2[:, :], lhsT=S2[:, j, :], rhs=R2[:, j, :],
                                         start=(j == 0), stop=(j == nb - 1))
                    rec2 = w2sb.tile([128, HEADS], F32, tag="rec2")
                    nc.vector.tensor_scalar(out=rec2[:, :], in0=psW2[:, HC2:R2COL],
                                            scalar1=1e-16, scalar2=float(HEADS),
                                            op0=OP.add, op1=OP.mult)
                    nc.vector.reciprocal(rec2[:, :], rec2[:, :])
                    og = w2sb.tile([128, HC2], F32, tag="og")
                    nc.vector.tensor_tensor(
                        out=og[:, :].rearrange("p (h c) -> p h c", h=HEADS),
                        in0=psW2[:, 0:HC2].rearrange("p (h c) -> p h c", h=HEADS),
                        in1=rec2[:, :].unsqueeze(2).to_broadcast([128, HEADS, OUT]),
                        op=OP.mult)
                    ored = w2sb.tile([128, OUT], F32, tag="ored")
                    nc.vector.tensor_reduce(
                        out=ored[:, :],
                        in_=og[:, :].rearrange("p (h c) -> p c h", h=HEADS),
                        axis=mybir.AxisListType.X, op=OP.add)
                    nc.vector.tensor_tensor(out=ored[:, :], in0=ored[:, :],
                                            in1=skip2sb[:, w, :], op=OP.add)
                    nc.vector.tensor_tensor(out=outsb[:, w, :], in0=ored[:, :],
                                            in1=bias2[:, :], op=OP.add)
                    b0 += nb

            # ---- final output DMA ----
            wf = NPC // 128 if 'E' in stages else 0
            rem = NPC % 128 if 'E' in stages else 0
            if wf:
                nc.sync.dma_start(
                    out=out_d[0:wf * 128, :].rearrange("(w p) c -> p w c", p=128),
                    in_=outsb[:, 0:wf, :])
            if rem:
                nc.sync.dma_start(out=out_d[wf * 128:NPC, :], in_=outsb[0:rem, wf, :])

    fix_library_reloads(nc)
    split_multi_waits(nc)
    return nc


def make_in_maps(pp, inputs, N, F_IN=128, HID=64, HEADS=4, OUT=2):
    NPC = pp["npc"]
    NB = pp["NB"]
    x = np.ascontiguousarray(np.asarray(inputs["x"], dtype=np.float32))
    xT = np.ascontiguousarray(x.T)
    f32 = lambda a, shp: np.ascontiguousarray(np.asarray(a, dtype=np.float32).reshape(shp))

    W1s = f32(inputs["W1s"], (F_IN, HEADS * HID))
    W1d = f32(inputs["W1d"], (F_IN, HEADS * HID))
    a1s = f32(inputs["a1s"], (HEADS, HID))
    a1d = f32(inputs["a1d"], (HEADS, HID))
    W2s = f32(inputs["W2s"], (HID, HEADS * OUT))
    W2d = f32(inputs["W2d"], (HID, HEADS * OUT))
    a2s = f32(inputs["a2s"], (HEADS, OUT))
    a2d = f32(inputs["a2d"], (HEADS, OUT))
    # host weight folding: a_src/a_dst projections as extra W columns
    fold1s = np.einsum('fhc,hc->fh', W1s.reshape(F_IN, HEADS, HID), a1s)
    fold1d = np.einsum('fhc,hc->fh', W1d.reshape(F_IN, HEADS, HID), a1d)
    fold2s = np.einsum('fhc,hc->fh', W2s.reshape(HID, HEADS, OUT), a2s)
    fold2d = np.einsum('fhc,hc->fh', W2d.reshape(HID, HEADS, OUT), a2d)
    W1aug = np.ascontiguousarray(np.concatenate([W1s, fold1s], axis=1))
    Wl1aug = np.ascontiguousarray(
        np.concatenate([f32(inputs["Wl1"], (F_IN, HID)), fold1d], axis=1))
    W2aug = np.ascontiguousarray(np.concatenate(
        [W2s, fold2s, fold2d, f32(inputs["Wl2"], (HID, OUT))], axis=1))
    bias1 = np.tile((f32(inputs["b1"], (1, HID)) + f32(inputs["bl1"], (1, HID))), (128, 1))
    bias2 = np.tile((f32(inputs["b2"], (1, OUT)) + f32(inputs["bl2"], (1, OUT))), (128, 1))

    common = {
        "xT": xT,
        "W1aug": W1aug,
        "Wl1aug": Wl1aug,
        "W2aug": W2aug,
        "bias1": np.ascontiguousarray(bias1),
        "bias2": np.ascontiguousarray(bias2),
    }
    in_maps = []
    for c in range(8):
        m = dict(common)
        m["xTown"] = np.ascontiguousarray(xT[:, c * NPC:(c + 1) * NPC])
        m["gidx"] = pp["gidx"][c]
        m["dstloc"] = pp["dstloc"][c]
        lin = pp["dstloc"][c].T.reshape(-1).astype(np.int16)   # slot order
        lin = np.where(lin < 0, 0, lin)
        wrap = np.tile(lin.reshape(NB * 8, 16).T, (8, 1))
        m["dlidx"] = np.ascontiguousarray(wrap)
        in_maps.append(m)
    return in_maps


_BUILD_CACHE = {}
LAST_RESULTS = None


def kernel(**inputs):
    """Full inputs in, full [N, 2] float32 output out."""
    global LAST_RESULTS
    trace = bool(inputs.pop("_trace", False))
    pp = preprocess(inputs["edge_index"], N_NODES)
    key = (pp["NB"], tuple(pp["nbw_low"]), tuple(pp["nbw_high"]))
    if key not in _BUILD_CACHE:
        _BUILD_CACHE[key] = build(pp, N_NODES)
    nc = _BUILD_CACHE[key]
    in_maps = make_in_maps(pp, inputs, N_NODES)
    res = run_bass_kernel_spmd(nc, in_maps, list(range(8)), trace=trace)
    LAST_RESULTS = res
    out = np.concatenate([res.results[c]["out"] for c in range(8)], axis=0)
    return out.astype(np.float32)


# revision 19
# speedup vs baseline: 2.9257x; 1.3388x over previous
"""Self-contained Trainium2 Bass kernel for the 2-layer GAT problem
(nn_GAT_26714696581831). 8-core SPMD: edges sorted by dst, 8 dst-range
shards; per-window one-hot matmul aggregation with dma_gather row fetches
spread across 4 SWDGE queues (4 Q7 core pairs generate descriptors in
parallel).

kernel(**inputs) takes the FULL unsharded inputs and returns the FULL
[50000, 2] output.
"""
import sys
sys.path.insert(0, '/opt/trn_rl_repo')
import numpy as np
import concourse.bass as bass
import concourse.mybir as mybir
import concourse.tile as tile
from concourse import library_config
from concourse.masks import make_identity
from concourse.bass_utils import run_bass_kernel_spmd

N_NODES = 50000
"""Workarounds for this walrus build, which rejects any instruction carrying
more than one sync-wait command: hoist extra waits onto same-engine NoOps
inserted immediately before the instruction."""


_ctr = [0]

def split_multi_waits(nc, max_waits=1):
    for fn in nc.m.functions:
        for bb in fn.blocks:
            insts = bb.instructions
            i = 0
            while i < len(insts):
                ins = insts[i]
                si = ins.sync_info
                if si is not None and si.on_wait and len(si.on_wait) > max_waits:
                    waits = list(si.on_wait)
                    keep = waits[-max_waits:]
                    hoist = waits[:-max_waits]
                    si.on_wait = keep
                    for w in hoist:
                        _ctr[0] += 1
                        n = mybir.InstNoOp(name=f"waitsplit-{_ctr[0]}", ins=[], outs=[])
                        n.engine = ins.engine
                        n.sync_info = mybir.SyncInfo(on_wait=[w], on_update=[])
                        insts.insert(i, n)
                        i += 1
                i += 1


def fix_library_reloads(nc):
    """bass_rust leaves InstPseudoReloadLibraryIndex.instr empty; this walrus
    rejects zero-length ISA instructions. Encode the 64-byte
    PSEUDO_LIBRARY_RELOAD_INDEX struct with the live ISA tables."""
    isa = nc.isa
    sn = 'NEURON_ISA_TPB_PSEUDO_LIBRARY_RELOAD_INDEX_STRUCT'
    e = isa.get_enum("NEURON_ISA_TPB_PSEUDO_OPCODE")
    val = e.NEURON_ISA_TPB_PSEUDO_OPCODE_PSEUDO_LIBRARY_RELOAD_INDEX.value
    for fn in nc.m.functions:
        for bb in fn.blocks:
            for ins in bb.instructions:
                if type(ins).__name__ == 'InstPseudoReloadLibraryIndex' and not ins.instr:
                    b = isa.asm({"header": {"opcode": 223, "inst_word_len": 16},
                                 "pseudo_opcode": val,
                                 "lib_index": ins.lib_index}, sn)
                    ins.instr = [int(x) for x in b]




WIN = 128                  # dst nodes per window
SPLIT = 32768              # int16 positive limit for gather indices


def preprocess(edge_index, n_nodes, ncores=8):
    src = np.asarray(edge_index[0], dtype=np.int64)
    dst = np.asarray(edge_index[1], dtype=np.int64)
    npc = n_nodes // ncores
    nwin = (npc + WIN - 1) // WIN

    order = np.argsort(dst, kind="stable")
    src_s = src[order]
    dst_s = dst[order]

    counts = np.bincount(dst_s // npc, minlength=ncores)
    core_slices = np.concatenate([[0], np.cumsum(counts)])

    nlow = np.zeros((ncores, nwin), dtype=np.int64)
    nhigh = np.zeros((ncores, nwin), dtype=np.int64)
    per_core_win_edges = []
    for c in range(ncores):
        s0, s1 = core_slices[c], core_slices[c + 1]
        csrc = src_s[s0:s1]
        cdst = dst_s[s0:s1]
        wloc = (cdst - c * npc) // WIN
        dloc = (cdst - c * npc) % WIN
        wins = []
        for w in range(nwin):
            m = wloc == w
            ws, wd = csrc[m], dloc[m]
            lo = ws < SPLIT
            wins.append((ws[lo], ws[~lo] - SPLIT, wd[lo], wd[~lo]))
            nlow[c, w] = lo.sum()
            nhigh[c, w] = (~lo).sum()
        per_core_win_edges.append(wins)

    nbw_low = ((nlow.max(axis=0) + 127) // 128).astype(int)
    nbw_high = ((nhigh.max(axis=0) + 127) // 128).astype(int)
    for w in range(nwin):
        if nbw_low[w] + nbw_high[w] == 0:
            nbw_low[w] = 1
    NB = int(nbw_low.sum() + nbw_high.sum())

    gidx_lin = np.zeros((ncores, NB * 128), dtype=np.int16)
    srcidx_lin = np.zeros((ncores, NB * 128), dtype=np.int32)
    dstidx_lin = np.zeros((ncores, NB * 128), dtype=np.int32)
    dstloc_lin = np.full((ncores, NB * 128), -1, dtype=np.int16)

    for c in range(ncores):
        b0 = 0
        for w in range(nwin):
            slo, shi, dlo, dhi = per_core_win_edges[c][w]
            o = b0 * 128
            gidx_lin[c, o:o + len(slo)] = slo.astype(np.int16)
            srcidx_lin[c, o:o + len(slo)] = slo
            dstidx_lin[c, o:o + len(dlo)] = dlo + w * WIN + c * npc
            dstloc_lin[c, o:o + len(dlo)] = dlo.astype(np.int16)
            b0 += int(nbw_low[w])
            o = b0 * 128
            gidx_lin[c, o:o + len(shi)] = shi.astype(np.int16)
            srcidx_lin[c, o:o + len(shi)] = shi + SPLIT
            dstidx_lin[c, o:o + len(dhi)] = dhi + w * WIN + c * npc
            dstloc_lin[c, o:o + len(dhi)] = dhi.astype(np.int16)
            b0 += int(nbw_high[w])
        assert b0 == NB

    def wrap16(lin):  # [NC, NB*128] -> [NC, 128, NB*8] dma_gather layout
        x = lin.reshape(ncores, NB * 8, 16).transpose(0, 2, 1)
        return np.ascontiguousarray(np.tile(x, (1, 8, 1)))

    # dst-local indices for the a_dst gather (per-core local table, < 32768)
    adidx_lin = np.empty((ncores, NB * 128), dtype=np.int16)
    for c in range(ncores):
        loc = dstidx_lin[c].astype(np.int64) - c * npc
        loc[dstloc_lin[c] < 0] = 0          # pad slots -> row 0
        adidx_lin[c] = loc.astype(np.int16)

    def slotlay(lin, dtype):  # [NC, NB*128] -> [NC, 128, NB] ([p,b] = slot b*128+p)
        return np.ascontiguousarray(lin.reshape(ncores, NB, 128).transpose(0, 2, 1)).astype(dtype)

    return dict(
        NB=NB, nwin=nwin, npc=npc, ncores=ncores,
        nbw_low=nbw_low, nbw_high=nbw_high,
        gidx=wrap16(gidx_lin),
        adidx=wrap16(adidx_lin),
        srcidx=slotlay(srcidx_lin, np.int32),
        dstidx=slotlay(dstidx_lin, np.int32),
        dstloc=slotlay(dstloc_lin, np.int16),
    )




F32 = mybir.dt.float32
BF16 = mybir.dt.bfloat16
I16 = mybir.dt.int16
U16 = mybir.dt.uint16
AF = mybir.ActivationFunctionType
OP = mybir.AluOpType

SPLIT = 32768
GCHUNK = 8   # blocks per dma_gather call (1024 idx: single-packet-safe)
NQ = 4       # SWDGE queues: queue q's descriptors are generated by Q7 core
             # pair (2q, 2q+1); round-robin spreads desc-gen over all 8 cores


def chunked_gather(nc, out_tile, in_ap, idx_sb, b0, nblk, elem, regs, qrr, boff=0):
    """Issue dma_gather in <=GCHUNK-block chunks writing out_tile[:, boff+i...].

    qrr is a 4-entry running block-count per SWDGE queue; each chunk goes to
    the least-loaded queue so all four Q7 desc-gen core pairs stay balanced.
    """
    done = 0
    while done < nblk:
        step = min(GCHUNK, nblk - done)
        n = step * 128
        if n not in regs:
            regs[n] = nc.gpsimd.to_reg(n)
        q = min(range(NQ), key=lambda i: qrr[i])
        nc.gpsimd.dma_gather(
            out_tile[:, boff + done:boff + done + step, :], in_ap,
            idx_sb[:, (b0 + done) * 8:(b0 + done + step) * 8],
            n, regs[n], elem, queue_num=q)
        qrr[q] += step
        done += step


def build(pp, N, F_IN=128, HID=64, HEADS=4, OUT=2, neg_slope=0.2, stages='ABCDE'):
    NB = pp["NB"]
    NWIN = pp["nwin"]
    NPC = pp["npc"]
    HC1 = HEADS * HID          # 256
    HC2 = HEADS * OUT          # 8
    NBWmax = int(max(pp["nbw_low"][w] + pp["nbw_high"][w] for w in range(NWIN)))
    NCHUNK = (N + 127) // 128
    T1C = HC1 + 128            # 384 u16 cols = 768B rows
    L1COL = HC1 + HEADS        # 260: xs + a_src fold
    K1COL = HID + HEADS        # 68  (skip + W_ad fold)
    W2COL = HC2 + 2 * HEADS + OUT  # 18
    R2COL = HC2 + HEADS        # 12
    T2C = 128                  # u16 cols = 256B rows
    T2W = 16                   # used u16 cols of a table2 row (z2 bf16 + a2s f32)
    BBATCH = 4                 # stage-B chunks per DMA batch
    WSPLIT = NWIN // 2         # C-windows per t2 chunk (chunked allgather)
    ROWA = WSPLIT * 128        # own rows in chunk A
    ROWB = NPC - ROWA          # own rows in chunk B

    nc = bass.Bass("TRN2", target_bir_lowering=False, debug=False,
                   num_devices=8, num_swdge_queues=NQ)

    # ---- I/O ----
    xT = nc.dram_tensor("xT", [F_IN, N], F32, kind="ExternalInput")
    xTown = nc.dram_tensor("xTown", [F_IN, NPC], F32, kind="ExternalInput")
    W1aug_d = nc.dram_tensor("W1aug", [F_IN, L1COL], F32, kind="ExternalInput")
    Wl1aug_d = nc.dram_tensor("Wl1aug", [F_IN, K1COL], F32, kind="ExternalInput")
    W2aug_d = nc.dram_tensor("W2aug", [HID, W2COL], F32, kind="ExternalInput")
    bias1_d = nc.dram_tensor("bias1", [128, HID], F32, kind="ExternalInput")
    bias2_d = nc.dram_tensor("bias2", [128, OUT], F32, kind="ExternalInput")
    gidx_d = nc.dram_tensor("gidx", [128, NB * 8], I16, kind="ExternalInput")
    dstloc_d = nc.dram_tensor("dstloc", [128, NB], I16, kind="ExternalInput")
    # transposed one-hot dst matrices St[d, slot] (host-built): fetches
    # per-edge a_dst values via a tiny matmul instead of a dma_gather
    Stdat_d = nc.dram_tensor("Stdat", [128, NB * 128], BF16, kind="ExternalInput")
    out_d = nc.dram_tensor("out", [NPC, OUT], F32, kind="ExternalOutput")

    # internal DRAM. table1 split lo/hi so low-src gathers can start while
    # stage B is still projecting the high node range.
    table1lo = nc.dram_tensor("table1lo", [SPLIT, T1C], U16)
    table1hi = nc.dram_tensor("table1hi", [N - SPLIT, T1C], U16)
    # layer-2 tables: compact [*, 16] payload allgathered in two window
    # chunks (A fires mid-stage-C), then expanded into 256B gather rows.
    t2cA = nc.dram_tensor("t2cA", [ROWA, T2W], U16)
    t2cB = nc.dram_tensor("t2cB", [ROWB, T2W], U16)
    tab2cA = nc.dram_tensor("tab2cA", [8 * ROWA, T2W], U16, addr_space="Shared")
    tab2cB = nc.dram_tensor("tab2cB", [8 * ROWB, T2W], U16, addr_space="Shared")
    table2 = nc.dram_tensor("table2", [N, T2C], U16)

    with tile.TileContext(nc) as tc:
        with tc.tile_pool(name="const", bufs=1) as cpool, \
             tc.tile_pool(name="resident", bufs=1) as rpool:

            # ---- constants (all weight folding/permutation done on host) ----
            W1aug = cpool.tile([F_IN, L1COL], F32)
            nc.sync.dma_start(out=W1aug[:, :], in_=W1aug_d[:, :])
            Wl1aug = cpool.tile([F_IN, K1COL], F32)
            nc.sync.dma_start(out=Wl1aug[:, :], in_=Wl1aug_d[:, :])
            W2aug = cpool.tile([HID, W2COL], F32)
            nc.sync.dma_start(out=W2aug[:, :], in_=W2aug_d[:, :])
            bias1 = cpool.tile([128, HID], F32)
            nc.sync.dma_start(out=bias1[:, :], in_=bias1_d[:, :])
            bias2 = cpool.tile([128, OUT], F32)
            nc.sync.dma_start(out=bias2[:, :], in_=bias2_d[:, :])

            iota_t = cpool.tile([128, 128], I16)
            nc.gpsimd.iota(iota_t[:, :], pattern=[[1, 128]], base=0, channel_multiplier=0)
            ident = cpool.tile([128, 128], F32)
            make_identity(nc, ident[:, :])

            gidx_sb = rpool.tile([128, NB * 8], I16)
            nc.sync.dma_start(out=gidx_sb[:, :], in_=gidx_d[:, :])
            dstloc_sb = rpool.tile([128, NB], I16)
            nc.sync.dma_start(out=dstloc_sb[:, :], in_=dstloc_d[:, :])

            # all standard-library gpsimd ops (iota/affine_select/memset) are
            # above; from here on the Q7 carveout holds the mlp library.
            nc.gpsimd.load_library(library_config.mlp)
            gregs = {}
            qrr = [0, 0, 0, 0]

            if 'C' in stages:
                skip2sb = rpool.tile([128, NWIN, OUT], F32)
                outsb = rpool.tile([128, NWIN, OUT], F32)
                ad2sb = rpool.tile([128, NWIN, HEADS], BF16)
                # zero-fill: the last ragged window leaves tail partitions
                # unwritten and 0 x NaN would poison the psAD2 contraction
                nc.vector.memset(ad2sb[:, :, :], 0.0)

            # ---- stage B: project all N nodes, table1 = [xs bf16 | a_src f32] ----
            # low node range [0, SPLIT) first: its gathers unblock early.
            assert SPLIT % (128 * BBATCH) == 0
            NBAT = (NCHUNK + BBATCH - 1) // BBATCH if 'B' in stages else 0
            with tc.tile_pool(name="projps", bufs=2, space="PSUM") as ppp, \
                 tc.tile_pool(name="projsb", bufs=3) as psb:
                for i in range(NBAT):
                    o = i * 128 * BBATCH
                    cb = min(128 * BBATCH, N - o)          # rows this batch
                    nch = (cb + 127) // 128
                    tab, to = (table1lo, o) if o < SPLIT else (table1hi, o - SPLIT)
                    xb = psb.tile([F_IN, BBATCH * 128], F32, tag="xb")
                    nc.sync.dma_start(out=xb[:, 0:cb], in_=xT[:, o:o + cb])
                    stbf = psb.tile([128, BBATCH, HC1], BF16, tag="stbf")
                    sts = psb.tile([128, BBATCH, 2 * HEADS], U16, tag="sts")
                    for j in range(nch):
                        cn = min(128, cb - j * 128)
                        ps = ppp.tile([128, L1COL], F32, space="PSUM")
                        nc.tensor.matmul(out=ps[0:cn, :],
                                         lhsT=xb[:, j * 128:j * 128 + cn],
                                         rhs=W1aug[:, :], start=True, stop=True)
                        if j % 2 == 0:
                            nc.scalar.activation(out=stbf[0:cn, j, :],
                                                 in_=ps[0:cn, 0:HC1], func=AF.Copy)
                        else:
                            nc.vector.tensor_copy(stbf[0:cn, j, :], ps[0:cn, 0:HC1])
                        nc.vector.tensor_copy(sts[0:cn, j, :].bitcast(F32),
                                              ps[0:cn, HC1:L1COL])
                    if cb == BBATCH * 128:
                        nc.scalar.dma_start(
                            out=tab[to:to + cb, 0:HC1].bitcast(BF16)
                                .rearrange("(b p) c -> p b c", p=128),
                            in_=stbf[:, :, :])
                        nc.sync.dma_start(
                            out=tab[to:to + cb, HC1:HC1 + 2 * HEADS]
                                .rearrange("(b p) c -> p b c", p=128),
                            in_=sts[:, :, :])
                    else:
                        for j in range(nch):
                            cn = min(128, cb - j * 128)
                            oj = to + j * 128
                            nc.scalar.dma_start(
                                out=tab[oj:oj + cn, 0:HC1].bitcast(BF16),
                                in_=stbf[0:cn, j, :])
                            nc.sync.dma_start(
                                out=tab[oj:oj + cn, HC1:HC1 + 2 * HEADS],
                                in_=sts[0:cn, j, :])

            # ---- stage C: layer-1 edge softmax + aggregation per dst window,
            # with the layer-2 projection (old stage D) fused per window ----
            NWIN_C = NWIN if 'C' in stages else 0
            with tc.tile_pool(name="winps", bufs=2, space="PSUM") as wps, \
                 tc.tile_pool(name="klps", bufs=2, space="PSUM") as kps, \
                 tc.tile_pool(name="trps", bufs=2, space="PSUM") as tps, \
                 tc.tile_pool(name="adps", bufs=2, space="PSUM") as aps, \
                 tc.tile_pool(name="gpool", bufs=3) as gpl, \
                 tc.tile_pool(name="spool", bufs=6) as spl, \
                 tc.tile_pool(name="winsb", bufs=3) as wsb:
                b0 = 0
                for w in range(NWIN_C):
                    BL = int(pp["nbw_low"][w])
                    BH = int(pp["nbw_high"][w])
                    nb = BL + BH
                    cn_w = min(128, NPC - w * 128)
                    # skip matmul + a_dst of own nodes (psKL also holds the
                    # fused layer-2 projection region: [0:68]=psK, [68:86]=psL)
                    xo = wsb.tile([F_IN, 128], F32, tag="xo")
                    nc.sync.dma_start(out=xo[:, 0:cn_w], in_=xTown[:, w * 128:w * 128 + cn_w])
                    psKL = kps.tile([128, K1COL + W2COL], F32, space="PSUM")
                    psK = psKL[:, 0:K1COL]
                    psL = psKL[:, K1COL:K1COL + W2COL]
                    nc.tensor.matmul(out=psK[0:cn_w, :], lhsT=xo[:, 0:cn_w], rhs=Wl1aug[:, :],
                                     start=True, stop=True)
                    stK = wsb.tile([128, HEADS], BF16, tag="stK")
                    nc.vector.tensor_copy(stK[0:cn_w, :], psK[0:cn_w, HID:K1COL])
                    # one-hot dst matrices: S from is_eq, St streamed from host
                    S = spl.tile([128, NBWmax, 128], BF16, tag="S")
                    nc.vector.tensor_tensor(
                        out=S[:, 0:nb, :],
                        in0=dstloc_sb[:, b0:b0 + nb].unsqueeze(2).to_broadcast([128, nb, 128]),
                        in1=iota_t[:, :].unsqueeze(1).to_broadcast([128, nb, 128]),
                        op=OP.is_equal)
                    Stw = spl.tile([128, NBWmax * 128], BF16, tag="Stw")
                    nc.sync.dma_start(out=Stw[:, 0:nb * 128],
                                      in_=Stdat_d[:, b0 * 128:(b0 + nb) * 128])
                    # per-edge a_dst via St.T @ a_dst-window-table
                    psAD = aps.tile([128, NBWmax * HEADS], F32, space="PSUM")
                    for j in range(nb):
                        nc.tensor.matmul(out=psAD[:, j * HEADS:(j + 1) * HEADS],
                                         lhsT=Stw[:, j * 128:(j + 1) * 128],
                                         rhs=stK[:, :], start=True, stop=True)
                    # gathers
                    G = gpl.tile([128, NBWmax, T1C], U16, tag="G")
                    if BL:
                        chunked_gather(nc, G, table1lo[:, :],
                                       gidx_sb, b0, BL, T1C, gregs, qrr)
                    if BH:
                        chunked_gather(nc, G, table1hi[:, :],
                                       gidx_sb, b0 + BL, BH, T1C, gregs, qrr, boff=BL)
                    # e = a_src + a_dst; ex = exp(lrelu(e)) = max(exp(e), exp(0.2e))
                    ex = wsb.tile([128, NBWmax, HEADS], F32, tag="ex")
                    nc.vector.tensor_tensor(
                        out=ex[:, 0:nb, :],
                        in0=G[:, 0:nb, HC1:HC1 + 8].bitcast(F32),
                        in1=psAD[:, 0:nb * HEADS].rearrange("p (b h) -> p b h", h=HEADS),
                        op=OP.add)
                    exa = wsb.tile([128, NBWmax, HEADS], BF16, tag="exa")
                    nc.scalar.activation(out=exa[:, 0:nb, :], in_=ex[:, 0:nb, :],
                                         func=AF.Exp)
                    exb = wsb.tile([128, NBWmax, HEADS], BF16, tag="exb")
                    nc.scalar.activation(out=exb[:, 0:nb, :], in_=ex[:, 0:nb, :],
                                         func=AF.Exp, scale=float(neg_slope))
                    nc.vector.tensor_tensor(out=exb[:, 0:nb, :], in0=exa[:, 0:nb, :],
                                            in1=exb[:, 0:nb, :], op=OP.max)
                    Gp = gpl.tile([128, NBWmax, HC1 + HEADS], BF16, tag="Gp")
                    nc.vector.tensor_tensor(
                        out=Gp[:, 0:nb, 0:HC1].rearrange("p b (h c) -> p b h c", h=HEADS),
                        in0=G[:, 0:nb, 0:HC1].bitcast(BF16).rearrange("p b (h c) -> p b h c", h=HEADS),
                        in1=exb[:, 0:nb, :].unsqueeze(3).to_broadcast([128, nb, HEADS, HID]),
                        op=OP.mult)
                    nc.vector.tensor_copy(Gp[:, 0:nb, HC1:HC1 + HEADS], exb[:, 0:nb, :])
                    psW = wps.tile([128, HC1 + HEADS], F32, space="PSUM")
                    for j in range(nb):
                        nc.tensor.matmul(out=psW[:, :], lhsT=S[:, j, :], rhs=Gp[:, j, :],
                                         start=(j == 0), stop=(j == nb - 1))
                    # extract: h = sigmoid(gat/4sum + skip + bias)
                    rec = wsb.tile([128, HEADS], F32, tag="rec")
                    nc.vector.tensor_scalar(out=rec[:, :], in0=psW[:, HC1:HC1 + HEADS],
                                            scalar1=1e-16, scalar2=float(HEADS),
                                            op0=OP.add, op1=OP.mult)
                    nc.vector.reciprocal(rec[:, :], rec[:, :])
                    gat = wsb.tile([128, HC1], F32, tag="gat")
                    nc.vector.tensor_tensor(
                        out=gat[:, :].rearrange("p (h c) -> p h c", h=HEADS),
                        in0=psW[:, 0:HC1].rearrange("p (h c) -> p h c", h=HEADS),
                        in1=rec[:, :].unsqueeze(2).to_broadcast([128, HEADS, HID]),
                        op=OP.mult)
                    hred = wsb.tile([128, HID], F32, tag="hred")
                    nc.vector.tensor_reduce(
                        out=hred[:, :],
                        in_=gat[:, :].rearrange("p (h c) -> p c h", h=HEADS),
                        axis=mybir.AxisListType.X, op=OP.add)
                    nc.vector.tensor_tensor(out=hred[:, :], in0=hred[:, :],
                                            in1=psK[:, 0:HID], op=OP.add)
                    nc.vector.tensor_tensor(out=hred[:, :], in0=hred[:, :],
                                            in1=bias1[:, :], op=OP.add)
                    hwin = wsb.tile([128, HID], F32, tag="hwin")
                    nc.scalar.activation(out=hwin[:, :], in_=hred[:, :], func=AF.Sigmoid)
                    psT = tps.tile([HID, 128], F32, space="PSUM")
                    nc.tensor.transpose(out=psT[:, :], in_=hwin[:, :], identity=ident[:, :])
                    htw = wsb.tile([HID, 128], F32, tag="htw")
                    nc.vector.tensor_copy(htw[:, :], psT[:, :])
                    # fused layer-2 projection of this window's own nodes
                    nc.tensor.matmul(out=psL[0:cn_w, :], lhsT=htw[:, 0:cn_w],
                                     rhs=W2aug[:, :], start=True, stop=True)
                    t2st = wsb.tile([128, T2W], U16, tag="t2st")
                    nc.vector.tensor_copy(t2st[0:cn_w, 0:HC2].bitcast(BF16),
                                          psL[0:cn_w, 0:HC2])
                    nc.vector.tensor_copy(t2st[0:cn_w, HC2:T2W].bitcast(F32),
                                          psL[0:cn_w, HC2:HC2 + HEADS])
                    if w < WSPLIT:
                        nc.sync.dma_start(out=t2cA[w * 128:w * 128 + cn_w, :],
                                          in_=t2st[0:cn_w, :])
                    else:
                        ob = (w - WSPLIT) * 128
                        nc.sync.dma_start(out=t2cB[ob:ob + cn_w, :],
                                          in_=t2st[0:cn_w, :])
                    nc.vector.tensor_copy(ad2sb[0:cn_w, w, :],
                                          psL[0:cn_w, HC2 + HEADS:HC2 + 2 * HEADS])
                    nc.vector.tensor_copy(skip2sb[0:cn_w, w, :],
                                          psL[0:cn_w, HC2 + 2 * HEADS:W2COL])
                    b0 += nb
                    if w == WSPLIT - 1 and 'D' in stages:
                        # chunk A complete on our core: allgather + expand it
                        # while the rest of stage C runs
                        nc.gpsimd.collective_compute(
                            "AllGather", OP.bypass, replica_groups=[list(range(8))],
                            ins=[t2cA[:, :]], outs=[tab2cA[:, :]])
                        nc.sync.dma_start(
                            out=table2[:, 0:T2W]
                                .rearrange("(c r) x -> c r x", c=8)[:, 0:ROWA, :],
                            in_=tab2cA[:, :].rearrange("(c r) x -> c r x", c=8))

            if 'D' in stages:
                nc.gpsimd.collective_compute(
                    "AllGather", OP.bypass, replica_groups=[list(range(8))],
                    ins=[t2cB[:, :]], outs=[tab2cB[:, :]])
                nc.sync.dma_start(
                    out=table2[:, 0:T2W]
                        .rearrange("(c r) x -> c r x", c=8)[:, ROWA:NPC, :],
                    in_=tab2cB[:, :].rearrange("(c r) x -> c r x", c=8))

            # ---- stage E: layer-2 edge softmax + aggregation ----
            NWIN_E = NWIN if 'E' in stages else 0
            with tc.tile_pool(name="w2ps", bufs=2, space="PSUM") as wps2, \
                 tc.tile_pool(name="ad2ps", bufs=2, space="PSUM") as aps2, \
                 tc.tile_pool(name="s2pool", bufs=6) as spl2, \
                 tc.tile_pool(name="w2sb", bufs=3) as w2sb:
                b0 = 0
                for w in range(NWIN_E):
                    BL = int(pp["nbw_low"][w])
                    BH = int(pp["nbw_high"][w])
                    nb = BL + BH
                    St2 = spl2.tile([128, NBWmax * 128], BF16, tag="St2")
                    nc.sync.dma_start(out=St2[:, 0:nb * 128],
                                      in_=Stdat_d[:, b0 * 128:(b0 + nb) * 128])
                    psAD2 = aps2.tile([128, NBWmax * HEADS], F32, space="PSUM")
                    for j in range(nb):
                        nc.tensor.matmul(out=psAD2[:, j * HEADS:(j + 1) * HEADS],
                                         lhsT=St2[:, j * 128:(j + 1) * 128],
                                         rhs=ad2sb[:, w, :], start=True, stop=True)
                    g2s = w2sb.tile([128, NBWmax, T2C], U16, tag="g2s")
                    if BL:
                        chunked_gather(nc, g2s, table2[0:min(SPLIT, N), :],
                                       gidx_sb, b0, BL, T2C, gregs, qrr)
                    if BH:
                        chunked_gather(nc, g2s, table2[SPLIT:N, :],
                                       gidx_sb, b0 + BL, BH, T2C, gregs, qrr, boff=BL)
                    ex2 = w2sb.tile([128, NBWmax, HEADS], F32, tag="ex2")
                    nc.vector.tensor_tensor(
                        out=ex2[:, 0:nb, :],
                        in0=g2s[:, 0:nb, HC2:16].bitcast(F32),
                        in1=psAD2[:, 0:nb * HEADS].rearrange("p (b h) -> p b h", h=HEADS),
                        op=OP.add)
                    ex2a = w2sb.tile([128, NBWmax, HEADS], BF16, tag="ex2a")
                    nc.scalar.activation(out=ex2a[:, 0:nb, :], in_=ex2[:, 0:nb, :],
                                         func=AF.Exp)
                    ex2b = w2sb.tile([128, NBWmax, HEADS], BF16, tag="ex2b")
                    nc.scalar.activation(out=ex2b[:, 0:nb, :], in_=ex2[:, 0:nb, :],
                                         func=AF.Exp, scale=float(neg_slope))
                    nc.vector.tensor_tensor(out=ex2b[:, 0:nb, :], in0=ex2a[:, 0:nb, :],
                                            in1=ex2b[:, 0:nb, :], op=OP.max)
                    R2 = w2sb.tile([128, NBWmax, R2COL], BF16, tag="R2")
                    nc.vector.tensor_tensor(
                        out=R2[:, 0:nb, 0:HC2].rearrange("p b (h c) -> p b h c", h=HEADS),
                        in0=g2s[:, 0:nb, 0:HC2].bitcast(BF16).rearrange("p b (h c) -> p b h c", h=HEADS),
                        in1=ex2b[:, 0:nb, :].unsqueeze(3).to_broadcast([128, nb, HEADS, OUT]),
                        op=OP.mult)
                    nc.vector.tensor_copy(R2[:, 0:nb, HC2:R2COL], ex2b[:, 0:nb, :])
                    S2 = spl2.tile([128, NBWmax, 128], BF16, tag="S2")
                    nc.vector.tensor_tensor(
                        out=S2[:, 0:nb, :],
                        in0=dstloc_sb[:, b0:b0 + nb].unsqueeze(2).to_broadcast([128, nb, 128]),
                        in1=iota_t[:, :].unsqueeze(1).to_broadcast([128, nb, 128]),
                        op=OP.is_equal)
                    psW2 = wps2.tile([128, R2COL], F32, space="PSUM")
                    for j in range(nb):
                        nc.tensor.matmul(out=psW2[:, :], lhsT=S2[:, j, :], rhs=R2[:, j, :],
                                         start=(j == 0), stop=(j == nb - 1))
                    rec2 = w2sb.tile([128, HEADS], F32, tag="rec2")
                    nc.vector.tensor_scalar(out=rec2[:, :], in0=psW2[:, HC2:R2COL],
                                            scalar1=1e-16, scalar2=float(HEADS),
                                            op0=OP.add, op1=OP.mult)
                    nc.vector.reciprocal(rec2[:, :], rec2[:, :])
                    og = w2sb.tile([128, HC2], F32, tag="og")
                    nc.vector.tensor_tensor(
                        out=og[:, :].rearrange("p (h c) -> p h c", h=HEADS),
                        in0=psW2[:, 0:HC2].rearrange("p (h c) -> p h c", h=HEADS),
                        in1=rec2[:, :].unsqueeze(2).to_broadcast([128, HEADS, OUT]),
                        op=OP.mult)
                    ored = w2sb.tile([128, OUT], F32, tag="ored")
                    nc.vector.tensor_reduce(
                        out=ored[:, :],
                        in_=og[:, :].rearrange("p (h c) -> p c h", h=HEADS),
                        axis=mybir.AxisListType.X, op=OP.add)
                    nc.vector.tensor_tensor(out=ored[:, :], in0=ored[:, :],
                                            in1=skip2sb[:, w, :], op=OP.add)
                    nc.vector.tensor_tensor(out=outsb[:, w, :], in0=ored[:, :],
                                            in1=bias2[:, :], op=OP.add)
                    b0 += nb

            # ---- final output DMA ----
            wf = NPC // 128 if 'E' in stages else 0
            rem = NPC % 128 if 'E' in stages else 0
            if wf:
                nc.sync.dma_start(
                    out=out_d[0:wf * 128, :].rearrange("(w p) c -> p w c", p=128),
                    in_=outsb[:, 0:wf, :])
            if rem:
                nc.sync.dma_start(out=out_d[wf * 128:NPC, :], in_=outsb[0:rem, wf, :])

    fix_library_reloads(nc)
    split_multi_waits(nc)
    return nc


def make_in_maps(pp, inputs, N, F_IN=128, HID=64, HEADS=4, OUT=2):
    NPC = pp["npc"]
    NB = pp["NB"]
    x = np.ascontiguousarray(np.asarray(inputs["x"], dtype=np.float32))
    xT = np.ascontiguousarray(x.T)
    f32 = lambda a, shp: np.ascontiguousarray(np.asarray(a, dtype=np.float32).reshape(shp))

    W1s = f32(inputs["W1s"], (F_IN, HEADS * HID))
    W1d = f32(inputs["W1d"], (F_IN, HEADS * HID))
    a1s = f32(inputs["a1s"], (HEADS, HID))
    a1d = f32(inputs["a1d"], (HEADS, HID))
    W2s = f32(inputs["W2s"], (HID, HEADS * OUT))
    W2d = f32(inputs["W2d"], (HID, HEADS * OUT))
    a2s = f32(inputs["a2s"], (HEADS, OUT))
    a2d = f32(inputs["a2d"], (HEADS, OUT))
    # host weight folding: a_src/a_dst projections as extra W columns
    fold1s = np.einsum('fhc,hc->fh', W1s.reshape(F_IN, HEADS, HID), a1s)
    fold1d = np.einsum('fhc,hc->fh', W1d.reshape(F_IN, HEADS, HID), a1d)
    fold2s = np.einsum('fhc,hc->fh', W2s.reshape(HID, HEADS, OUT), a2s)
    fold2d = np.einsum('fhc,hc->fh', W2d.reshape(HID, HEADS, OUT), a2d)
    W1aug = np.ascontiguousarray(np.concatenate([W1s, fold1s], axis=1))
    Wl1aug = np.ascontiguousarray(
        np.concatenate([f32(inputs["Wl1"], (F_IN, HID)), fold1d], axis=1))
    W2aug = np.ascontiguousarray(np.concatenate(
        [W2s, fold2s, fold2d, f32(inputs["Wl2"], (HID, OUT))], axis=1))
    bias1 = np.tile((f32(inputs["b1"], (1, HID)) + f32(inputs["bl1"], (1, HID))), (128, 1))
    bias2 = np.tile((f32(inputs["b2"], (1, OUT)) + f32(inputs["bl2"], (1, OUT))), (128, 1))

    common = {
        "xT": xT,
        "W1aug": W1aug,
        "Wl1aug": Wl1aug,
        "W2aug": W2aug,
        "bias1": np.ascontiguousarray(bias1),
        "bias2": np.ascontiguousarray(bias2),
    }
    import ml_dtypes
    in_maps = []
    for c in range(8):
        m = dict(common)
        m["xTown"] = np.ascontiguousarray(xT[:, c * NPC:(c + 1) * NPC])
        m["gidx"] = pp["gidx"][c]
        m["dstloc"] = pp["dstloc"][c]
        # transposed one-hot: St[d, b*128+p] = (dstloc[p, b] == d), bf16
        dl = pp["dstloc"][c]                       # [128, NB]
        St = np.zeros((128, NB * 128), np.uint16)
        pp_, bb_ = np.nonzero(dl >= 0)
        St[dl[pp_, bb_].astype(np.int64), bb_ * 128 + pp_] = 0x3F80  # 1.0 bf16
        m["Stdat"] = St.view(ml_dtypes.bfloat16)
        in_maps.append(m)
    return in_maps


_BUILD_CACHE = {}
LAST_RESULTS = None


def kernel(**inputs):
    """Full inputs in, full [N, 2] float32 output out."""
    global LAST_RESULTS
    trace = bool(inputs.pop("_trace", False))
    pp = preprocess(inputs["edge_index"], N_NODES)
    key = (pp["NB"], tuple(pp["nbw_low"]), tuple(pp["nbw_high"]))
    if key not in _BUILD_CACHE:
        _BUILD_CACHE[key] = build(pp, N_NODES)
    nc = _BUILD_CACHE[key]
    in_maps = make_in_maps(pp, inputs, N_NODES)
    res = run_bass_kernel_spmd(nc, in_maps, list(range(8)), trace=trace)
    LAST_RESULTS = res
    out = np.concatenate([res.results[c]["out"] for c in range(8)], axis=0)
    return out.astype(np.float32)


# revision 29
# speedup vs baseline: 2.9554x; 1.0101x over previous
"""Self-contained Trainium2 Bass kernel for the 2-layer GAT problem
(nn_GAT_26714696581831). 8-core SPMD: edges sorted by dst, 8 dst-range
shards; per-window one-hot matmul aggregation with dma_gather row fetches
spread across 4 SWDGE queues (4 Q7 core pairs generate descriptors in
parallel).

kernel(**inputs) takes the FULL unsharded inputs and returns the FULL
[50000, 2] output.
"""
import sys
sys.path.insert(0, '/opt/trn_rl_repo')
import numpy as np
import concourse.bass as bass
import concourse.mybir as mybir
import concourse.tile as tile
from concourse import library_config
from concourse.masks import make_identity
from concourse.bass_utils import run_bass_kernel_spmd

N_NODES = 50000
"""Workarounds for this walrus build, which rejects any instruction carrying
more than one sync-wait command: hoist extra waits onto same-engine NoOps
inserted immediately before the instruction."""


_ctr = [0]

def split_multi_waits(nc, max_waits=1):
    for fn in nc.m.functions:
        for bb in fn.blocks:
            insts = bb.instructions
            i = 0
            while i < len(insts):
                ins = insts[i]
                si = ins.sync_info
                if si is not None and si.on_wait and len(si.on_wait) > max_waits:
                    waits = list(si.on_wait)
                    keep = waits[-max_waits:]
                    hoist = waits[:-max_waits]
                    si.on_wait = keep
                    for w in hoist:
                        _ctr[0] += 1
                        n = mybir.InstNoOp(name=f"waitsplit-{_ctr[0]}", ins=[], outs=[])
                        n.engine = ins.engine
                        n.sync_info = mybir.SyncInfo(on_wait=[w], on_update=[])
                        insts.insert(i, n)
                        i += 1
                i += 1


def fix_library_reloads(nc):
    """bass_rust leaves InstPseudoReloadLibraryIndex.instr empty; this walrus
    rejects zero-length ISA instructions. Encode the 64-byte
    PSEUDO_LIBRARY_RELOAD_INDEX struct with the live ISA tables."""
    isa = nc.isa
    sn = 'NEURON_ISA_TPB_PSEUDO_LIBRARY_RELOAD_INDEX_STRUCT'
    e = isa.get_enum("NEURON_ISA_TPB_PSEUDO_OPCODE")
    val = e.NEURON_ISA_TPB_PSEUDO_OPCODE_PSEUDO_LIBRARY_RELOAD_INDEX.value
    for fn in nc.m.functions:
        for bb in fn.blocks:
            for ins in bb.instructions:
                if type(ins).__name__ == 'InstPseudoReloadLibraryIndex' and not ins.instr:
                    b = isa.asm({"header": {"opcode": 223, "inst_word_len": 16},
                                 "pseudo_opcode": val,
                                 "lib_index": ins.lib_index}, sn)
                    ins.instr = [int(x) for x in b]




WIN = 128                  # dst nodes per window
SPLIT = 32768              # int16 positive limit for gather indices


def preprocess(edge_index, n_nodes, ncores=8):
    src = np.asarray(edge_index[0], dtype=np.int64)
    dst = np.asarray(edge_index[1], dtype=np.int64)
    npc = n_nodes // ncores
    nwin = (npc + WIN - 1) // WIN

    order = np.argsort(dst, kind="stable")
    src_s = src[order]
    dst_s = dst[order]

    counts = np.bincount(dst_s // npc, minlength=ncores)
    core_slices = np.concatenate([[0], np.cumsum(counts)])

    nlow = np.zeros((ncores, nwin), dtype=np.int64)
    nhigh = np.zeros((ncores, nwin), dtype=np.int64)
    per_core_win_edges = []
    for c in range(ncores):
        s0, s1 = core_slices[c], core_slices[c + 1]
        csrc = src_s[s0:s1]
        cdst = dst_s[s0:s1]
        wloc = (cdst - c * npc) // WIN
        dloc = (cdst - c * npc) % WIN
        wins = []
        for w in range(nwin):
            m = wloc == w
            ws, wd = csrc[m], dloc[m]
            lo = ws < SPLIT
            wins.append((ws[lo], ws[~lo] - SPLIT, wd[lo], wd[~lo]))
            nlow[c, w] = lo.sum()
            nhigh[c, w] = (~lo).sum()
        per_core_win_edges.append(wins)

    nbw_low = ((nlow.max(axis=0) + 127) // 128).astype(int)
    nbw_high = ((nhigh.max(axis=0) + 127) // 128).astype(int)
    for w in range(nwin):
        if nbw_low[w] + nbw_high[w] == 0:
            nbw_low[w] = 1
    NB = int(nbw_low.sum() + nbw_high.sum())

    gidx_lin = np.zeros((ncores, NB * 128), dtype=np.int16)
    srcidx_lin = np.zeros((ncores, NB * 128), dtype=np.int32)
    dstidx_lin = np.zeros((ncores, NB * 128), dtype=np.int32)
    dstloc_lin = np.full((ncores, NB * 128), -1, dtype=np.int16)

    for c in range(ncores):
        b0 = 0
        for w in range(nwin):
            slo, shi, dlo, dhi = per_core_win_edges[c][w]
            o = b0 * 128
            gidx_lin[c, o:o + len(slo)] = slo.astype(np.int16)
            srcidx_lin[c, o:o + len(slo)] = slo
            dstidx_lin[c, o:o + len(dlo)] = dlo + w * WIN + c * npc
            dstloc_lin[c, o:o + len(dlo)] = dlo.astype(np.int16)
            b0 += int(nbw_low[w])
            o = b0 * 128
            gidx_lin[c, o:o + len(shi)] = shi.astype(np.int16)
            srcidx_lin[c, o:o + len(shi)] = shi + SPLIT
            dstidx_lin[c, o:o + len(dhi)] = dhi + w * WIN + c * npc
            dstloc_lin[c, o:o + len(dhi)] = dhi.astype(np.int16)
            b0 += int(nbw_high[w])
        assert b0 == NB

    def wrap16(lin):  # [NC, NB*128] -> [NC, 128, NB*8] dma_gather layout
        x = lin.reshape(ncores, NB * 8, 16).transpose(0, 2, 1)
        return np.ascontiguousarray(np.tile(x, (1, 8, 1)))

    # dst-local indices for the a_dst gather (per-core local table, < 32768)
    adidx_lin = np.empty((ncores, NB * 128), dtype=np.int16)
    for c in range(ncores):
        loc = dstidx_lin[c].astype(np.int64) - c * npc
        loc[dstloc_lin[c] < 0] = 0          # pad slots -> row 0
        adidx_lin[c] = loc.astype(np.int16)

    def slotlay(lin, dtype):  # [NC, NB*128] -> [NC, 128, NB] ([p,b] = slot b*128+p)
        return np.ascontiguousarray(lin.reshape(ncores, NB, 128).transpose(0, 2, 1)).astype(dtype)

    return dict(
        NB=NB, nwin=nwin, npc=npc, ncores=ncores,
        nbw_low=nbw_low, nbw_high=nbw_high,
        gidx=wrap16(gidx_lin),
        adidx=wrap16(adidx_lin),
        srcidx=slotlay(srcidx_lin, np.int32),
        dstidx=slotlay(dstidx_lin, np.int32),
        dstloc=slotlay(dstloc_lin, np.int16),
    )




F32 = mybir.dt.float32
BF16 = mybir.dt.bfloat16
I16 = mybir.dt.int16
U16 = mybir.dt.uint16
AF = mybir.ActivationFunctionType
OP = mybir.AluOpType

SPLIT = 32768
GCHUNK = 8   # blocks per dma_gather call (1024 idx: single-packet-safe)
NQ = 4       # SWDGE queues: queue q's descriptors are generated by Q7 core
             # pair (2q, 2q+1); round-robin spreads desc-gen over all 8 cores


def chunked_gather(nc, out_tile, in_ap, idx_sb, b0, nblk, elem, regs, qrr, boff=0):
    """Issue dma_gather in <=GCHUNK-block chunks writing out_tile[:, boff+i...].

    Chunk sizes are balanced (11 -> 6+5, not 8+3) and queues strictly
    rotate so consecutive calls always hit different Q7 desc-gen core
    pairs - in-order instruction retirement then pipelines ~4 deep.
    """
    nchunks = (nblk + GCHUNK - 1) // GCHUNK
    base, rem = divmod(nblk, nchunks)
    done = 0
    for i in range(nchunks):
        step = base + (1 if i < rem else 0)
        n = step * 128
        if n not in regs:
            regs[n] = nc.gpsimd.to_reg(n)
        nc.gpsimd.dma_gather(
            out_tile[:, boff + done:boff + done + step, :], in_ap,
            idx_sb[:, (b0 + done) * 8:(b0 + done + step) * 8],
            n, regs[n], elem, queue_num=qrr[0] % NQ)
        qrr[0] += 1
        done += step


def build(pp, N, F_IN=128, HID=64, HEADS=4, OUT=2, neg_slope=0.2, stages='ABCDE'):
    NB = pp["NB"]
    NWIN = pp["nwin"]
    NPC = pp["npc"]
    HC1 = HEADS * HID          # 256
    HC2 = HEADS * OUT          # 8
    NBWmax = int(max(pp["nbw_low"][w] + pp["nbw_high"][w] for w in range(NWIN)))
    NCHUNK = (N + 127) // 128
    T1C = HC1 + 128            # 384 u16 cols = 768B rows
    L1COL = HC1 + HEADS        # 260: xs + a_src fold
    K1COL = HID + HEADS        # 68  (skip + W_ad fold)
    W2COL = HC2 + 2 * HEADS + OUT  # 18
    R2COL = HC2 + HEADS        # 12
    T2C = 128                  # u16 cols = 256B rows
    T2W = 16                   # used u16 cols of a table2 row (z2 bf16 + a2s f32)
    BBATCH = 8                 # stage-B chunks per DMA batch
    WSPLIT = NWIN // 2         # C-windows per t2 chunk (chunked allgather)
    ROWA = WSPLIT * 128        # own rows in chunk A
    ROWB = NPC - ROWA          # own rows in chunk B

    nc = bass.Bass("TRN2", target_bir_lowering=False, debug=False,
                   num_devices=8, num_swdge_queues=NQ)

    # ---- I/O ----
    xT = nc.dram_tensor("xT", [F_IN, N], F32, kind="ExternalInput")
    xTown = nc.dram_tensor("xTown", [F_IN, NPC], F32, kind="ExternalInput")
    W1aug_d = nc.dram_tensor("W1aug", [F_IN, L1COL], F32, kind="ExternalInput")
    Wl1aug_d = nc.dram_tensor("Wl1aug", [F_IN, K1COL], F32, kind="ExternalInput")
    W2aug_d = nc.dram_tensor("W2aug", [HID, W2COL], F32, kind="ExternalInput")
    bias1_d = nc.dram_tensor("bias1", [128, HID], F32, kind="ExternalInput")
    bias2_d = nc.dram_tensor("bias2", [128, OUT], F32, kind="ExternalInput")
    gidx_d = nc.dram_tensor("gidx", [128, NB * 8], I16, kind="ExternalInput")
    # one-hot dst matrices, host-built and streamed: S[slot, d] is the
    # scatter-matmul lhsT; St[d, slot] fetches per-edge a_dst via matmul
    Sdat_d = nc.dram_tensor("Sdat", [128, NB * 128], BF16, kind="ExternalInput")
    Stdat_d = nc.dram_tensor("Stdat", [128, NB * 128], BF16, kind="ExternalInput")
    out_d = nc.dram_tensor("out", [NPC, OUT], F32, kind="ExternalOutput")

    # internal DRAM. table1 split lo/hi so low-src gathers can start while
    # stage B is still projecting the high node range.
    table1lo = nc.dram_tensor("table1lo", [SPLIT, T1C], U16)
    table1hi = nc.dram_tensor("table1hi", [N - SPLIT, T1C], U16)
    # layer-2 tables: compact [*, 16] payload allgathered in two window
    # chunks (A fires mid-stage-C), then expanded into 256B gather rows.
    t2cA = nc.dram_tensor("t2cA", [ROWA, T2W], U16)
    t2cB = nc.dram_tensor("t2cB", [ROWB, T2W], U16)
    tab2cA = nc.dram_tensor("tab2cA", [8 * ROWA, T2W], U16, addr_space="Shared")
    tab2cB = nc.dram_tensor("tab2cB", [8 * ROWB, T2W], U16, addr_space="Shared")
    table2 = nc.dram_tensor("table2", [N, T2C], U16)

    with tile.TileContext(nc) as tc:
        with tc.tile_pool(name="const", bufs=1) as cpool, \
             tc.tile_pool(name="resident", bufs=1) as rpool:

            # ---- constants (all weight folding/permutation done on host) ----
            W1aug = cpool.tile([F_IN, L1COL], F32)
            nc.sync.dma_start(out=W1aug[:, :], in_=W1aug_d[:, :])
            Wl1aug = cpool.tile([F_IN, K1COL], F32)
            nc.sync.dma_start(out=Wl1aug[:, :], in_=Wl1aug_d[:, :])
            W2aug = cpool.tile([HID, W2COL], F32)
            nc.sync.dma_start(out=W2aug[:, :], in_=W2aug_d[:, :])
            bias1 = cpool.tile([128, HID], F32)
            nc.sync.dma_start(out=bias1[:, :], in_=bias1_d[:, :])
            bias2 = cpool.tile([128, OUT], F32)
            nc.sync.dma_start(out=bias2[:, :], in_=bias2_d[:, :])

            ident = cpool.tile([128, 128], F32)
            make_identity(nc, ident[:, :])

            gidx_sb = rpool.tile([128, NB * 8], I16)
            nc.sync.dma_start(out=gidx_sb[:, :], in_=gidx_d[:, :])

            # all standard-library gpsimd ops (iota/affine_select/memset) are
            # above; from here on the Q7 carveout holds the mlp library.
            nc.gpsimd.load_library(library_config.mlp)
            gregs = {}
            qrr = [0, 0, 0, 0]

            if 'C' in stages:
                skip2sb = rpool.tile([128, NWIN, OUT], F32)
                outsb = rpool.tile([128, NWIN, OUT], F32)
                ad2sb = rpool.tile([128, NWIN, HEADS], BF16)
                # zero-fill: the last ragged window leaves tail partitions
                # unwritten and 0 x NaN would poison the psAD2 contraction
                nc.vector.memset(ad2sb[:, :, :], 0.0)

            # ---- stage B: project all N nodes, table1 row = [xs | a_src] bf16 ----
            assert SPLIT % (128 * BBATCH) == 0
            NBAT = (NCHUNK + BBATCH - 1) // BBATCH if 'B' in stages else 0
            with tc.tile_pool(name="projps", bufs=4, space="PSUM") as ppp, \
                 tc.tile_pool(name="projsb", bufs=3) as psb:
                for i in range(NBAT):
                    o = i * 128 * BBATCH
                    cb = min(128 * BBATCH, N - o)          # rows this batch
                    nch = (cb + 127) // 128
                    tab, to = (table1lo, o) if o < SPLIT else (table1hi, o - SPLIT)
                    xb = psb.tile([F_IN, BBATCH * 128], F32, tag="xb")
                    nc.sync.dma_start(out=xb[:, 0:cb], in_=xT[:, o:o + cb])
                    stbf = psb.tile([128, BBATCH, L1COL], BF16, tag="stbf")
                    for j in range(nch):
                        cn = min(128, cb - j * 128)
                        ps = ppp.tile([128, L1COL], F32, space="PSUM")
                        nc.tensor.matmul(out=ps[0:cn, :],
                                         lhsT=xb[:, j * 128:j * 128 + cn],
                                         rhs=W1aug[:, :], start=True, stop=True)
                        if j % 2 == 0:
                            nc.scalar.activation(out=stbf[0:cn, j, :],
                                                 in_=ps[0:cn, :], func=AF.Copy)
                        else:
                            nc.vector.tensor_copy(stbf[0:cn, j, :], ps[0:cn, :])
                    if cb == BBATCH * 128:
                        nc.scalar.dma_start(
                            out=tab[to:to + cb, 0:L1COL].bitcast(BF16)
                                .rearrange("(b p) c -> p b c", p=128),
                            in_=stbf[:, :, :])
                    else:
                        for j in range(nch):
                            cn = min(128, cb - j * 128)
                            oj = to + j * 128
                            nc.scalar.dma_start(
                                out=tab[oj:oj + cn, 0:L1COL].bitcast(BF16),
                                in_=stbf[0:cn, j, :])

            # ---- stage C: layer-1 edge softmax + aggregation per dst window,
            # with the layer-2 projection (old stage D) fused per window ----
            NWIN_C = NWIN if 'C' in stages else 0
            with tc.tile_pool(name="winps", bufs=2, space="PSUM") as wps, \
                 tc.tile_pool(name="klps", bufs=2, space="PSUM") as kps, \
                 tc.tile_pool(name="trps", bufs=2, space="PSUM") as tps, \
                 tc.tile_pool(name="adps", bufs=2, space="PSUM") as aps, \
                 tc.tile_pool(name="gpool", bufs=3) as gpl, \
                 tc.tile_pool(name="spool", bufs=6) as spl, \
                 tc.tile_pool(name="winsb", bufs=3) as wsb:
                b0 = 0
                for w in range(NWIN_C):
                    BL = int(pp["nbw_low"][w])
                    BH = int(pp["nbw_high"][w])
                    nb = BL + BH
                    cn_w = min(128, NPC - w * 128)
                    # skip matmul + a_dst of own nodes (psKL also holds the
                    # fused layer-2 projection region: [0:68]=psK, [68:86]=psL)
                    xo = wsb.tile([F_IN, 128], F32, tag="xo")
                    nc.sync.dma_start(out=xo[:, 0:cn_w], in_=xTown[:, w * 128:w * 128 + cn_w])
                    psKL = kps.tile([128, K1COL + W2COL], F32, space="PSUM")
                    psK = psKL[:, 0:K1COL]
                    psL = psKL[:, K1COL:K1COL + W2COL]
                    nc.tensor.matmul(out=psK[0:cn_w, :], lhsT=xo[:, 0:cn_w], rhs=Wl1aug[:, :],
                                     start=True, stop=True)
                    stK = wsb.tile([128, HEADS], BF16, tag="stK")
                    nc.vector.tensor_copy(stK[0:cn_w, :], psK[0:cn_w, HID:K1COL])
                    # one-hot dst matrices, streamed from host
                    S = spl.tile([128, NBWmax, 128], BF16, tag="S")
                    nc.scalar.dma_start(
                        out=S[:, 0:nb, :].rearrange("p b d -> p (b d)"),
                        in_=Sdat_d[:, b0 * 128:(b0 + nb) * 128])
                    Stw = spl.tile([128, NBWmax * 128], BF16, tag="Stw")
                    nc.scalar.dma_start(out=Stw[:, 0:nb * 128],
                                        in_=Stdat_d[:, b0 * 128:(b0 + nb) * 128])
                    # per-edge a_dst via St.T @ a_dst-window-table
                    psAD = aps.tile([128, NBWmax * HEADS], F32, space="PSUM")
                    for j in range(nb):
                        nc.tensor.matmul(out=psAD[:, j * HEADS:(j + 1) * HEADS],
                                         lhsT=Stw[:, j * 128:(j + 1) * 128],
                                         rhs=stK[:, :], start=True, stop=True)
                    # gathers
                    G = gpl.tile([128, NBWmax, T1C], U16, tag="G")
                    if BL:
                        chunked_gather(nc, G, table1lo[:, :],
                                       gidx_sb, b0, BL, T1C, gregs, qrr)
                    if BH:
                        chunked_gather(nc, G, table1hi[:, :],
                                       gidx_sb, b0 + BL, BH, T1C, gregs, qrr, boff=BL)
                    # e = a_src + a_dst; ex = exp(lrelu(e)) = max(exp(e), exp(0.2e))
                    ex = wsb.tile([128, NBWmax, HEADS], F32, tag="ex")
                    nc.vector.tensor_tensor(
                        out=ex[:, 0:nb, :],
                        in0=G[:, 0:nb, HC1:HC1 + HEADS].bitcast(BF16),
                        in1=psAD[:, 0:nb * HEADS].rearrange("p (b h) -> p b h", h=HEADS),
                        op=OP.add)
                    exa = wsb.tile([128, NBWmax, HEADS], BF16, tag="exa")
                    nc.scalar.activation(out=exa[:, 0:nb, :], in_=ex[:, 0:nb, :],
                                         func=AF.Exp)
                    exb = wsb.tile([128, NBWmax, HEADS], BF16, tag="exb")
                    nc.scalar.activation(out=exb[:, 0:nb, :], in_=ex[:, 0:nb, :],
                                         func=AF.Exp, scale=float(neg_slope))
                    nc.vector.tensor_tensor(out=exb[:, 0:nb, :], in0=exa[:, 0:nb, :],
                                            in1=exb[:, 0:nb, :], op=OP.max)
                    Gp = gpl.tile([128, NBWmax, HC1 + HEADS], BF16, tag="Gp")
                    nc.vector.tensor_tensor(
                        out=Gp[:, 0:nb, 0:HC1].rearrange("p b (h c) -> p b h c", h=HEADS),
                        in0=G[:, 0:nb, 0:HC1].bitcast(BF16).rearrange("p b (h c) -> p b h c", h=HEADS),
                        in1=exb[:, 0:nb, :].unsqueeze(3).to_broadcast([128, nb, HEADS, HID]),
                        op=OP.mult)
                    nc.vector.tensor_copy(Gp[:, 0:nb, HC1:HC1 + HEADS], exb[:, 0:nb, :])
                    psW = wps.tile([128, HC1 + HEADS], F32, space="PSUM")
                    for j in range(nb):
                        nc.tensor.matmul(out=psW[:, :], lhsT=S[:, j, :], rhs=Gp[:, j, :],
                                         start=(j == 0), stop=(j == nb - 1))
                    # extract: h = sigmoid(gat/4sum + skip + bias)
                    rec = wsb.tile([128, HEADS], F32, tag="rec")
                    nc.vector.tensor_scalar(out=rec[:, :], in0=psW[:, HC1:HC1 + HEADS],
                                            scalar1=1e-16, scalar2=float(HEADS),
                                            op0=OP.add, op1=OP.mult)
                    nc.vector.reciprocal(rec[:, :], rec[:, :])
                    gat = wsb.tile([128, HC1], F32, tag="gat")
                    nc.vector.tensor_tensor(
                        out=gat[:, :].rearrange("p (h c) -> p h c", h=HEADS),
                        in0=psW[:, 0:HC1].rearrange("p (h c) -> p h c", h=HEADS),
                        in1=rec[:, :].unsqueeze(2).to_broadcast([128, HEADS, HID]),
                        op=OP.mult)
                    hred = wsb.tile([128, HID], F32, tag="hred")
                    nc.vector.tensor_reduce(
                        out=hred[:, :],
                        in_=gat[:, :].rearrange("p (h c) -> p c h", h=HEADS),
                        axis=mybir.AxisListType.X, op=OP.add)
                    nc.vector.tensor_tensor(out=hred[:, :], in0=hred[:, :],
                                            in1=psK[:, 0:HID], op=OP.add)
                    nc.vector.tensor_tensor(out=hred[:, :], in0=hred[:, :],
                                            in1=bias1[:, :], op=OP.add)
                    hwin = wsb.tile([128, HID], F32, tag="hwin")
                    nc.scalar.activation(out=hwin[:, :], in_=hred[:, :], func=AF.Sigmoid)
                    psT = tps.tile([HID, 128], F32, space="PSUM")
                    nc.tensor.transpose(out=psT[:, :], in_=hwin[:, :], identity=ident[:, :])
                    htw = wsb.tile([HID, 128], F32, tag="htw")
                    nc.vector.tensor_copy(htw[:, :], psT[:, :])
                    # fused layer-2 projection of this window's own nodes
                    nc.tensor.matmul(out=psL[0:cn_w, :], lhsT=htw[:, 0:cn_w],
                                     rhs=W2aug[:, :], start=True, stop=True)
                    t2st = wsb.tile([128, T2W], U16, tag="t2st")
                    nc.vector.tensor_copy(t2st[0:cn_w, 0:HC2].bitcast(BF16),
                                          psL[0:cn_w, 0:HC2])
                    nc.vector.tensor_copy(t2st[0:cn_w, HC2:T2W].bitcast(F32),
                                          psL[0:cn_w, HC2:HC2 + HEADS])
                    if w < WSPLIT:
                        nc.sync.dma_start(out=t2cA[w * 128:w * 128 + cn_w, :],
                                          in_=t2st[0:cn_w, :])
                    else:
                        ob = (w - WSPLIT) * 128
                        nc.sync.dma_start(out=t2cB[ob:ob + cn_w, :],
                                          in_=t2st[0:cn_w, :])
                    nc.vector.tensor_copy(ad2sb[0:cn_w, w, :],
                                          psL[0:cn_w, HC2 + HEADS:HC2 + 2 * HEADS])
                    nc.vector.tensor_copy(skip2sb[0:cn_w, w, :],
                                          psL[0:cn_w, HC2 + 2 * HEADS:W2COL])
                    b0 += nb
                    if w == WSPLIT - 1 and 'D' in stages:
                        # chunk A complete on our core: allgather + expand it
                        # while the rest of stage C runs
                        nc.gpsimd.collective_compute(
                            "AllGather", OP.bypass, replica_groups=[list(range(8))],
                            ins=[t2cA[:, :]], outs=[tab2cA[:, :]])
                        nc.sync.dma_start(
                            out=table2[:, 0:T2W]
                                .rearrange("(c r) x -> c r x", c=8)[:, 0:ROWA, :],
                            in_=tab2cA[:, :].rearrange("(c r) x -> c r x", c=8))

            if 'D' in stages:
                nc.gpsimd.collective_compute(
                    "AllGather", OP.bypass, replica_groups=[list(range(8))],
                    ins=[t2cB[:, :]], outs=[tab2cB[:, :]])
                nc.sync.dma_start(
                    out=table2[:, 0:T2W]
                        .rearrange("(c r) x -> c r x", c=8)[:, ROWA:NPC, :],
                    in_=tab2cB[:, :].rearrange("(c r) x -> c r x", c=8))

            # ---- stage E: layer-2 edge softmax + aggregation ----
            NWIN_E = NWIN if 'E' in stages else 0
            with tc.tile_pool(name="w2ps", bufs=2, space="PSUM") as wps2, \
                 tc.tile_pool(name="ad2ps", bufs=2, space="PSUM") as aps2, \
                 tc.tile_pool(name="s2pool", bufs=6) as spl2, \
                 tc.tile_pool(name="w2sb", bufs=3) as w2sb:
                b0 = 0
                for w in range(NWIN_E):
                    BL = int(pp["nbw_low"][w])
                    BH = int(pp["nbw_high"][w])
                    nb = BL + BH
                    St2 = spl2.tile([128, NBWmax * 128], BF16, tag="St2")
                    nc.scalar.dma_start(out=St2[:, 0:nb * 128],
                                        in_=Stdat_d[:, b0 * 128:(b0 + nb) * 128])
                    psAD2 = aps2.tile([128, NBWmax * HEADS], F32, space="PSUM")
                    for j in range(nb):
                        nc.tensor.matmul(out=psAD2[:, j * HEADS:(j + 1) * HEADS],
                                         lhsT=St2[:, j * 128:(j + 1) * 128],
                                         rhs=ad2sb[:, w, :], start=True, stop=True)
                    g2s = w2sb.tile([128, NBWmax, T2C], U16, tag="g2s")
                    if BL:
                        chunked_gather(nc, g2s, table2[0:min(SPLIT, N), :],
                                       gidx_sb, b0, BL, T2C, gregs, qrr)
                    if BH:
                        chunked_gather(nc, g2s, table2[SPLIT:N, :],
                                       gidx_sb, b0 + BL, BH, T2C, gregs, qrr, boff=BL)
                    ex2 = w2sb.tile([128, NBWmax, HEADS], F32, tag="ex2")
                    nc.vector.tensor_tensor(
                        out=ex2[:, 0:nb, :],
                        in0=g2s[:, 0:nb, HC2:16].bitcast(F32),
                        in1=psAD2[:, 0:nb * HEADS].rearrange("p (b h) -> p b h", h=HEADS),
                        op=OP.add)
                    ex2a = w2sb.tile([128, NBWmax, HEADS], BF16, tag="ex2a")
                    nc.scalar.activation(out=ex2a[:, 0:nb, :], in_=ex2[:, 0:nb, :],
                                         func=AF.Exp)
                    ex2b = w2sb.tile([128, NBWmax, HEADS], BF16, tag="ex2b")
                    nc.scalar.activation(out=ex2b[:, 0:nb, :], in_=ex2[:, 0:nb, :],
                                         func=AF.Exp, scale=float(neg_slope))
                    nc.vector.tensor_tensor(out=ex2b[:, 0:nb, :], in0=ex2a[:, 0:nb, :],
                                            in1=ex2b[:, 0:nb, :], op=OP.max)
                    R2 = w2sb.tile([128, NBWmax, R2COL], BF16, tag="R2")
                    nc.vector.tensor_tensor(
                        out=R2[:, 0:nb, 0:HC2].rearrange("p b (h c) -> p b h c", h=HEADS),
                        in0=g2s[:, 0:nb, 0:HC2].bitcast(BF16).rearrange("p b (h c) -> p b h c", h=HEADS),
                        in1=ex2b[:, 0:nb, :].unsqueeze(3).to_broadcast([128, nb, HEADS, OUT]),
                        op=OP.mult)
                    nc.vector.tensor_copy(R2[:, 0:nb, HC2:R2COL], ex2b[:, 0:nb, :])
                    S2 = spl2.tile([128, NBWmax, 128], BF16, tag="S2")
                    nc.scalar.dma_start(
                        out=S2[:, 0:nb, :].rearrange("p b d -> p (b d)"),
                        in_=Sdat_d[:, b0 * 128:(b0 + nb) * 128])
                    psW2 = wps2.tile([128, R2COL], F32, space="PSUM")
                    for j in range(nb):
                        nc.tensor.matmul(out=psW2[:, :], lhsT=S2[:, j, :], rhs=R2[:, j, :],
                                         start=(j == 0), stop=(j == nb - 1))
                    rec2 = w2sb.tile([128, HEADS], F32, tag="rec2")
                    nc.vector.tensor_scalar(out=rec2[:, :], in0=psW2[:, HC2:R2COL],
                                            scalar1=1e-16, scalar2=float(HEADS),
                                            op0=OP.add, op1=OP.mult)
                    nc.vector.reciprocal(rec2[:, :], rec2[:, :])
                    og = w2sb.tile([128, HC2], F32, tag="og")
                    nc.vector.tensor_tensor(
                        out=og[:, :].rearrange("p (h c) -> p h c", h=HEADS),
                        in0=psW2[:, 0:HC2].rearrange("p (h c) -> p h c", h=HEADS),
                        in1=rec2[:, :].unsqueeze(2).to_broadcast([128, HEADS, OUT]),
                        op=OP.mult)
                    ored = w2sb.tile([128, OUT], F32, tag="ored")
                    nc.vector.tensor_reduce(
                        out=ored[:, :],
                        in_=og[:, :].rearrange("p (h c) -> p c h", h=HEADS),
                        axis=mybir.AxisListType.X, op=OP.add)
                    nc.vector.tensor_tensor(out=ored[:, :], in0=ored[:, :],
                                            in1=skip2sb[:, w, :], op=OP.add)
                    nc.vector.tensor_tensor(out=outsb[:, w, :], in0=ored[:, :],
                                            in1=bias2[:, :], op=OP.add)
                    b0 += nb

            # ---- final output DMA ----
            wf = NPC // 128 if 'E' in stages else 0
            rem = NPC % 128 if 'E' in stages else 0
            if wf:
                nc.sync.dma_start(
                    out=out_d[0:wf * 128, :].rearrange("(w p) c -> p w c", p=128),
                    in_=outsb[:, 0:wf, :])
            if rem:
                nc.sync.dma_start(out=out_d[wf * 128:NPC, :], in_=outsb[0:rem, wf, :])

    fix_library_reloads(nc)
    split_multi_waits(nc)
    return nc


def make_in_maps(pp, inputs, N, F_IN=128, HID=64, HEADS=4, OUT=2):
    NPC = pp["npc"]
    NB = pp["NB"]
    x = np.ascontiguousarray(np.asarray(inputs["x"], dtype=np.float32))
    xT = np.ascontiguousarray(x.T)
    f32 = lambda a, shp: np.ascontiguousarray(np.asarray(a, dtype=np.float32).reshape(shp))

    W1s = f32(inputs["W1s"], (F_IN, HEADS * HID))
    W1d = f32(inputs["W1d"], (F_IN, HEADS * HID))
    a1s = f32(inputs["a1s"], (HEADS, HID))
    a1d = f32(inputs["a1d"], (HEADS, HID))
    W2s = f32(inputs["W2s"], (HID, HEADS * OUT))
    W2d = f32(inputs["W2d"], (HID, HEADS * OUT))
    a2s = f32(inputs["a2s"], (HEADS, OUT))
    a2d = f32(inputs["a2d"], (HEADS, OUT))
    # host weight folding: a_src/a_dst projections as extra W columns
    fold1s = np.einsum('fhc,hc->fh', W1s.reshape(F_IN, HEADS, HID), a1s)
    fold1d = np.einsum('fhc,hc->fh', W1d.reshape(F_IN, HEADS, HID), a1d)
    fold2s = np.einsum('fhc,hc->fh', W2s.reshape(HID, HEADS, OUT), a2s)
    fold2d = np.einsum('fhc,hc->fh', W2d.reshape(HID, HEADS, OUT), a2d)
    W1aug = np.ascontiguousarray(np.concatenate([W1s, fold1s], axis=1))
    Wl1aug = np.ascontiguousarray(
        np.concatenate([f32(inputs["Wl1"], (F_IN, HID)), fold1d], axis=1))
    W2aug = np.ascontiguousarray(np.concatenate(
        [W2s, fold2s, fold2d, f32(inputs["Wl2"], (HID, OUT))], axis=1))
    bias1 = np.tile((f32(inputs["b1"], (1, HID)) + f32(inputs["bl1"], (1, HID))), (128, 1))
    bias2 = np.tile((f32(inputs["b2"], (1, OUT)) + f32(inputs["bl2"], (1, OUT))), (128, 1))

    common = {
        "xT": xT,
        "W1aug": W1aug,
        "Wl1aug": Wl1aug,
        "W2aug": W2aug,
        "bias1": np.ascontiguousarray(bias1),
        "bias2": np.ascontiguousarray(bias2),
    }
    import ml_dtypes
    in_maps = []
    for c in range(8):
        m = dict(common)
        m["xTown"] = np.ascontiguousarray(xT[:, c * NPC:(c + 1) * NPC])
        m["gidx"] = pp["gidx"][c]
        # one-hot dst matrices, bf16 (0x3F80 = 1.0):
        #   St[d, b*128+p] = (dstloc[p, b] == d)   (a_dst fetch lhsT)
        #   S[p, b*128+d]  = (dstloc[p, b] == d)   (scatter lhsT)
        dl = pp["dstloc"][c]                       # [128, NB]
        pp_, bb_ = np.nonzero(dl >= 0)
        dv = dl[pp_, bb_].astype(np.int64)
        St = np.zeros((128, NB * 128), np.uint16)
        St[dv, bb_ * 128 + pp_] = 0x3F80
        m["Stdat"] = St.view(ml_dtypes.bfloat16)
        S = np.zeros((128, NB * 128), np.uint16)
        S[pp_, bb_ * 128 + dv] = 0x3F80
        m["Sdat"] = S.view(ml_dtypes.bfloat16)
        in_maps.append(m)
    return in_maps


_BUILD_CACHE = {}
LAST_RESULTS = None


def kernel(**inputs):
    """Full inputs in, full [N, 2] float32 output out."""
    global LAST_RESULTS
    trace = bool(inputs.pop("_trace", False))
    pp = preprocess(inputs["edge_index"], N_NODES)
    key = (pp["NB"], tuple(pp["nbw_low"]), tuple(pp["nbw_high"]))
    if key not in _BUILD_CACHE:
        _BUILD_CACHE[key] = build(pp, N_NODES)
    nc = _BUILD_CACHE[key]
    in_maps = make_in_maps(pp, inputs, N_NODES)
    res = run_bass_kernel_spmd(nc, in_maps, list(range(8)), trace=trace)
    LAST_RESULTS = res
    out = np.concatenate([res.results[c]["out"] for c in range(8)], axis=0)
    return out.astype(np.float32)


# revision 38
# speedup vs baseline: 3.3280x; 1.1261x over previous
"""Self-contained Trainium2 Bass kernel for the 2-layer GAT problem
(nn_GAT_26714696581831). 8-core SPMD: edges sorted by dst, 8 dst-range
shards; per-window one-hot matmul aggregation with dma_gather row fetches
spread across 4 SWDGE queues (4 Q7 core pairs generate descriptors in
parallel).

kernel(**inputs) takes the FULL unsharded inputs and returns the FULL
[50000, 2] output.
"""
import sys
sys.path.insert(0, '/opt/trn_rl_repo')
import numpy as np
import concourse.bass as bass
import concourse.mybir as mybir
import concourse.tile as tile
from concourse import library_config
from concourse.masks import make_identity
from concourse.bass_utils import run_bass_kernel_spmd

N_NODES = 50000
"""Workarounds for this walrus build, which rejects any instruction carrying
more than one sync-wait command: hoist extra waits onto same-engine NoOps
inserted immediately before the instruction."""


_ctr = [0]

def split_multi_waits(nc, max_waits=1):
    for fn in nc.m.functions:
        for bb in fn.blocks:
            insts = bb.instructions
            i = 0
            while i < len(insts):
                ins = insts[i]
                si = ins.sync_info
                if si is not None and si.on_wait and len(si.on_wait) > max_waits:
                    waits = list(si.on_wait)
                    keep = waits[-max_waits:]
                    hoist = waits[:-max_waits]
                    si.on_wait = keep
                    for w in hoist:
                        _ctr[0] += 1
                        n = mybir.InstNoOp(name=f"waitsplit-{_ctr[0]}", ins=[], outs=[])
                        n.engine = ins.engine
                        n.sync_info = mybir.SyncInfo(on_wait=[w], on_update=[])
                        insts.insert(i, n)
                        i += 1
                i += 1


def fix_library_reloads(nc):
    """bass_rust leaves InstPseudoReloadLibraryIndex.instr empty; this walrus
    rejects zero-length ISA instructions. Encode the 64-byte
    PSEUDO_LIBRARY_RELOAD_INDEX struct with the live ISA tables."""
    isa = nc.isa
    sn = 'NEURON_ISA_TPB_PSEUDO_LIBRARY_RELOAD_INDEX_STRUCT'
    e = isa.get_enum("NEURON_ISA_TPB_PSEUDO_OPCODE")
    val = e.NEURON_ISA_TPB_PSEUDO_OPCODE_PSEUDO_LIBRARY_RELOAD_INDEX.value
    for fn in nc.m.functions:
        for bb in fn.blocks:
            for ins in bb.instructions:
                if type(ins).__name__ == 'InstPseudoReloadLibraryIndex' and not ins.instr:
                    b = isa.asm({"header": {"opcode": 223, "inst_word_len": 16},
                                 "pseudo_opcode": val,
                                 "lib_index": ins.lib_index}, sn)
                    ins.instr = [int(x) for x in b]




WIN = 128                  # dst nodes per window
SPLIT = 32768              # int16 positive limit for gather indices


def preprocess(edge_index, n_nodes, ncores=8):
    src = np.asarray(edge_index[0], dtype=np.int64)
    dst = np.asarray(edge_index[1], dtype=np.int64)
    npc = n_nodes // ncores
    nwin = (npc + WIN - 1) // WIN

    order = np.argsort(dst, kind="stable")
    src_s = src[order]
    dst_s = dst[order]

    counts = np.bincount(dst_s // npc, minlength=ncores)
    core_slices = np.concatenate([[0], np.cumsum(counts)])

    nlow = np.zeros((ncores, nwin), dtype=np.int64)
    nhigh = np.zeros((ncores, nwin), dtype=np.int64)
    per_core_win_edges = []
    for c in range(ncores):
        s0, s1 = core_slices[c], core_slices[c + 1]
        csrc = src_s[s0:s1]
        cdst = dst_s[s0:s1]
        wloc = (cdst - c * npc) // WIN
        dloc = (cdst - c * npc) % WIN
        wins = []
        for w in range(nwin):
            m = wloc == w
            ws, wd = csrc[m], dloc[m]
            lo = ws < SPLIT
            wins.append((ws[lo], ws[~lo] - SPLIT, wd[lo], wd[~lo]))
            nlow[c, w] = lo.sum()
            nhigh[c, w] = (~lo).sum()
        per_core_win_edges.append(wins)

    nbw_low = ((nlow.max(axis=0) + 127) // 128).astype(int)
    nbw_high = ((nhigh.max(axis=0) + 127) // 128).astype(int)
    for w in range(nwin):
        if nbw_low[w] + nbw_high[w] == 0:
            nbw_low[w] = 1
    NB = int(nbw_low.sum() + nbw_high.sum())

    gidx_lin = np.zeros((ncores, NB * 128), dtype=np.int16)
    srcidx_lin = np.zeros((ncores, NB * 128), dtype=np.int32)
    dstidx_lin = np.zeros((ncores, NB * 128), dtype=np.int32)
    dstloc_lin = np.full((ncores, NB * 128), -1, dtype=np.int16)

    for c in range(ncores):
        b0 = 0
        for w in range(nwin):
            slo, shi, dlo, dhi = per_core_win_edges[c][w]
            o = b0 * 128
            gidx_lin[c, o:o + len(slo)] = slo.astype(np.int16)
            srcidx_lin[c, o:o + len(slo)] = slo
            dstidx_lin[c, o:o + len(dlo)] = dlo + w * WIN + c * npc
            dstloc_lin[c, o:o + len(dlo)] = dlo.astype(np.int16)
            b0 += int(nbw_low[w])
            o = b0 * 128
            gidx_lin[c, o:o + len(shi)] = shi.astype(np.int16)
            srcidx_lin[c, o:o + len(shi)] = shi + SPLIT
            dstidx_lin[c, o:o + len(dhi)] = dhi + w * WIN + c * npc
            dstloc_lin[c, o:o + len(dhi)] = dhi.astype(np.int16)
            b0 += int(nbw_high[w])
        assert b0 == NB

    def wrap16(lin):  # [NC, NB*128] -> [NC, 128, NB*8] dma_gather layout
        x = lin.reshape(ncores, NB * 8, 16).transpose(0, 2, 1)
        return np.ascontiguousarray(np.tile(x, (1, 8, 1)))

    # dst-local indices for the a_dst gather (per-core local table, < 32768)
    adidx_lin = np.empty((ncores, NB * 128), dtype=np.int16)
    for c in range(ncores):
        loc = dstidx_lin[c].astype(np.int64) - c * npc
        loc[dstloc_lin[c] < 0] = 0          # pad slots -> row 0
        adidx_lin[c] = loc.astype(np.int16)

    def slotlay(lin, dtype):  # [NC, NB*128] -> [NC, 128, NB] ([p,b] = slot b*128+p)
        return np.ascontiguousarray(lin.reshape(ncores, NB, 128).transpose(0, 2, 1)).astype(dtype)

    return dict(
        NB=NB, nwin=nwin, npc=npc, ncores=ncores,
        nbw_low=nbw_low, nbw_high=nbw_high,
        gidx=wrap16(gidx_lin),
        adidx=wrap16(adidx_lin),
        srcidx=slotlay(srcidx_lin, np.int32),
        dstidx=slotlay(dstidx_lin, np.int32),
        dstloc=slotlay(dstloc_lin, np.int16),
    )




F32 = mybir.dt.float32
BF16 = mybir.dt.bfloat16
I16 = mybir.dt.int16
U16 = mybir.dt.uint16
AF = mybir.ActivationFunctionType
OP = mybir.AluOpType

SPLIT = 32768
GCHUNK = 8   # blocks per dma_gather call (1024 idx: single-packet-safe)
NQ = 4       # SWDGE queues: queue q's descriptors are generated by Q7 core
             # pair (2q, 2q+1); round-robin spreads desc-gen over all 8 cores


def chunked_gather(nc, out_tile, in_ap, idx_sb, b0, nblk, elem, regs, qrr, boff=0):
    """Issue dma_gather in <=GCHUNK-block chunks writing out_tile[:, boff+i...].

    Chunk sizes are balanced (11 -> 6+5, not 8+3) and queues strictly
    rotate so consecutive calls always hit different Q7 desc-gen core
    pairs - in-order instruction retirement then pipelines ~4 deep.
    """
    nchunks = (nblk + GCHUNK - 1) // GCHUNK
    base, rem = divmod(nblk, nchunks)
    done = 0
    for i in range(nchunks):
        step = base + (1 if i < rem else 0)
        n = step * 128
        if n not in regs:
            regs[n] = nc.gpsimd.to_reg(n)
        nc.gpsimd.dma_gather(
            out_tile[:, boff + done:boff + done + step, :], in_ap,
            idx_sb[:, (b0 + done) * 8:(b0 + done + step) * 8],
            n, regs[n], elem, queue_num=qrr[0] % NQ)
        qrr[0] += 1
        done += step


def build(pp, N, F_IN=128, HID=64, HEADS=4, OUT=2, neg_slope=0.2, stages='ABCDE'):
    NB = pp["NB"]
    NWIN = pp["nwin"]
    NPC = pp["npc"]
    HC1 = HEADS * HID          # 256
    HC2 = HEADS * OUT          # 8
    NBWmax = int(max(pp["nbw_low"][w] + pp["nbw_high"][w] for w in range(NWIN)))
    NCHUNK = (N + 127) // 128
    T1C = HC1 + 128            # 384 u16 cols = 768B rows
    L1COL = HC1 + HEADS        # 260: xs + a_src fold
    K1COL = HID + HEADS        # 68  (skip + W_ad fold)
    W2COL = HC2 + 2 * HEADS + OUT  # 18
    R2COL = HC2 + HEADS        # 12
    T2C = 128                  # u16 cols = 256B rows
    T2W = 16                   # used u16 cols of a table2 row (z2 bf16 + a2s f32)
    BBATCH = 16                # stage-B chunks per DMA batch
    WSPLIT = NWIN // 2         # C-windows per t2 chunk (chunked allgather)
    ROWA = WSPLIT * 128        # own rows in chunk A
    ROWB = NPC - ROWA          # own rows in chunk B

    nc = bass.Bass("TRN2", target_bir_lowering=False, debug=False,
                   num_devices=8, num_swdge_queues=NQ)

    # ---- I/O ----
    xT = nc.dram_tensor("xT", [F_IN, N], F32, kind="ExternalInput")
    xTown = nc.dram_tensor("xTown", [F_IN, NPC], F32, kind="ExternalInput")
    W1aug_d = nc.dram_tensor("W1aug", [F_IN, L1COL], F32, kind="ExternalInput")
    Wl1aug_d = nc.dram_tensor("Wl1aug", [F_IN, K1COL], F32, kind="ExternalInput")
    W2aug_d = nc.dram_tensor("W2aug", [HID, W2COL], F32, kind="ExternalInput")
    bias1_d = nc.dram_tensor("bias1", [128, HID], F32, kind="ExternalInput")
    bias2_d = nc.dram_tensor("bias2", [128, OUT], F32, kind="ExternalInput")
    gidx_d = nc.dram_tensor("gidx", [128, NB * 8], I16, kind="ExternalInput")
    dstloc_d = nc.dram_tensor("dstloc", [128, NB], I16, kind="ExternalInput")
    # transposed one-hot dst matrices St[d, slot] (host-built, streamed):
    # fetch per-edge a_dst via a tiny matmul instead of a dma_gather
    Stdat_d = nc.dram_tensor("Stdat", [128, NB * 128], BF16, kind="ExternalInput")
    out_d = nc.dram_tensor("out", [NPC, OUT], F32, kind="ExternalOutput")

    # internal DRAM. table1 split lo/hi so low-src gathers can start while
    # stage B is still projecting the high node range.
    table1lo = nc.dram_tensor("table1lo", [SPLIT, T1C], U16)
    table1hi = nc.dram_tensor("table1hi", [N - SPLIT, T1C], U16)
    # layer-2 tables: compact [*, 16] payload allgathered in two window
    # chunks (A fires mid-stage-C), then expanded into 256B gather rows.
    t2cA = nc.dram_tensor("t2cA", [ROWA, T2W], U16)
    t2cB = nc.dram_tensor("t2cB", [ROWB, T2W], U16)
    tab2cA = nc.dram_tensor("tab2cA", [8 * ROWA, T2W], U16, addr_space="Shared")
    tab2cB = nc.dram_tensor("tab2cB", [8 * ROWB, T2W], U16, addr_space="Shared")
    table2 = nc.dram_tensor("table2", [N, T2C], U16)

    with tile.TileContext(nc) as tc:
        with tc.tile_pool(name="const", bufs=1) as cpool, \
             tc.tile_pool(name="resident", bufs=1) as rpool:

            # ---- constants (all weight folding/permutation done on host) ----
            W1aug = cpool.tile([F_IN, L1COL], F32)
            nc.sync.dma_start(out=W1aug[:, :], in_=W1aug_d[:, :])
            Wl1aug = cpool.tile([F_IN, K1COL], F32)
            nc.sync.dma_start(out=Wl1aug[:, :], in_=Wl1aug_d[:, :])
            W2aug = cpool.tile([HID, W2COL], F32)
            nc.sync.dma_start(out=W2aug[:, :], in_=W2aug_d[:, :])
            bias1 = cpool.tile([128, HID], F32)
            nc.sync.dma_start(out=bias1[:, :], in_=bias1_d[:, :])
            bias2 = cpool.tile([128, OUT], F32)
            nc.sync.dma_start(out=bias2[:, :], in_=bias2_d[:, :])

            iota_t = cpool.tile([128, 128], I16)
            nc.gpsimd.iota(iota_t[:, :], pattern=[[1, 128]], base=0, channel_multiplier=0)
            ident = cpool.tile([128, 128], F32)
            make_identity(nc, ident[:, :])

            gidx_sb = rpool.tile([128, NB * 8], I16)
            nc.sync.dma_start(out=gidx_sb[:, :], in_=gidx_d[:, :])
            dstloc_sb = rpool.tile([128, NB], I16)
            nc.sync.dma_start(out=dstloc_sb[:, :], in_=dstloc_d[:, :])

            # all standard-library gpsimd ops (iota/affine_select/memset) are
            # above; from here on the Q7 carveout holds the mlp library.
            nc.gpsimd.load_library(library_config.mlp)
            gregs = {}
            qrr = [0, 0, 0, 0]

            if 'C' in stages:
                skip2sb = rpool.tile([128, NWIN, OUT], F32)
                outsb = rpool.tile([128, NWIN, OUT], F32)
                ad2sb = rpool.tile([128, NWIN, HEADS], BF16)
                # zero-fill: the last ragged window leaves tail partitions
                # unwritten and 0 x NaN would poison the psAD2 contraction
                nc.vector.memset(ad2sb[:, :, :], 0.0)

            # ---- stage B: project all N nodes, table1 row = [xs | a_src] bf16 ----
            assert SPLIT % (128 * BBATCH) == 0
            NBAT = (NCHUNK + BBATCH - 1) // BBATCH if 'B' in stages else 0
            with tc.tile_pool(name="projps", bufs=4, space="PSUM") as ppp, \
                 tc.tile_pool(name="projsb", bufs=3) as psb:
                for i in range(NBAT):
                    o = i * 128 * BBATCH
                    cb = min(128 * BBATCH, N - o)          # rows this batch
                    nch = (cb + 127) // 128
                    tab, to = (table1lo, o) if o < SPLIT else (table1hi, o - SPLIT)
                    xb = psb.tile([F_IN, BBATCH * 128], F32, tag="xb")
                    nc.sync.dma_start(out=xb[:, 0:cb], in_=xT[:, o:o + cb])
                    stbf = psb.tile([128, BBATCH, L1COL], BF16, tag="stbf")
                    for j in range(nch):
                        cn = min(128, cb - j * 128)
                        ps = ppp.tile([128, L1COL], F32, space="PSUM")
                        nc.tensor.matmul(out=ps[0:cn, :],
                                         lhsT=xb[:, j * 128:j * 128 + cn],
                                         rhs=W1aug[:, :], start=True, stop=True)
                        if j % 2 == 0:
                            nc.scalar.activation(out=stbf[0:cn, j, :],
                                                 in_=ps[0:cn, :], func=AF.Copy)
                        else:
                            nc.vector.tensor_copy(stbf[0:cn, j, :], ps[0:cn, :])
                    if cb == BBATCH * 128:
                        nc.scalar.dma_start(
                            out=tab[to:to + cb, 0:L1COL].bitcast(BF16)
                                .rearrange("(b p) c -> p b c", p=128),
                            in_=stbf[:, :, :])
                    else:
                        for j in range(nch):
                            cn = min(128, cb - j * 128)
                            oj = to + j * 128
                            nc.scalar.dma_start(
                                out=tab[oj:oj + cn, 0:L1COL].bitcast(BF16),
                                in_=stbf[0:cn, j, :])

            # ---- stage C: layer-1 edge softmax + aggregation per dst window,
            # with the layer-2 projection (old stage D) fused per window ----
            NWIN_C = NWIN if 'C' in stages else 0
            with tc.tile_pool(name="winps", bufs=2, space="PSUM") as wps, \
                 tc.tile_pool(name="klps", bufs=3, space="PSUM") as kps, \
                 tc.tile_pool(name="adps", bufs=3, space="PSUM") as aps, \
                 tc.tile_pool(name="gpool", bufs=4) as gpl, \
                 tc.tile_pool(name="stpool", bufs=3) as stpl, \
                 tc.tile_pool(name="spool", bufs=4) as spl, \
                 tc.tile_pool(name="winsb", bufs=3) as wsb:
                b0 = 0
                for w in range(NWIN_C):
                    BL = int(pp["nbw_low"][w])
                    BH = int(pp["nbw_high"][w])
                    nb = BL + BH
                    cn_w = min(128, NPC - w * 128)
                    # one PSUM tile per window: [0:68]=psK (skip+a_dst own),
                    # [68:86]=psL (fused layer-2 proj), [96:224]=psT (h transpose)
                    xo = wsb.tile([F_IN, 128], F32, tag="xo")
                    nc.sync.dma_start(out=xo[:, 0:cn_w], in_=xTown[:, w * 128:w * 128 + cn_w])
                    psKL = kps.tile([128, 224], F32, space="PSUM")
                    psK = psKL[:, 0:K1COL]
                    psL = psKL[:, K1COL:K1COL + W2COL]
                    psT = psKL[0:HID, 96:224]
                    nc.tensor.matmul(out=psK[0:cn_w, :], lhsT=xo[:, 0:cn_w], rhs=Wl1aug[:, :],
                                     start=True, stop=True)
                    stK = wsb.tile([128, HEADS], BF16, tag="stK")
                    nc.vector.tensor_copy(stK[0:cn_w, :], psK[0:cn_w, HID:K1COL])
                    # one-hot dst matrices: S built on DVE, St streamed from host
                    S = spl.tile([128, NBWmax, 128], BF16, tag="S")
                    nc.vector.tensor_tensor(
                        out=S[:, 0:nb, :],
                        in0=dstloc_sb[:, b0:b0 + nb].unsqueeze(2).to_broadcast([128, nb, 128]),
                        in1=iota_t[:, :].unsqueeze(1).to_broadcast([128, nb, 128]),
                        op=OP.is_equal)
                    Stw = stpl.tile([128, NBWmax * 128], BF16, tag="Stw")
                    nc.scalar.dma_start(out=Stw[:, 0:nb * 128],
                                        in_=Stdat_d[:, b0 * 128:(b0 + nb) * 128])
                    # per-edge a_dst via St.T @ a_dst-window-table
                    psAD = aps.tile([128, NBWmax * HEADS], F32, space="PSUM")
                    for j in range(nb):
                        nc.tensor.matmul(out=psAD[:, j * HEADS:(j + 1) * HEADS],
                                         lhsT=Stw[:, j * 128:(j + 1) * 128],
                                         rhs=stK[:, :], start=True, stop=True)
                    # gathers
                    G = gpl.tile([128, NBWmax, T1C], U16, tag="G")
                    if BL:
                        chunked_gather(nc, G, table1lo[:, :],
                                       gidx_sb, b0, BL, T1C, gregs, qrr)
                    if BH:
                        chunked_gather(nc, G, table1hi[:, :],
                                       gidx_sb, b0 + BL, BH, T1C, gregs, qrr, boff=BL)
                    # e = a_src + a_dst; ex = exp(lrelu(e)) = max(exp(e), exp(0.2e))
                    ex = wsb.tile([128, NBWmax, HEADS], F32, tag="ex")
                    nc.vector.tensor_tensor(
                        out=ex[:, 0:nb, :],
                        in0=G[:, 0:nb, HC1:HC1 + HEADS].bitcast(BF16),
                        in1=psAD[:, 0:nb * HEADS].rearrange("p (b h) -> p b h", h=HEADS),
                        op=OP.add)
                    exa = wsb.tile([128, NBWmax, HEADS], BF16, tag="exa")
                    nc.scalar.activation(out=exa[:, 0:nb, :], in_=ex[:, 0:nb, :],
                                         func=AF.Exp)
                    exb = wsb.tile([128, NBWmax, HEADS], BF16, tag="exb")
                    nc.scalar.activation(out=exb[:, 0:nb, :], in_=ex[:, 0:nb, :],
                                         func=AF.Exp, scale=float(neg_slope))
                    nc.vector.tensor_tensor(out=exb[:, 0:nb, :], in0=exa[:, 0:nb, :],
                                            in1=exb[:, 0:nb, :], op=OP.max)
                    Gp = gpl.tile([128, NBWmax, HC1 + HEADS], BF16, tag="Gp")
                    nc.vector.tensor_tensor(
                        out=Gp[:, 0:nb, 0:HC1].rearrange("p b (h c) -> p b h c", h=HEADS),
                        in0=G[:, 0:nb, 0:HC1].bitcast(BF16).rearrange("p b (h c) -> p b h c", h=HEADS),
                        in1=exb[:, 0:nb, :].unsqueeze(3).to_broadcast([128, nb, HEADS, HID]),
                        op=OP.mult)
                    nc.vector.tensor_copy(Gp[:, 0:nb, HC1:HC1 + HEADS], exb[:, 0:nb, :])
                    psW = wps.tile([128, HC1 + HEADS], F32, space="PSUM")
                    for j in range(nb):
                        nc.tensor.matmul(out=psW[:, :], lhsT=S[:, j, :], rhs=Gp[:, j, :],
                                         start=(j == 0), stop=(j == nb - 1))
                    # extract: h = sigmoid(gat/4sum + skip + bias)
                    rec = wsb.tile([128, HEADS], F32, tag="rec")
                    nc.vector.tensor_scalar(out=rec[:, :], in0=psW[:, HC1:HC1 + HEADS],
                                            scalar1=1e-16, scalar2=float(HEADS),
                                            op0=OP.add, op1=OP.mult)
                    nc.vector.reciprocal(rec[:, :], rec[:, :])
                    gat = wsb.tile([128, HC1], F32, tag="gat")
                    nc.vector.tensor_tensor(
                        out=gat[:, :].rearrange("p (h c) -> p h c", h=HEADS),
                        in0=psW[:, 0:HC1].rearrange("p (h c) -> p h c", h=HEADS),
                        in1=rec[:, :].unsqueeze(2).to_broadcast([128, HEADS, HID]),
                        op=OP.mult)
                    hred = wsb.tile([128, HID], F32, tag="hred")
                    nc.vector.tensor_reduce(
                        out=hred[:, :],
                        in_=gat[:, :].rearrange("p (h c) -> p c h", h=HEADS),
                        axis=mybir.AxisListType.X, op=OP.add)
                    nc.vector.tensor_tensor(out=hred[:, :], in0=hred[:, :],
                                            in1=psK[:, 0:HID], op=OP.add)
                    nc.vector.tensor_tensor(out=hred[:, :], in0=hred[:, :],
                                            in1=bias1[:, :], op=OP.add)
                    hwin = wsb.tile([128, HID], F32, tag="hwin")
                    nc.scalar.activation(out=hwin[:, :], in_=hred[:, :], func=AF.Sigmoid)
                    nc.tensor.transpose(out=psT[:, :], in_=hwin[:, :], identity=ident[:, :])
                    htw = wsb.tile([HID, 128], F32, tag="htw")
                    nc.vector.tensor_copy(htw[:, :], psT[:, :])
                    # fused layer-2 projection of this window's own nodes
                    nc.tensor.matmul(out=psL[0:cn_w, :], lhsT=htw[:, 0:cn_w],
                                     rhs=W2aug[:, :], start=True, stop=True)
                    t2st = wsb.tile([128, T2W], U16, tag="t2st")
                    nc.vector.tensor_copy(t2st[0:cn_w, 0:HC2].bitcast(BF16),
                                          psL[0:cn_w, 0:HC2])
                    nc.vector.tensor_copy(t2st[0:cn_w, HC2:T2W].bitcast(F32),
                                          psL[0:cn_w, HC2:HC2 + HEADS])
                    if w < WSPLIT:
                        nc.sync.dma_start(out=t2cA[w * 128:w * 128 + cn_w, :],
                                          in_=t2st[0:cn_w, :])
                    else:
                        ob = (w - WSPLIT) * 128
                        nc.sync.dma_start(out=t2cB[ob:ob + cn_w, :],
                                          in_=t2st[0:cn_w, :])
                    nc.vector.tensor_copy(ad2sb[0:cn_w, w, :],
                                          psL[0:cn_w, HC2 + HEADS:HC2 + 2 * HEADS])
                    nc.vector.tensor_copy(skip2sb[0:cn_w, w, :],
                                          psL[0:cn_w, HC2 + 2 * HEADS:W2COL])
                    b0 += nb
                    if w == WSPLIT - 1 and 'D' in stages:
                        # chunk A complete on our core: allgather + expand it
                        # while the rest of stage C runs
                        nc.gpsimd.collective_compute(
                            "AllGather", OP.bypass, replica_groups=[list(range(8))],
                            ins=[t2cA[:, :]], outs=[tab2cA[:, :]])
                        nc.sync.dma_start(
                            out=table2[:, 0:T2W]
                                .rearrange("(c r) x -> c r x", c=8)[:, 0:ROWA, :],
                            in_=tab2cA[:, :].rearrange("(c r) x -> c r x", c=8))

            if 'D' in stages:
                nc.gpsimd.collective_compute(
                    "AllGather", OP.bypass, replica_groups=[list(range(8))],
                    ins=[t2cB[:, :]], outs=[tab2cB[:, :]])
                nc.sync.dma_start(
                    out=table2[:, 0:T2W]
                        .rearrange("(c r) x -> c r x", c=8)[:, ROWA:NPC, :],
                    in_=tab2cB[:, :].rearrange("(c r) x -> c r x", c=8))

            # ---- stage E: layer-2 edge softmax + aggregation ----
            NWIN_E = NWIN if 'E' in stages else 0
            with tc.tile_pool(name="w2ps", bufs=3, space="PSUM") as wps2, \
                 tc.tile_pool(name="ad2ps", bufs=3, space="PSUM") as aps2, \
                 tc.tile_pool(name="s2pool", bufs=4) as spl2, \
                 tc.tile_pool(name="st2pool", bufs=3) as stpl2, \
                 tc.tile_pool(name="w2sb", bufs=4) as w2sb:
                b0 = 0
                for w in range(NWIN_E):
                    BL = int(pp["nbw_low"][w])
                    BH = int(pp["nbw_high"][w])
                    nb = BL + BH
                    St2 = stpl2.tile([128, NBWmax * 128], BF16, tag="St2")
                    nc.scalar.dma_start(out=St2[:, 0:nb * 128],
                                        in_=Stdat_d[:, b0 * 128:(b0 + nb) * 128])
                    psAD2 = aps2.tile([128, NBWmax * HEADS], F32, space="PSUM")
                    for j in range(nb):
                        nc.tensor.matmul(out=psAD2[:, j * HEADS:(j + 1) * HEADS],
                                         lhsT=St2[:, j * 128:(j + 1) * 128],
                                         rhs=ad2sb[:, w, :], start=True, stop=True)
                    g2s = w2sb.tile([128, NBWmax, T2C], U16, tag="g2s")
                    if BL:
                        chunked_gather(nc, g2s, table2[0:min(SPLIT, N), :],
                                       gidx_sb, b0, BL, T2C, gregs, qrr)
                    if BH:
                        chunked_gather(nc, g2s, table2[SPLIT:N, :],
                                       gidx_sb, b0 + BL, BH, T2C, gregs, qrr, boff=BL)
                    ex2 = w2sb.tile([128, NBWmax, HEADS], F32, tag="ex2")
                    nc.vector.tensor_tensor(
                        out=ex2[:, 0:nb, :],
                        in0=g2s[:, 0:nb, HC2:16].bitcast(F32),
                        in1=psAD2[:, 0:nb * HEADS].rearrange("p (b h) -> p b h", h=HEADS),
                        op=OP.add)
                    ex2a = w2sb.tile([128, NBWmax, HEADS], BF16, tag="ex2a")
                    nc.scalar.activation(out=ex2a[:, 0:nb, :], in_=ex2[:, 0:nb, :],
                                         func=AF.Exp)
                    ex2b = w2sb.tile([128, NBWmax, HEADS], BF16, tag="ex2b")
                    nc.scalar.activation(out=ex2b[:, 0:nb, :], in_=ex2[:, 0:nb, :],
                                         func=AF.Exp, scale=float(neg_slope))
                    nc.vector.tensor_tensor(out=ex2b[:, 0:nb, :], in0=ex2a[:, 0:nb, :],
                                            in1=ex2b[:, 0:nb, :], op=OP.max)
                    R2 = w2sb.tile([128, NBWmax, R2COL], BF16, tag="R2")
                    nc.vector.tensor_tensor(
                        out=R2[:, 0:nb, 0:HC2].rearrange("p b (h c) -> p b h c", h=HEADS),
                        in0=g2s[:, 0:nb, 0:HC2].bitcast(BF16).rearrange("p b (h c) -> p b h c", h=HEADS),
                        in1=ex2b[:, 0:nb, :].unsqueeze(3).to_broadcast([128, nb, HEADS, OUT]),
                        op=OP.mult)
                    nc.vector.tensor_copy(R2[:, 0:nb, HC2:R2COL], ex2b[:, 0:nb, :])
                    S2 = spl2.tile([128, NBWmax, 128], BF16, tag="S2")
                    nc.vector.tensor_tensor(
                        out=S2[:, 0:nb, :],
                        in0=dstloc_sb[:, b0:b0 + nb].unsqueeze(2).to_broadcast([128, nb, 128]),
                        in1=iota_t[:, :].unsqueeze(1).to_broadcast([128, nb, 128]),
                        op=OP.is_equal)
                    psW2 = wps2.tile([128, R2COL], F32, space="PSUM")
                    for j in range(nb):
                        nc.tensor.matmul(out=psW2[:, :], lhsT=S2[:, j, :], rhs=R2[:, j, :],
                                         start=(j == 0), stop=(j == nb - 1))
                    rec2 = w2sb.tile([128, HEADS], F32, tag="rec2")
                    nc.vector.tensor_scalar(out=rec2[:, :], in0=psW2[:, HC2:R2COL],
                                            scalar1=1e-16, scalar2=float(HEADS),
                                            op0=OP.add, op1=OP.mult)
                    nc.vector.reciprocal(rec2[:, :], rec2[:, :])
                    og = w2sb.tile([128, HC2], F32, tag="og")
                    nc.vector.tensor_tensor(
                        out=og[:, :].rearrange("p (h c) -> p h c", h=HEADS),
                        in0=psW2[:, 0:HC2].rearrange("p (h c) -> p h c", h=HEADS),
                        in1=rec2[:, :].unsqueeze(2).to_broadcast([128, HEADS, OUT]),
                        op=OP.mult)
                    ored = w2sb.tile([128, OUT], F32, tag="ored")
                    nc.vector.tensor_reduce(
                        out=ored[:, :],
                        in_=og[:, :].rearrange("p (h c) -> p c h", h=HEADS),
                        axis=mybir.AxisListType.X, op=OP.add)
                    nc.vector.tensor_tensor(out=ored[:, :], in0=ored[:, :],
                                            in1=skip2sb[:, w, :], op=OP.add)
                    nc.vector.tensor_tensor(out=outsb[:, w, :], in0=ored[:, :],
                                            in1=bias2[:, :], op=OP.add)
                    b0 += nb

            # ---- final output DMA ----
            wf = NPC // 128 if 'E' in stages else 0
            rem = NPC % 128 if 'E' in stages else 0
            if wf:
                nc.sync.dma_start(
                    out=out_d[0:wf * 128, :].rearrange("(w p) c -> p w c", p=128),
                    in_=outsb[:, 0:wf, :])
            if rem:
                nc.sync.dma_start(out=out_d[wf * 128:NPC, :], in_=outsb[0:rem, wf, :])

    fix_library_reloads(nc)
    split_multi_waits(nc)
    return nc


def make_in_maps(pp, inputs, N, F_IN=128, HID=64, HEADS=4, OUT=2):
    NPC = pp["npc"]
    NB = pp["NB"]
    x = np.ascontiguousarray(np.asarray(inputs["x"], dtype=np.float32))
    xT = np.ascontiguousarray(x.T)
    f32 = lambda a, shp: np.ascontiguousarray(np.asarray(a, dtype=np.float32).reshape(shp))

    W1s = f32(inputs["W1s"], (F_IN, HEADS * HID))
    W1d = f32(inputs["W1d"], (F_IN, HEADS * HID))
    a1s = f32(inputs["a1s"], (HEADS, HID))
    a1d = f32(inputs["a1d"], (HEADS, HID))
    W2s = f32(inputs["W2s"], (HID, HEADS * OUT))
    W2d = f32(inputs["W2d"], (HID, HEADS * OUT))
    a2s = f32(inputs["a2s"], (HEADS, OUT))
    a2d = f32(inputs["a2d"], (HEADS, OUT))
    # host weight folding: a_src/a_dst projections as extra W columns
    fold1s = np.einsum('fhc,hc->fh', W1s.reshape(F_IN, HEADS, HID), a1s)
    fold1d = np.einsum('fhc,hc->fh', W1d.reshape(F_IN, HEADS, HID), a1d)
    fold2s = np.einsum('fhc,hc->fh', W2s.reshape(HID, HEADS, OUT), a2s)
    fold2d = np.einsum('fhc,hc->fh', W2d.reshape(HID, HEADS, OUT), a2d)
    W1aug = np.ascontiguousarray(np.concatenate([W1s, fold1s], axis=1))
    Wl1aug = np.ascontiguousarray(
        np.concatenate([f32(inputs["Wl1"], (F_IN, HID)), fold1d], axis=1))
    W2aug = np.ascontiguousarray(np.concatenate(
        [W2s, fold2s, fold2d, f32(inputs["Wl2"], (HID, OUT))], axis=1))
    bias1 = np.tile((f32(inputs["b1"], (1, HID)) + f32(inputs["bl1"], (1, HID))), (128, 1))
    bias2 = np.tile((f32(inputs["b2"], (1, OUT)) + f32(inputs["bl2"], (1, OUT))), (128, 1))

    common = {
        "xT": xT,
        "W1aug": W1aug,
        "Wl1aug": Wl1aug,
        "W2aug": W2aug,
        "bias1": np.ascontiguousarray(bias1),
        "bias2": np.ascontiguousarray(bias2),
    }
    import ml_dtypes
    in_maps = []
    for c in range(8):
        m = dict(common)
        m["xTown"] = np.ascontiguousarray(xT[:, c * NPC:(c + 1) * NPC])
        m["gidx"] = pp["gidx"][c]
        m["dstloc"] = pp["dstloc"][c]
        # transposed one-hot: St[d, b*128+p] = (dstloc[p, b] == d), bf16
        dl = pp["dstloc"][c]                       # [128, NB]
        pp_, bb_ = np.nonzero(dl >= 0)
        dv = dl[pp_, bb_].astype(np.int64)
        St = np.zeros((128, NB * 128), np.uint16)
        St[dv, bb_ * 128 + pp_] = 0x3F80           # 1.0 bf16
        m["Stdat"] = St.view(ml_dtypes.bfloat16)
        in_maps.append(m)
    return in_maps


_BUILD_CACHE = {}
LAST_RESULTS = None


def kernel(**inputs):
    """Full inputs in, full [N, 2] float32 output out."""
    global LAST_RESULTS
    trace = bool(inputs.pop("_trace", False))
    pp = preprocess(inputs["edge_index"], N_NODES)
    key = (pp["NB"], tuple(pp["nbw_low"]), tuple(pp["nbw_high"]))
    if key not in _BUILD_CACHE:
        _BUILD_CACHE[key] = build(pp, N_NODES)
    nc = _BUILD_CACHE[key]
    in_maps = make_in_maps(pp, inputs, N_NODES)
    res = run_bass_kernel_spmd(nc, in_maps, list(range(8)), trace=trace)
    LAST_RESULTS = res
    out = np.concatenate([res.results[c]["out"] for c in range(8)], axis=0)
    return out.astype(np.float32)


# revision 42
# speedup vs baseline: 3.7354x; 1.1224x over previous
"""Self-contained Trainium2 Bass kernel for the 2-layer GAT problem
(nn_GAT_26714696581831). 8-core SPMD: edges sorted by dst, 8 dst-range
shards; per-window one-hot matmul aggregation with dma_gather row fetches
spread across 4 SWDGE queues (4 Q7 core pairs generate descriptors in
parallel).

kernel(**inputs) takes the FULL unsharded inputs and returns the FULL
[50000, 2] output.
"""
import sys
sys.path.insert(0, '/opt/trn_rl_repo')
import numpy as np
import concourse.bass as bass
import concourse.mybir as mybir
import concourse.tile as tile
from concourse import library_config
from concourse.masks import make_identity
from concourse.bass_utils import run_bass_kernel_spmd

N_NODES = 50000
"""Workarounds for this walrus build, which rejects any instruction carrying
more than one sync-wait command: hoist extra waits onto same-engine NoOps
inserted immediately before the instruction."""


_ctr = [0]

def split_multi_waits(nc, max_waits=1):
    for fn in nc.m.functions:
        for bb in fn.blocks:
            insts = bb.instructions
            i = 0
            while i < len(insts):
                ins = insts[i]
                si = ins.sync_info
                if si is not None and si.on_wait and len(si.on_wait) > max_waits:
                    waits = list(si.on_wait)
                    keep = waits[-max_waits:]
                    hoist = waits[:-max_waits]
                    si.on_wait = keep
                    for w in hoist:
                        _ctr[0] += 1
                        n = mybir.InstNoOp(name=f"waitsplit-{_ctr[0]}", ins=[], outs=[])
                        n.engine = ins.engine
                        n.sync_info = mybir.SyncInfo(on_wait=[w], on_update=[])
                        insts.insert(i, n)
                        i += 1
                i += 1


def fix_library_reloads(nc):
    """bass_rust leaves InstPseudoReloadLibraryIndex.instr empty; this walrus
    rejects zero-length ISA instructions. Encode the 64-byte
    PSEUDO_LIBRARY_RELOAD_INDEX struct with the live ISA tables."""
    isa = nc.isa
    sn = 'NEURON_ISA_TPB_PSEUDO_LIBRARY_RELOAD_INDEX_STRUCT'
    e = isa.get_enum("NEURON_ISA_TPB_PSEUDO_OPCODE")
    val = e.NEURON_ISA_TPB_PSEUDO_OPCODE_PSEUDO_LIBRARY_RELOAD_INDEX.value
    for fn in nc.m.functions:
        for bb in fn.blocks:
            for ins in bb.instructions:
                if type(ins).__name__ == 'InstPseudoReloadLibraryIndex' and not ins.instr:
                    b = isa.asm({"header": {"opcode": 223, "inst_word_len": 16},
                                 "pseudo_opcode": val,
                                 "lib_index": ins.lib_index}, sn)
                    ins.instr = [int(x) for x in b]




WIN = 128                  # dst nodes per window
SPLIT = 32768              # int16 positive limit for gather indices


def preprocess(edge_index, n_nodes, ncores=8):
    src = np.asarray(edge_index[0], dtype=np.int64)
    dst = np.asarray(edge_index[1], dtype=np.int64)
    npc = n_nodes // ncores
    nwin = (npc + WIN - 1) // WIN

    order = np.argsort(dst, kind="stable")
    src_s = src[order]
    dst_s = dst[order]

    counts = np.bincount(dst_s // npc, minlength=ncores)
    core_slices = np.concatenate([[0], np.cumsum(counts)])

    nlow = np.zeros((ncores, nwin), dtype=np.int64)
    nhigh = np.zeros((ncores, nwin), dtype=np.int64)
    per_core_win_edges = []
    for c in range(ncores):
        s0, s1 = core_slices[c], core_slices[c + 1]
        csrc = src_s[s0:s1]
        cdst = dst_s[s0:s1]
        wloc = (cdst - c * npc) // WIN
        dloc = (cdst - c * npc) % WIN
        wins = []
        for w in range(nwin):
            m = wloc == w
            ws, wd = csrc[m], dloc[m]
            lo = ws < SPLIT
            wins.append((ws[lo], ws[~lo] - SPLIT, wd[lo], wd[~lo]))
            nlow[c, w] = lo.sum()
            nhigh[c, w] = (~lo).sum()
        per_core_win_edges.append(wins)

    nbw_low = ((nlow.max(axis=0) + 127) // 128).astype(int)
    nbw_high = ((nhigh.max(axis=0) + 127) // 128).astype(int)
    for w in range(nwin):
        if nbw_low[w] + nbw_high[w] == 0:
            nbw_low[w] = 1
    NB = int(nbw_low.sum() + nbw_high.sum())

    gidx_lin = np.zeros((ncores, NB * 128), dtype=np.int16)
    srcidx_lin = np.zeros((ncores, NB * 128), dtype=np.int32)
    dstidx_lin = np.zeros((ncores, NB * 128), dtype=np.int32)
    dstloc_lin = np.full((ncores, NB * 128), -1, dtype=np.int16)

    for c in range(ncores):
        b0 = 0
        for w in range(nwin):
            slo, shi, dlo, dhi = per_core_win_edges[c][w]
            o = b0 * 128
            gidx_lin[c, o:o + len(slo)] = slo.astype(np.int16)
            srcidx_lin[c, o:o + len(slo)] = slo
            dstidx_lin[c, o:o + len(dlo)] = dlo + w * WIN + c * npc
            dstloc_lin[c, o:o + len(dlo)] = dlo.astype(np.int16)
            b0 += int(nbw_low[w])
            o = b0 * 128
            gidx_lin[c, o:o + len(shi)] = shi.astype(np.int16)
            srcidx_lin[c, o:o + len(shi)] = shi + SPLIT
            dstidx_lin[c, o:o + len(dhi)] = dhi + w * WIN + c * npc
            dstloc_lin[c, o:o + len(dhi)] = dhi.astype(np.int16)
            b0 += int(nbw_high[w])
        assert b0 == NB

    def wrap16(lin):  # [NC, NB*128] -> [NC, 128, NB*8] dma_gather layout
        x = lin.reshape(ncores, NB * 8, 16).transpose(0, 2, 1)
        return np.ascontiguousarray(np.tile(x, (1, 8, 1)))

    # dst-local indices for the a_dst gather (per-core local table, < 32768)
    adidx_lin = np.empty((ncores, NB * 128), dtype=np.int16)
    for c in range(ncores):
        loc = dstidx_lin[c].astype(np.int64) - c * npc
        loc[dstloc_lin[c] < 0] = 0          # pad slots -> row 0
        adidx_lin[c] = loc.astype(np.int16)

    def slotlay(lin, dtype):  # [NC, NB*128] -> [NC, 128, NB] ([p,b] = slot b*128+p)
        return np.ascontiguousarray(lin.reshape(ncores, NB, 128).transpose(0, 2, 1)).astype(dtype)

    return dict(
        NB=NB, nwin=nwin, npc=npc, ncores=ncores,
        nbw_low=nbw_low, nbw_high=nbw_high,
        gidx=wrap16(gidx_lin),
        adidx=wrap16(adidx_lin),
        srcidx=slotlay(srcidx_lin, np.int32),
        dstidx=slotlay(dstidx_lin, np.int32),
        dstloc=slotlay(dstloc_lin, np.int16),
    )




F32 = mybir.dt.float32
BF16 = mybir.dt.bfloat16
F8 = mybir.dt.float8e4
I16 = mybir.dt.int16
U16 = mybir.dt.uint16
AF = mybir.ActivationFunctionType
OP = mybir.AluOpType

SPLIT = 32768
GCHUNK = 8   # blocks per dma_gather call (1024 idx: single-packet-safe)
NQ = 4       # SWDGE queues: queue q's descriptors are generated by Q7 core
             # pair (2q, 2q+1); round-robin spreads desc-gen over all 8 cores


def chunked_gather(nc, out_tile, in_ap, idx_sb, b0, nblk, elem, regs, qrr, boff=0):
    """Issue dma_gather in <=GCHUNK-block chunks writing out_tile[:, boff+i...].

    Chunk sizes are balanced (11 -> 6+5, not 8+3) and queues strictly
    rotate so consecutive calls always hit different Q7 desc-gen core
    pairs - in-order instruction retirement then pipelines ~4 deep.
    """
    nchunks = (nblk + GCHUNK - 1) // GCHUNK
    base, rem = divmod(nblk, nchunks)
    done = 0
    for i in range(nchunks):
        step = base + (1 if i < rem else 0)
        n = step * 128
        if n not in regs:
            regs[n] = nc.gpsimd.to_reg(n)
        nc.gpsimd.dma_gather(
            out_tile[:, boff + done:boff + done + step, :], in_ap,
            idx_sb[:, (b0 + done) * 8:(b0 + done + step) * 8],
            n, regs[n], elem, queue_num=qrr[0] % NQ)
        qrr[0] += 1
        done += step


def build(pp, N, F_IN=128, HID=64, HEADS=4, OUT=2, neg_slope=0.2, stages='ABCDE'):
    NB = pp["NB"]
    NWIN = pp["nwin"]
    NPC = pp["npc"]
    HC1 = HEADS * HID          # 256
    HC2 = HEADS * OUT          # 8
    NBWmax = int(max(pp["nbw_low"][w] + pp["nbw_high"][w] for w in range(NWIN)))
    NCHUNK = (N + 127) // 128
    T1C = 256                  # u16 cols = 512B rows: 256 fp8 xs + 4 bf16 a_src
    L1COL = HC1 + HEADS        # 260: xs + a_src fold
    K1COL = HID + HEADS        # 68  (skip + W_ad fold)
    W2COL = HC2 + 2 * HEADS + OUT  # 18
    R2COL = HC2 + HEADS        # 12
    T2C = 128                  # u16 cols = 256B rows
    T2W = 16                   # used u16 cols of a table2 row (z2 bf16 + a2s f32)
    BBATCH = 16                # stage-B chunks per DMA batch
    WSPLIT = NWIN // 2         # C-windows per t2 chunk (chunked allgather)
    ROWA = WSPLIT * 128        # own rows in chunk A
    ROWB = NPC - ROWA          # own rows in chunk B

    nc = bass.Bass("TRN2", target_bir_lowering=False, debug=False,
                   num_devices=8, num_swdge_queues=NQ)

    # ---- I/O ----
    xT = nc.dram_tensor("xT", [F_IN, N], BF16, kind="ExternalInput")
    xTown = nc.dram_tensor("xTown", [F_IN, NPC], BF16, kind="ExternalInput")
    W1aug_d = nc.dram_tensor("W1aug", [F_IN, L1COL], BF16, kind="ExternalInput")
    Wl1aug_d = nc.dram_tensor("Wl1aug", [F_IN, K1COL], BF16, kind="ExternalInput")
    W2aug_d = nc.dram_tensor("W2aug", [HID, W2COL], F32, kind="ExternalInput")
    bias1_d = nc.dram_tensor("bias1", [128, HID], F32, kind="ExternalInput")
    bias2_d = nc.dram_tensor("bias2", [128, OUT], F32, kind="ExternalInput")
    gidx_d = nc.dram_tensor("gidx", [128, NB * 8], I16, kind="ExternalInput")
    dstloc_d = nc.dram_tensor("dstloc", [128, NB], I16, kind="ExternalInput")
    # transposed one-hot dst matrices St[d, slot] (host-built, streamed):
    # fetch per-edge a_dst via a tiny matmul instead of a dma_gather
    Stdat_d = nc.dram_tensor("Stdat", [128, NB * 128], BF16, kind="ExternalInput")
    out_d = nc.dram_tensor("out", [NPC, OUT], F32, kind="ExternalOutput")

    # internal DRAM. table1 split lo/hi so low-src gathers can start while
    # stage B is still projecting the high node range.
    table1lo = nc.dram_tensor("table1lo", [SPLIT, T1C], U16)
    table1hi = nc.dram_tensor("table1hi", [N - SPLIT, T1C], U16)
    # layer-2 tables: compact [*, 16] payload allgathered in two window
    # chunks (A fires mid-stage-C), then expanded into 256B gather rows.
    t2cA = nc.dram_tensor("t2cA", [ROWA, T2W], U16)
    t2cB = nc.dram_tensor("t2cB", [ROWB, T2W], U16)
    tab2cA = nc.dram_tensor("tab2cA", [8 * ROWA, T2W], U16, addr_space="Shared")
    tab2cB = nc.dram_tensor("tab2cB", [8 * ROWB, T2W], U16, addr_space="Shared")
    table2 = nc.dram_tensor("table2", [N, T2C], U16)

    with tile.TileContext(nc) as tc:
        with tc.tile_pool(name="const", bufs=1) as cpool, \
             tc.tile_pool(name="resident", bufs=1) as rpool:

            # ---- constants (all weight folding/permutation done on host) ----
            W1aug = cpool.tile([F_IN, L1COL], BF16)
            nc.sync.dma_start(out=W1aug[:, :], in_=W1aug_d[:, :])
            Wl1aug = cpool.tile([F_IN, K1COL], BF16)
            nc.sync.dma_start(out=Wl1aug[:, :], in_=Wl1aug_d[:, :])
            W2aug = cpool.tile([HID, W2COL], F32)
            nc.sync.dma_start(out=W2aug[:, :], in_=W2aug_d[:, :])
            bias1 = cpool.tile([128, HID], F32)
            nc.sync.dma_start(out=bias1[:, :], in_=bias1_d[:, :])
            bias2 = cpool.tile([128, OUT], F32)
            nc.sync.dma_start(out=bias2[:, :], in_=bias2_d[:, :])

            iota_t = cpool.tile([128, 128], I16)
            nc.gpsimd.iota(iota_t[:, :], pattern=[[1, 128]], base=0, channel_multiplier=0)
            ident = cpool.tile([128, 128], F32)
            make_identity(nc, ident[:, :])

            gidx_sb = rpool.tile([128, NB * 8], I16)
            nc.sync.dma_start(out=gidx_sb[:, :], in_=gidx_d[:, :])
            dstloc_sb = rpool.tile([128, NB], I16)
            nc.sync.dma_start(out=dstloc_sb[:, :], in_=dstloc_d[:, :])

            # all standard-library gpsimd ops (iota/affine_select/memset) are
            # above; from here on the Q7 carveout holds the mlp library.
            nc.gpsimd.load_library(library_config.mlp)
            gregs = {}
            qrr = [0, 0, 0, 0]

            if 'C' in stages:
                skip2sb = rpool.tile([128, NWIN, OUT], F32)
                outsb = rpool.tile([128, NWIN, OUT], F32)
                ad2sb = rpool.tile([128, NWIN, HEADS], BF16)
                # zero-fill: the last ragged window leaves tail partitions
                # unwritten and 0 x NaN would poison the psAD2 contraction
                nc.vector.memset(ad2sb[:, :, :], 0.0)

            # ---- stage B: project all N nodes, table1 row = [xs | a_src] bf16 ----
            assert SPLIT % (128 * BBATCH) == 0
            NBAT = (NCHUNK + BBATCH - 1) // BBATCH if 'B' in stages else 0
            with tc.tile_pool(name="projps", bufs=4, space="PSUM") as ppp, \
                 tc.tile_pool(name="projsb", bufs=3) as psb:
                for i in range(NBAT):
                    o = i * 128 * BBATCH
                    cb = min(128 * BBATCH, N - o)          # rows this batch
                    nch = (cb + 127) // 128
                    tab, to = (table1lo, o) if o < SPLIT else (table1hi, o - SPLIT)
                    xb = psb.tile([F_IN, BBATCH * 128], BF16, tag="xb")
                    nc.sync.dma_start(out=xb[:, 0:cb], in_=xT[:, o:o + cb])
                    stq = psb.tile([128, BBATCH, 132], U16, tag="stq")
                    for j in range(nch):
                        cn = min(128, cb - j * 128)
                        ps = ppp.tile([128, L1COL], F32, space="PSUM")
                        nc.tensor.matmul(out=ps[0:cn, :],
                                         lhsT=xb[:, j * 128:j * 128 + cn],
                                         rhs=W1aug[:, :], start=True, stop=True)
                        if j % 2 == 0:
                            nc.scalar.activation(out=stq[0:cn, j, 0:128].bitcast(F8),
                                                 in_=ps[0:cn, 0:HC1], func=AF.Copy)
                        else:
                            nc.vector.tensor_copy(stq[0:cn, j, 0:128].bitcast(F8),
                                                  ps[0:cn, 0:HC1])
                        nc.vector.tensor_copy(stq[0:cn, j, 128:132].bitcast(BF16),
                                              ps[0:cn, HC1:L1COL])
                    if cb == BBATCH * 128:
                        nc.scalar.dma_start(
                            out=tab[to:to + cb, 0:132]
                                .rearrange("(b p) c -> p b c", p=128),
                            in_=stq[:, :, :])
                    else:
                        for j in range(nch):
                            cn = min(128, cb - j * 128)
                            oj = to + j * 128
                            nc.scalar.dma_start(
                                out=tab[oj:oj + cn, 0:132],
                                in_=stq[0:cn, j, :])

            # ---- stage C: layer-1 edge softmax + aggregation per dst window,
            # with the layer-2 projection (old stage D) fused per window ----
            NWIN_C = NWIN if 'C' in stages else 0
            with tc.tile_pool(name="winps", bufs=2, space="PSUM") as wps, \
                 tc.tile_pool(name="klps", bufs=3, space="PSUM") as kps, \
                 tc.tile_pool(name="adps", bufs=3, space="PSUM") as aps, \
                 tc.tile_pool(name="gpool", bufs=4) as gpl, \
                 tc.tile_pool(name="stpool", bufs=3) as stpl, \
                 tc.tile_pool(name="spool", bufs=4) as spl, \
                 tc.tile_pool(name="winsb", bufs=3) as wsb:
                b0 = 0
                for w in range(NWIN_C):
                    BL = int(pp["nbw_low"][w])
                    BH = int(pp["nbw_high"][w])
                    nb = BL + BH
                    cn_w = min(128, NPC - w * 128)
                    # one PSUM tile per window: [0:68]=psK (skip+a_dst own),
                    # [68:86]=psL (fused layer-2 proj), [96:224]=psT (h transpose)
                    xo = wsb.tile([F_IN, 128], BF16, tag="xo")
                    nc.sync.dma_start(out=xo[:, 0:cn_w], in_=xTown[:, w * 128:w * 128 + cn_w])
                    psKL = kps.tile([128, 224], F32, space="PSUM")
                    psK = psKL[:, 0:K1COL]
                    psL = psKL[:, K1COL:K1COL + W2COL]
                    psT = psKL[0:HID, 96:224]
                    nc.tensor.matmul(out=psK[0:cn_w, :], lhsT=xo[:, 0:cn_w], rhs=Wl1aug[:, :],
                                     start=True, stop=True)
                    stK = wsb.tile([128, HEADS], BF16, tag="stK")
                    nc.vector.tensor_copy(stK[0:cn_w, :], psK[0:cn_w, HID:K1COL])
                    # one-hot dst matrices: S built on DVE, St streamed from host
                    S = spl.tile([128, NBWmax, 128], BF16, tag="S")
                    nc.vector.tensor_tensor(
                        out=S[:, 0:nb, :],
                        in0=dstloc_sb[:, b0:b0 + nb].unsqueeze(2).to_broadcast([128, nb, 128]),
                        in1=iota_t[:, :].unsqueeze(1).to_broadcast([128, nb, 128]),
                        op=OP.is_equal)
                    Stw = stpl.tile([128, NBWmax * 128], BF16, tag="Stw")
                    nc.scalar.dma_start(out=Stw[:, 0:nb * 128],
                                        in_=Stdat_d[:, b0 * 128:(b0 + nb) * 128])
                    # per-edge a_dst via St.T @ a_dst-window-table
                    psAD = aps.tile([128, NBWmax * HEADS], F32, space="PSUM")
                    for j in range(nb):
                        nc.tensor.matmul(out=psAD[:, j * HEADS:(j + 1) * HEADS],
                                         lhsT=Stw[:, j * 128:(j + 1) * 128],
                                         rhs=stK[:, :], start=True, stop=True)
                    # gathers
                    G = gpl.tile([128, NBWmax, T1C], U16, tag="G")
                    if BL:
                        chunked_gather(nc, G, table1lo[:, :],
                                       gidx_sb, b0, BL, T1C, gregs, qrr)
                    if BH:
                        chunked_gather(nc, G, table1hi[:, :],
                                       gidx_sb, b0 + BL, BH, T1C, gregs, qrr, boff=BL)
                    # e = a_src + a_dst; ex = exp(lrelu(e)) = max(exp(e), exp(0.2e))
                    ex = wsb.tile([128, NBWmax, HEADS], F32, tag="ex")
                    nc.vector.tensor_tensor(
                        out=ex[:, 0:nb, :],
                        in0=G[:, 0:nb, 128:132].bitcast(BF16),
                        in1=psAD[:, 0:nb * HEADS].rearrange("p (b h) -> p b h", h=HEADS),
                        op=OP.add)
                    exa = wsb.tile([128, NBWmax, HEADS], BF16, tag="exa")
                    nc.scalar.activation(out=exa[:, 0:nb, :], in_=ex[:, 0:nb, :],
                                         func=AF.Exp)
                    exb = wsb.tile([128, NBWmax, HEADS], BF16, tag="exb")
                    nc.scalar.activation(out=exb[:, 0:nb, :], in_=ex[:, 0:nb, :],
                                         func=AF.Exp, scale=float(neg_slope))
                    nc.vector.tensor_tensor(out=exb[:, 0:nb, :], in0=exa[:, 0:nb, :],
                                            in1=exb[:, 0:nb, :], op=OP.max)
                    Gp = gpl.tile([128, NBWmax, HC1 + HEADS], BF16, tag="Gp")
                    nc.vector.tensor_tensor(
                        out=Gp[:, 0:nb, 0:HC1].rearrange("p b (h c) -> p b h c", h=HEADS),
                        in0=G[:, 0:nb, 0:128].bitcast(F8).rearrange("p b (h c) -> p b h c", h=HEADS),
                        in1=exb[:, 0:nb, :].unsqueeze(3).to_broadcast([128, nb, HEADS, HID]),
                        op=OP.mult)
                    nc.vector.tensor_copy(Gp[:, 0:nb, HC1:HC1 + HEADS], exb[:, 0:nb, :])
                    psW = wps.tile([128, HC1 + HEADS], F32, space="PSUM")
                    for j in range(nb):
                        nc.tensor.matmul(out=psW[:, :], lhsT=S[:, j, :], rhs=Gp[:, j, :],
                                         start=(j == 0), stop=(j == nb - 1))
                    # extract: h = sigmoid(gat/4sum + skip + bias)
                    rec = wsb.tile([128, HEADS], F32, tag="rec")
                    nc.vector.tensor_scalar(out=rec[:, :], in0=psW[:, HC1:HC1 + HEADS],
                                            scalar1=1e-16, scalar2=float(HEADS),
                                            op0=OP.add, op1=OP.mult)
                    nc.vector.reciprocal(rec[:, :], rec[:, :])
                    gat = wsb.tile([128, HC1], F32, tag="gat")
                    nc.vector.tensor_tensor(
                        out=gat[:, :].rearrange("p (h c) -> p h c", h=HEADS),
                        in0=psW[:, 0:HC1].rearrange("p (h c) -> p h c", h=HEADS),
                        in1=rec[:, :].unsqueeze(2).to_broadcast([128, HEADS, HID]),
                        op=OP.mult)
                    hred = wsb.tile([128, HID], F32, tag="hred")
                    nc.vector.tensor_reduce(
                        out=hred[:, :],
                        in_=gat[:, :].rearrange("p (h c) -> p c h", h=HEADS),
                        axis=mybir.AxisListType.X, op=OP.add)
                    nc.vector.tensor_tensor(out=hred[:, :], in0=hred[:, :],
                                            in1=psK[:, 0:HID], op=OP.add)
                    nc.vector.tensor_tensor(out=hred[:, :], in0=hred[:, :],
                                            in1=bias1[:, :], op=OP.add)
                    hwin = wsb.tile([128, HID], F32, tag="hwin")
                    nc.scalar.activation(out=hwin[:, :], in_=hred[:, :], func=AF.Sigmoid)
                    nc.tensor.transpose(out=psT[:, :], in_=hwin[:, :], identity=ident[:, :])
                    htw = wsb.tile([HID, 128], F32, tag="htw")
                    nc.vector.tensor_copy(htw[:, :], psT[:, :])
                    # fused layer-2 projection of this window's own nodes
                    nc.tensor.matmul(out=psL[0:cn_w, :], lhsT=htw[:, 0:cn_w],
                                     rhs=W2aug[:, :], start=True, stop=True)
                    t2st = wsb.tile([128, T2W], U16, tag="t2st")
                    nc.vector.tensor_copy(t2st[0:cn_w, 0:HC2].bitcast(BF16),
                                          psL[0:cn_w, 0:HC2])
                    nc.vector.tensor_copy(t2st[0:cn_w, HC2:T2W].bitcast(F32),
                                          psL[0:cn_w, HC2:HC2 + HEADS])
                    if w < WSPLIT:
                        nc.sync.dma_start(out=t2cA[w * 128:w * 128 + cn_w, :],
                                          in_=t2st[0:cn_w, :])
                    else:
                        ob = (w - WSPLIT) * 128
                        nc.sync.dma_start(out=t2cB[ob:ob + cn_w, :],
                                          in_=t2st[0:cn_w, :])
                    nc.vector.tensor_copy(ad2sb[0:cn_w, w, :],
                                          psL[0:cn_w, HC2 + HEADS:HC2 + 2 * HEADS])
                    nc.vector.tensor_copy(skip2sb[0:cn_w, w, :],
                                          psL[0:cn_w, HC2 + 2 * HEADS:W2COL])
                    b0 += nb
                    if w == WSPLIT - 1 and 'D' in stages:
                        # chunk A complete on our core: allgather + expand it
                        # while the rest of stage C runs
                        nc.gpsimd.collective_compute(
                            "AllGather", OP.bypass, replica_groups=[list(range(8))],
                            ins=[t2cA[:, :]], outs=[tab2cA[:, :]])
                        nc.sync.dma_start(
                            out=table2[:, 0:T2W]
                                .rearrange("(c r) x -> c r x", c=8)[:, 0:ROWA, :],
                            in_=tab2cA[:, :].rearrange("(c r) x -> c r x", c=8))

            if 'D' in stages:
                nc.gpsimd.collective_compute(
                    "AllGather", OP.bypass, replica_groups=[list(range(8))],
                    ins=[t2cB[:, :]], outs=[tab2cB[:, :]])
                nc.sync.dma_start(
                    out=table2[:, 0:T2W]
                        .rearrange("(c r) x -> c r x", c=8)[:, ROWA:NPC, :],
                    in_=tab2cB[:, :].rearrange("(c r) x -> c r x", c=8))

            # ---- stage E: layer-2 edge softmax + aggregation ----
            NWIN_E = NWIN if 'E' in stages else 0
            with tc.tile_pool(name="w2ps", bufs=3, space="PSUM") as wps2, \
                 tc.tile_pool(name="ad2ps", bufs=3, space="PSUM") as aps2, \
                 tc.tile_pool(name="s2pool", bufs=4) as spl2, \
                 tc.tile_pool(name="st2pool", bufs=3) as stpl2, \
                 tc.tile_pool(name="w2sb", bufs=4) as w2sb:
                b0 = 0
                for w in range(NWIN_E):
                    BL = int(pp["nbw_low"][w])
                    BH = int(pp["nbw_high"][w])
                    nb = BL + BH
                    St2 = stpl2.tile([128, NBWmax * 128], BF16, tag="St2")
                    nc.scalar.dma_start(out=St2[:, 0:nb * 128],
                                        in_=Stdat_d[:, b0 * 128:(b0 + nb) * 128])
                    psAD2 = aps2.tile([128, NBWmax * HEADS], F32, space="PSUM")
                    for j in range(nb):
                        nc.tensor.matmul(out=psAD2[:, j * HEADS:(j + 1) * HEADS],
                                         lhsT=St2[:, j * 128:(j + 1) * 128],
                                         rhs=ad2sb[:, w, :], start=True, stop=True)
                    g2s = w2sb.tile([128, NBWmax, T2C], U16, tag="g2s")
                    if BL:
                        chunked_gather(nc, g2s, table2[0:min(SPLIT, N), :],
                                       gidx_sb, b0, BL, T2C, gregs, qrr)
                    if BH:
                        chunked_gather(nc, g2s, table2[SPLIT:N, :],
                                       gidx_sb, b0 + BL, BH, T2C, gregs, qrr, boff=BL)
                    ex2 = w2sb.tile([128, NBWmax, HEADS], F32, tag="ex2")
                    nc.vector.tensor_tensor(
                        out=ex2[:, 0:nb, :],
                        in0=g2s[:, 0:nb, HC2:16].bitcast(F32),
                        in1=psAD2[:, 0:nb * HEADS].rearrange("p (b h) -> p b h", h=HEADS),
                        op=OP.add)
                    ex2a = w2sb.tile([128, NBWmax, HEADS], BF16, tag="ex2a")
                    nc.scalar.activation(out=ex2a[:, 0:nb, :], in_=ex2[:, 0:nb, :],
                                         func=AF.Exp)
                    ex2b = w2sb.tile([128, NBWmax, HEADS], BF16, tag="ex2b")
                    nc.scalar.activation(out=ex2b[:, 0:nb, :], in_=ex2[:, 0:nb, :],
                                         func=AF.Exp, scale=float(neg_slope))
                    nc.vector.tensor_tensor(out=ex2b[:, 0:nb, :], in0=ex2a[:, 0:nb, :],
                                            in1=ex2b[:, 0:nb, :], op=OP.max)
                    R2 = w2sb.tile([128, NBWmax, R2COL], BF16, tag="R2")
                    nc.vector.tensor_tensor(
                        out=R2[:, 0:nb, 0:HC2].rearrange("p b (h c) -> p b h c", h=HEADS),
                        in0=g2s[:, 0:nb, 0:HC2].bitcast(BF16).rearrange("p b (h c) -> p b h c", h=HEADS),
                        in1=ex2b[:, 0:nb, :].unsqueeze(3).to_broadcast([128, nb, HEADS, OUT]),
                        op=OP.mult)
                    nc.vector.tensor_copy(R2[:, 0:nb, HC2:R2COL], ex2b[:, 0:nb, :])
                    S2 = spl2.tile([128, NBWmax, 128], BF16, tag="S2")
                    nc.vector.tensor_tensor(
                        out=S2[:, 0:nb, :],
                        in0=dstloc_sb[:, b0:b0 + nb].unsqueeze(2).to_broadcast([128, nb, 128]),
                        in1=iota_t[:, :].unsqueeze(1).to_broadcast([128, nb, 128]),
                        op=OP.is_equal)
                    psW2 = wps2.tile([128, R2COL], F32, space="PSUM")
                    for j in range(nb):
                        nc.tensor.matmul(out=psW2[:, :], lhsT=S2[:, j, :], rhs=R2[:, j, :],
                                         start=(j == 0), stop=(j == nb - 1))
                    rec2 = w2sb.tile([128, HEADS], F32, tag="rec2")
                    nc.vector.tensor_scalar(out=rec2[:, :], in0=psW2[:, HC2:R2COL],
                                            scalar1=1e-16, scalar2=float(HEADS),
                                            op0=OP.add, op1=OP.mult)
                    nc.vector.reciprocal(rec2[:, :], rec2[:, :])
                    og = w2sb.tile([128, HC2], F32, tag="og")
                    nc.vector.tensor_tensor(
                        out=og[:, :].rearrange("p (h c) -> p h c", h=HEADS),
                        in0=psW2[:, 0:HC2].rearrange("p (h c) -> p h c", h=HEADS),
                        in1=rec2[:, :].unsqueeze(2).to_broadcast([128, HEADS, OUT]),
                        op=OP.mult)
                    ored = w2sb.tile([128, OUT], F32, tag="ored")
                    nc.vector.tensor_reduce(
                        out=ored[:, :],
                        in_=og[:, :].rearrange("p (h c) -> p c h", h=HEADS),
                        axis=mybir.AxisListType.X, op=OP.add)
                    nc.vector.tensor_tensor(out=ored[:, :], in0=ored[:, :],
                                            in1=skip2sb[:, w, :], op=OP.add)
                    nc.vector.tensor_tensor(out=outsb[:, w, :], in0=ored[:, :],
                                            in1=bias2[:, :], op=OP.add)
                    b0 += nb

            # ---- final output DMA ----
            wf = NPC // 128 if 'E' in stages else 0
            rem = NPC % 128 if 'E' in stages else 0
            if wf:
                nc.sync.dma_start(
                    out=out_d[0:wf * 128, :].rearrange("(w p) c -> p w c", p=128),
                    in_=outsb[:, 0:wf, :])
            if rem:
                nc.sync.dma_start(out=out_d[wf * 128:NPC, :], in_=outsb[0:rem, wf, :])

    fix_library_reloads(nc)
    split_multi_waits(nc)
    return nc


def make_in_maps(pp, inputs, N, F_IN=128, HID=64, HEADS=4, OUT=2):
    NPC = pp["npc"]
    NB = pp["NB"]
    x = np.ascontiguousarray(np.asarray(inputs["x"], dtype=np.float32))
    xT = np.ascontiguousarray(x.T)
    f32 = lambda a, shp: np.ascontiguousarray(np.asarray(a, dtype=np.float32).reshape(shp))

    W1s = f32(inputs["W1s"], (F_IN, HEADS * HID))
    W1d = f32(inputs["W1d"], (F_IN, HEADS * HID))
    a1s = f32(inputs["a1s"], (HEADS, HID))
    a1d = f32(inputs["a1d"], (HEADS, HID))
    W2s = f32(inputs["W2s"], (HID, HEADS * OUT))
    W2d = f32(inputs["W2d"], (HID, HEADS * OUT))
    a2s = f32(inputs["a2s"], (HEADS, OUT))
    a2d = f32(inputs["a2d"], (HEADS, OUT))
    # host weight folding: a_src/a_dst projections as extra W columns
    fold1s = np.einsum('fhc,hc->fh', W1s.reshape(F_IN, HEADS, HID), a1s)
    fold1d = np.einsum('fhc,hc->fh', W1d.reshape(F_IN, HEADS, HID), a1d)
    fold2s = np.einsum('fhc,hc->fh', W2s.reshape(HID, HEADS, OUT), a2s)
    fold2d = np.einsum('fhc,hc->fh', W2d.reshape(HID, HEADS, OUT), a2d)
    W1aug = np.ascontiguousarray(np.concatenate([W1s, fold1s], axis=1))
    Wl1aug = np.ascontiguousarray(
        np.concatenate([f32(inputs["Wl1"], (F_IN, HID)), fold1d], axis=1))
    W2aug = np.ascontiguousarray(np.concatenate(
        [W2s, fold2s, fold2d, f32(inputs["Wl2"], (HID, OUT))], axis=1))
    bias1 = np.tile((f32(inputs["b1"], (1, HID)) + f32(inputs["bl1"], (1, HID))), (128, 1))
    bias2 = np.tile((f32(inputs["b2"], (1, OUT)) + f32(inputs["bl2"], (1, OUT))), (128, 1))

    import ml_dtypes
    bf = ml_dtypes.bfloat16
    common = {
        "xT": np.ascontiguousarray(xT.astype(bf)),
        "W1aug": np.ascontiguousarray(W1aug.astype(bf)),
        "Wl1aug": np.ascontiguousarray(Wl1aug.astype(bf)),
        "W2aug": W2aug,
        "bias1": np.ascontiguousarray(bias1),
        "bias2": np.ascontiguousarray(bias2),
    }
    in_maps = []
    for c in range(8):
        m = dict(common)
        m["xTown"] = np.ascontiguousarray(xT[:, c * NPC:(c + 1) * NPC].astype(bf))
        m["gidx"] = pp["gidx"][c]
        m["dstloc"] = pp["dstloc"][c]
        # transposed one-hot: St[d, b*128+p] = (dstloc[p, b] == d), bf16
        dl = pp["dstloc"][c]                       # [128, NB]
        pp_, bb_ = np.nonzero(dl >= 0)
        dv = dl[pp_, bb_].astype(np.int64)
        St = np.zeros((128, NB * 128), np.uint16)
        St[dv, bb_ * 128 + pp_] = 0x3F80           # 1.0 bf16
        m["Stdat"] = St.view(ml_dtypes.bfloat16)
        in_maps.append(m)
    return in_maps


_BUILD_CACHE = {}
LAST_RESULTS = None


def kernel(**inputs):
    """Full inputs in, full [N, 2] float32 output out."""
    global LAST_RESULTS
    trace = bool(inputs.pop("_trace", False))
    pp = preprocess(inputs["edge_index"], N_NODES)
    key = (pp["NB"], tuple(pp["nbw_low"]), tuple(pp["nbw_high"]))
    if key not in _BUILD_CACHE:
        _BUILD_CACHE[key] = build(pp, N_NODES)
    nc = _BUILD_CACHE[key]
    in_maps = make_in_maps(pp, inputs, N_NODES)
    res = run_bass_kernel_spmd(nc, in_maps, list(range(8)), trace=trace)
    LAST_RESULTS = res
    out = np.concatenate([res.results[c]["out"] for c in range(8)], axis=0)
    return out.astype(np.float32)


# revision 45
# speedup vs baseline: 4.0308x; 1.0791x over previous
"""Self-contained Trainium2 Bass kernel for the 2-layer GAT problem
(nn_GAT_26714696581831). 8-core SPMD: edges sorted by dst, 8 dst-range
shards; per-window one-hot matmul aggregation with dma_gather row fetches
spread across 4 SWDGE queues (4 Q7 core pairs generate descriptors in
parallel).

kernel(**inputs) takes the FULL unsharded inputs and returns the FULL
[50000, 2] output.
"""
import sys
sys.path.insert(0, '/opt/trn_rl_repo')
import numpy as np
import concourse.bass as bass
import concourse.mybir as mybir
import concourse.tile as tile
from concourse import library_config
from concourse.masks import make_identity
from concourse.bass_utils import run_bass_kernel_spmd

N_NODES = 50000
"""Workarounds for this walrus build, which rejects any instruction carrying
more than one sync-wait command: hoist extra waits onto same-engine NoOps
inserted immediately before the instruction."""


_ctr = [0]

def split_multi_waits(nc, max_waits=1):
    for fn in nc.m.functions:
        for bb in fn.blocks:
            insts = bb.instructions
            i = 0
            while i < len(insts):
                ins = insts[i]
                si = ins.sync_info
                if si is not None and si.on_wait and len(si.on_wait) > max_waits:
                    waits = list(si.on_wait)
                    keep = waits[-max_waits:]
                    hoist = waits[:-max_waits]
                    si.on_wait = keep
                    for w in hoist:
                        _ctr[0] += 1
                        n = mybir.InstNoOp(name=f"waitsplit-{_ctr[0]}", ins=[], outs=[])
                        n.engine = ins.engine
                        n.sync_info = mybir.SyncInfo(on_wait=[w], on_update=[])
                        insts.insert(i, n)
                        i += 1
                i += 1


def fix_library_reloads(nc):
    """bass_rust leaves InstPseudoReloadLibraryIndex.instr empty; this walrus
    rejects zero-length ISA instructions. Encode the 64-byte
    PSEUDO_LIBRARY_RELOAD_INDEX struct with the live ISA tables."""
    isa = nc.isa
    sn = 'NEURON_ISA_TPB_PSEUDO_LIBRARY_RELOAD_INDEX_STRUCT'
    e = isa.get_enum("NEURON_ISA_TPB_PSEUDO_OPCODE")
    val = e.NEURON_ISA_TPB_PSEUDO_OPCODE_PSEUDO_LIBRARY_RELOAD_INDEX.value
    for fn in nc.m.functions:
        for bb in fn.blocks:
            for ins in bb.instructions:
                if type(ins).__name__ == 'InstPseudoReloadLibraryIndex' and not ins.instr:
                    b = isa.asm({"header": {"opcode": 223, "inst_word_len": 16},
                                 "pseudo_opcode": val,
                                 "lib_index": ins.lib_index}, sn)
                    ins.instr = [int(x) for x in b]




WIN = 128                  # dst nodes per window
RA = 4096                  # chunk-A rows per core (32 windows); 8*RA = 32768
SPLIT = 32768              # total chunk-A rows = int16 positive limit


def preprocess(edge_index, n_nodes, ncores=8):
    src = np.asarray(edge_index[0], dtype=np.int64)
    dst = np.asarray(edge_index[1], dtype=np.int64)
    npc = n_nodes // ncores
    nwin = (npc + WIN - 1) // WIN
    ra = RA
    rb = npc - ra

    order = np.argsort(dst, kind="stable")
    src_s = src[order]
    dst_s = dst[order]

    # src ids remapped to the chunked table layout: chunk A = rows r<ra of
    # every core (table row c*ra+r), chunk B = the rest (row c*rb+(r-ra)).
    # Both index spaces fit int16; C (table1) and E (table2) share them.
    sc, sr = src_s // npc, src_s % npc
    is_a = sr < ra
    sidx = np.where(is_a, sc * ra + sr, sc * rb + (sr - ra))

    counts = np.bincount(dst_s // npc, minlength=ncores)
    core_slices = np.concatenate([[0], np.cumsum(counts)])

    nlow = np.zeros((ncores, nwin), dtype=np.int64)
    nhigh = np.zeros((ncores, nwin), dtype=np.int64)
    per_core_win_edges = []
    for c in range(ncores):
        s0, s1 = core_slices[c], core_slices[c + 1]
        csrc = sidx[s0:s1]
        cisa = is_a[s0:s1]
        cdst = dst_s[s0:s1]
        wloc = (cdst - c * npc) // WIN
        dloc = (cdst - c * npc) % WIN
        wins = []
        for w in range(nwin):
            m = wloc == w
            ws, wd, wa = csrc[m], dloc[m], cisa[m]
            wins.append((ws[wa], ws[~wa], wd[wa], wd[~wa]))
            nlow[c, w] = wa.sum()
            nhigh[c, w] = (~wa).sum()
        per_core_win_edges.append(wins)

    nbw_low = ((nlow.max(axis=0) + 127) // 128).astype(int)
    nbw_high = ((nhigh.max(axis=0) + 127) // 128).astype(int)
    for w in range(nwin):
        if nbw_low[w] + nbw_high[w] == 0:
            nbw_low[w] = 1
    NB = int(nbw_low.sum() + nbw_high.sum())

    gidx_lin = np.zeros((ncores, NB * 128), dtype=np.int16)
    srcidx_lin = np.zeros((ncores, NB * 128), dtype=np.int32)
    dstidx_lin = np.zeros((ncores, NB * 128), dtype=np.int32)
    dstloc_lin = np.full((ncores, NB * 128), -1, dtype=np.int16)

    for c in range(ncores):
        b0 = 0
        for w in range(nwin):
            slo, shi, dlo, dhi = per_core_win_edges[c][w]
            o = b0 * 128
            gidx_lin[c, o:o + len(slo)] = slo.astype(np.int16)
            srcidx_lin[c, o:o + len(slo)] = slo
            dstidx_lin[c, o:o + len(dlo)] = dlo + w * WIN + c * npc
            dstloc_lin[c, o:o + len(dlo)] = dlo.astype(np.int16)
            b0 += int(nbw_low[w])
            o = b0 * 128
            gidx_lin[c, o:o + len(shi)] = shi.astype(np.int16)
            srcidx_lin[c, o:o + len(shi)] = shi
            dstidx_lin[c, o:o + len(dhi)] = dhi + w * WIN + c * npc
            dstloc_lin[c, o:o + len(dhi)] = dhi.astype(np.int16)
            b0 += int(nbw_high[w])
        assert b0 == NB

    def wrap16(lin):  # [NC, NB*128] -> [NC, 128, NB*8] dma_gather layout
        x = lin.reshape(ncores, NB * 8, 16).transpose(0, 2, 1)
        return np.ascontiguousarray(np.tile(x, (1, 8, 1)))

    # dst-local indices for the a_dst gather (per-core local table, < 32768)
    adidx_lin = np.empty((ncores, NB * 128), dtype=np.int16)
    for c in range(ncores):
        loc = dstidx_lin[c].astype(np.int64) - c * npc
        loc[dstloc_lin[c] < 0] = 0          # pad slots -> row 0
        adidx_lin[c] = loc.astype(np.int16)

    def slotlay(lin, dtype):  # [NC, NB*128] -> [NC, 128, NB] ([p,b] = slot b*128+p)
        return np.ascontiguousarray(lin.reshape(ncores, NB, 128).transpose(0, 2, 1)).astype(dtype)

    return dict(
        NB=NB, nwin=nwin, npc=npc, ncores=ncores,
        nbw_low=nbw_low, nbw_high=nbw_high,
        gidx=wrap16(gidx_lin),
        adidx=wrap16(adidx_lin),
        srcidx=slotlay(srcidx_lin, np.int32),
        dstidx=slotlay(dstidx_lin, np.int32),
        dstloc=slotlay(dstloc_lin, np.int16),
    )




F32 = mybir.dt.float32
BF16 = mybir.dt.bfloat16
F8 = mybir.dt.float8e4
I16 = mybir.dt.int16
U16 = mybir.dt.uint16
AF = mybir.ActivationFunctionType
OP = mybir.AluOpType

SPLIT = 32768
GCHUNK = 8   # blocks per dma_gather call (1024 idx: single-packet-safe)
NQ = 4       # SWDGE queues: queue q's descriptors are generated by Q7 core
             # pair (2q, 2q+1); round-robin spreads desc-gen over all 8 cores


def chunked_gather(nc, out_tile, in_ap, idx_sb, b0, nblk, elem, regs, qrr, boff=0):
    """Issue dma_gather in <=GCHUNK-block chunks writing out_tile[:, boff+i...].

    Chunk sizes are balanced (11 -> 6+5, not 8+3) and queues strictly
    rotate so consecutive calls always hit different Q7 desc-gen core
    pairs - in-order instruction retirement then pipelines ~4 deep.
    """
    nchunks = (nblk + GCHUNK - 1) // GCHUNK
    base, rem = divmod(nblk, nchunks)
    done = 0
    for i in range(nchunks):
        step = base + (1 if i < rem else 0)
        n = step * 128
        if n not in regs:
            regs[n] = nc.gpsimd.to_reg(n)
        nc.gpsimd.dma_gather(
            out_tile[:, boff + done:boff + done + step, :], in_ap,
            idx_sb[:, (b0 + done) * 8:(b0 + done + step) * 8],
            n, regs[n], elem, queue_num=qrr[0] % NQ)
        qrr[0] += 1
        done += step


def build(pp, N, F_IN=128, HID=64, HEADS=4, OUT=2, neg_slope=0.2, stages='ABCDE'):
    NB = pp["NB"]
    NWIN = pp["nwin"]
    NPC = pp["npc"]
    HC1 = HEADS * HID          # 256
    HC2 = HEADS * OUT          # 8
    NBWmax = int(max(pp["nbw_low"][w] + pp["nbw_high"][w] for w in range(NWIN)))
    NCHUNK = (N + 127) // 128
    T1C = 256                  # u16 cols = 512B rows: 256 fp8 xs + 4 bf16 a_src
    L1COL = HC1 + HEADS        # 260: xs + a_src fold
    K1COL = HID + HEADS        # 68  (skip + W_ad fold)
    W2COL = HC2 + 2 * HEADS + OUT  # 18
    R2COL = HC2 + HEADS        # 12
    T2C = 128                  # u16 cols = 256B rows
    T2W = 16                   # used u16 cols of a table2 row (z2 bf16 + a2s f32)
    BBATCH = 16                # stage-B chunks per DMA batch
    WSPLIT = RA // 128         # C-windows per t2 chunk A (chunked allgather)
    ROWA = RA                  # own rows in chunk A (4096)
    ROWB = NPC - ROWA          # own rows in chunk B (2154)

    nc = bass.Bass("TRN2", target_bir_lowering=False, debug=False,
                   num_devices=8, num_swdge_queues=NQ)

    # ---- I/O ----
    xT = nc.dram_tensor("xT", [F_IN, N], BF16, kind="ExternalInput")
    xTown = nc.dram_tensor("xTown", [F_IN, NPC], BF16, kind="ExternalInput")
    W1aug_d = nc.dram_tensor("W1aug", [F_IN, L1COL], BF16, kind="ExternalInput")
    Wl1aug_d = nc.dram_tensor("Wl1aug", [F_IN, K1COL], BF16, kind="ExternalInput")
    W2aug_d = nc.dram_tensor("W2aug", [HID, W2COL], F32, kind="ExternalInput")
    bias1_d = nc.dram_tensor("bias1", [128, HID], F32, kind="ExternalInput")
    bias2_d = nc.dram_tensor("bias2", [128, OUT], F32, kind="ExternalInput")
    gidx_d = nc.dram_tensor("gidx", [128, NB * 8], I16, kind="ExternalInput")
    dstloc_d = nc.dram_tensor("dstloc", [128, NB], I16, kind="ExternalInput")
    # transposed one-hot dst matrices St[d, slot] (host-built, streamed):
    # fetch per-edge a_dst via a tiny matmul instead of a dma_gather
    Stdat_d = nc.dram_tensor("Stdat", [128, NB * 128], BF16, kind="ExternalInput")
    out_d = nc.dram_tensor("out", [NPC, OUT], F32, kind="ExternalOutput")

    # internal DRAM. All tables are split at the (core, r<RA) boundary:
    # chunk A = rows {c*NPC+r : r<RA} stored at c*RA+r (8*RA = 32768 rows),
    # chunk B = the rest. The same split chunks the layer-2 allgather, so
    # chunk-A tables complete mid-stage-C and stage E's A-side gathers can
    # start while C still runs.
    table1A = nc.dram_tensor("table1A", [8 * ROWA, T1C], U16)
    table1B = nc.dram_tensor("table1B", [8 * ROWB, T1C], U16)
    t2cA = nc.dram_tensor("t2cA", [ROWA, T2W], U16)
    t2cB = nc.dram_tensor("t2cB", [ROWB, T2W], U16)
    tab2cA = nc.dram_tensor("tab2cA", [8 * ROWA, T2W], U16, addr_space="Shared")
    tab2cB = nc.dram_tensor("tab2cB", [8 * ROWB, T2W], U16, addr_space="Shared")
    table2A = nc.dram_tensor("table2A", [8 * ROWA, T2C], U16)
    table2B = nc.dram_tensor("table2B", [8 * ROWB, T2C], U16)

    with tile.TileContext(nc) as tc:
        with tc.tile_pool(name="const", bufs=1) as cpool, \
             tc.tile_pool(name="resident", bufs=1) as rpool:

            # ---- constants (all weight folding/permutation done on host) ----
            W1aug = cpool.tile([F_IN, L1COL], BF16)
            nc.sync.dma_start(out=W1aug[:, :], in_=W1aug_d[:, :])
            Wl1aug = cpool.tile([F_IN, K1COL], BF16)
            nc.sync.dma_start(out=Wl1aug[:, :], in_=Wl1aug_d[:, :])
            W2aug = cpool.tile([HID, W2COL], F32)
            nc.sync.dma_start(out=W2aug[:, :], in_=W2aug_d[:, :])
            bias1 = cpool.tile([128, HID], F32)
            nc.sync.dma_start(out=bias1[:, :], in_=bias1_d[:, :])
            bias2 = cpool.tile([128, OUT], F32)
            nc.sync.dma_start(out=bias2[:, :], in_=bias2_d[:, :])

            iota_t = cpool.tile([128, 128], I16)
            nc.gpsimd.iota(iota_t[:, :], pattern=[[1, 128]], base=0, channel_multiplier=0)
            ident = cpool.tile([128, 128], F32)
            make_identity(nc, ident[:, :])

            gidx_sb = rpool.tile([128, NB * 8], I16)
            nc.sync.dma_start(out=gidx_sb[:, :], in_=gidx_d[:, :])
            dstloc_sb = rpool.tile([128, NB], I16)
            nc.sync.dma_start(out=dstloc_sb[:, :], in_=dstloc_d[:, :])

            # all standard-library gpsimd ops (iota/affine_select/memset) are
            # above; from here on the Q7 carveout holds the mlp library.
            nc.gpsimd.load_library(library_config.mlp)
            gregs = {}
            qrr = [0, 0, 0, 0]

            if 'C' in stages:
                skip2sb = rpool.tile([128, NWIN, OUT], F32)
                outsb = rpool.tile([128, NWIN, OUT], F32)
                ad2sb = rpool.tile([128, NWIN, HEADS], BF16)
                # zero-fill: the last ragged window leaves tail partitions
                # unwritten and 0 x NaN would poison the psAD2 contraction
                nc.vector.memset(ad2sb[:, :, :], 0.0)

            # ---- stage B: project all N nodes, table1 row = [xs | a_src] bf16 ----
            assert SPLIT % (128 * BBATCH) == 0
            NBAT = (NCHUNK + BBATCH - 1) // BBATCH if 'B' in stages else 0
            with tc.tile_pool(name="projps", bufs=4, space="PSUM") as ppp, \
                 tc.tile_pool(name="projsb", bufs=3) as psb:
                for i in range(NBAT):
                    o = i * 128 * BBATCH
                    cb = min(128 * BBATCH, N - o)          # rows this batch
                    nch = (cb + 127) // 128
                    tab, to = (table1A, o) if o < SPLIT else (table1B, o - SPLIT)
                    xb = psb.tile([F_IN, BBATCH * 128], BF16, tag="xb")
                    nc.sync.dma_start(out=xb[:, 0:cb], in_=xT[:, o:o + cb])
                    stq = psb.tile([128, BBATCH, T1C], U16, tag="stq")
                    for j in range(nch):
                        cn = min(128, cb - j * 128)
                        ps = ppp.tile([128, L1COL], F32, space="PSUM")
                        nc.tensor.matmul(out=ps[0:cn, :],
                                         lhsT=xb[:, j * 128:j * 128 + cn],
                                         rhs=W1aug[:, :], start=True, stop=True)
                        if j % 2 == 0:
                            nc.scalar.activation(out=stq[0:cn, j, 0:128].bitcast(F8),
                                                 in_=ps[0:cn, 0:HC1], func=AF.Copy)
                        else:
                            nc.vector.tensor_copy(stq[0:cn, j, 0:128].bitcast(F8),
                                                  ps[0:cn, 0:HC1])
                        nc.vector.tensor_copy(stq[0:cn, j, 128:132].bitcast(BF16),
                                              ps[0:cn, HC1:L1COL])
                    # full 512B-row writes: bigger aligned descriptors, no
                    # HBM read-modify-write (cols 132:256 are never read)
                    if cb == BBATCH * 128:
                        nc.scalar.dma_start(
                            out=tab[to:to + cb, :]
                                .rearrange("(b p) c -> p b c", p=128),
                            in_=stq[:, :, :])
                    else:
                        for j in range(nch):
                            cn = min(128, cb - j * 128)
                            oj = to + j * 128
                            nc.scalar.dma_start(
                                out=tab[oj:oj + cn, :],
                                in_=stq[0:cn, j, :])

            # ---- stage C: layer-1 edge softmax + aggregation per dst window,
            # with the layer-2 projection (old stage D) fused per window ----
            NWIN_C = NWIN if 'C' in stages else 0
            with tc.tile_pool(name="winps", bufs=2, space="PSUM") as wps, \
                 tc.tile_pool(name="klps", bufs=3, space="PSUM") as kps, \
                 tc.tile_pool(name="adps", bufs=3, space="PSUM") as aps, \
                 tc.tile_pool(name="gpool", bufs=4) as gpl, \
                 tc.tile_pool(name="stpool", bufs=3) as stpl, \
                 tc.tile_pool(name="spool", bufs=4) as spl, \
                 tc.tile_pool(name="winsb", bufs=3) as wsb:
                b0 = 0
                for w in range(NWIN_C):
                    BL = int(pp["nbw_low"][w])
                    BH = int(pp["nbw_high"][w])
                    nb = BL + BH
                    cn_w = min(128, NPC - w * 128)
                    # one PSUM tile per window: [0:68]=psK (skip+a_dst own),
                    # [68:86]=psL (fused layer-2 proj), [96:224]=psT (h transpose)
                    xo = wsb.tile([F_IN, 128], BF16, tag="xo")
                    nc.sync.dma_start(out=xo[:, 0:cn_w], in_=xTown[:, w * 128:w * 128 + cn_w])
                    psKL = kps.tile([128, 224], F32, space="PSUM")
                    psK = psKL[:, 0:K1COL]
                    psL = psKL[:, K1COL:K1COL + W2COL]
                    psT = psKL[0:HID, 96:224]
                    nc.tensor.matmul(out=psK[0:cn_w, :], lhsT=xo[:, 0:cn_w], rhs=Wl1aug[:, :],
                                     start=True, stop=True)
                    stK = wsb.tile([128, HEADS], BF16, tag="stK")
                    nc.vector.tensor_copy(stK[0:cn_w, :], psK[0:cn_w, HID:K1COL])
                    # one-hot dst matrices: S built on DVE, St streamed from host
                    S = spl.tile([128, NBWmax, 128], BF16, tag="S")
                    nc.vector.tensor_tensor(
                        out=S[:, 0:nb, :],
                        in0=dstloc_sb[:, b0:b0 + nb].unsqueeze(2).to_broadcast([128, nb, 128]),
                        in1=iota_t[:, :].unsqueeze(1).to_broadcast([128, nb, 128]),
                        op=OP.is_equal)
                    Stw = stpl.tile([128, NBWmax * 128], BF16, tag="Stw")
                    nc.scalar.dma_start(out=Stw[:, 0:nb * 128],
                                        in_=Stdat_d[:, b0 * 128:(b0 + nb) * 128])
                    # per-edge a_dst via St.T @ a_dst-window-table
                    psAD = aps.tile([128, NBWmax * HEADS], F32, space="PSUM")
                    for j in range(nb):
                        nc.tensor.matmul(out=psAD[:, j * HEADS:(j + 1) * HEADS],
                                         lhsT=Stw[:, j * 128:(j + 1) * 128],
                                         rhs=stK[:, :], start=True, stop=True)
                    # gathers
                    G = gpl.tile([128, NBWmax, T1C], U16, tag="G")
                    if BL:
                        chunked_gather(nc, G, table1A[:, :],
                                       gidx_sb, b0, BL, T1C, gregs, qrr)
                    if BH:
                        chunked_gather(nc, G, table1B[:, :],
                                       gidx_sb, b0 + BL, BH, T1C, gregs, qrr, boff=BL)
                    # e = a_src + a_dst; ex = exp(lrelu(e)) = max(exp(e), exp(0.2e))
                    ex = wsb.tile([128, NBWmax, HEADS], F32, tag="ex")
                    nc.vector.tensor_tensor(
                        out=ex[:, 0:nb, :],
                        in0=G[:, 0:nb, 128:132].bitcast(BF16),
                        in1=psAD[:, 0:nb * HEADS].rearrange("p (b h) -> p b h", h=HEADS),
                        op=OP.add)
                    exa = wsb.tile([128, NBWmax, HEADS], BF16, tag="exa")
                    nc.scalar.activation(out=exa[:, 0:nb, :], in_=ex[:, 0:nb, :],
                                         func=AF.Exp)
                    exb = wsb.tile([128, NBWmax, HEADS], BF16, tag="exb")
                    nc.scalar.activation(out=exb[:, 0:nb, :], in_=ex[:, 0:nb, :],
                                         func=AF.Exp, scale=float(neg_slope))
                    nc.vector.tensor_tensor(out=exb[:, 0:nb, :], in0=exa[:, 0:nb, :],
                                            in1=exb[:, 0:nb, :], op=OP.max)
                    Gp = gpl.tile([128, NBWmax, HC1 + HEADS], BF16, tag="Gp")
                    nc.vector.tensor_tensor(
                        out=Gp[:, 0:nb, 0:HC1].rearrange("p b (h c) -> p b h c", h=HEADS),
                        in0=G[:, 0:nb, 0:128].bitcast(F8).rearrange("p b (h c) -> p b h c", h=HEADS),
                        in1=exb[:, 0:nb, :].unsqueeze(3).to_broadcast([128, nb, HEADS, HID]),
                        op=OP.mult)
                    nc.vector.tensor_copy(Gp[:, 0:nb, HC1:HC1 + HEADS], exb[:, 0:nb, :])
                    psW = wps.tile([128, HC1 + HEADS], F32, space="PSUM")
                    for j in range(nb):
                        nc.tensor.matmul(out=psW[:, :], lhsT=S[:, j, :], rhs=Gp[:, j, :],
                                         start=(j == 0), stop=(j == nb - 1))
                    # extract: h = sigmoid(gat/4sum + skip + bias)
                    rec = wsb.tile([128, HEADS], F32, tag="rec")
                    nc.vector.tensor_scalar(out=rec[:, :], in0=psW[:, HC1:HC1 + HEADS],
                                            scalar1=1e-16, scalar2=float(HEADS),
                                            op0=OP.add, op1=OP.mult)
                    nc.vector.reciprocal(rec[:, :], rec[:, :])
                    gat = wsb.tile([128, HC1], F32, tag="gat")
                    nc.vector.tensor_tensor(
                        out=gat[:, :].rearrange("p (h c) -> p h c", h=HEADS),
                        in0=psW[:, 0:HC1].rearrange("p (h c) -> p h c", h=HEADS),
                        in1=rec[:, :].unsqueeze(2).to_broadcast([128, HEADS, HID]),
                        op=OP.mult)
                    hred = wsb.tile([128, HID], F32, tag="hred")
                    nc.vector.tensor_reduce(
                        out=hred[:, :],
                        in_=gat[:, :].rearrange("p (h c) -> p c h", h=HEADS),
                        axis=mybir.AxisListType.X, op=OP.add)
                    nc.vector.tensor_tensor(out=hred[:, :], in0=hred[:, :],
                                            in1=psK[:, 0:HID], op=OP.add)
                    nc.vector.tensor_tensor(out=hred[:, :], in0=hred[:, :],
                                            in1=bias1[:, :], op=OP.add)
                    hwin = wsb.tile([128, HID], F32, tag="hwin")
                    nc.scalar.activation(out=hwin[:, :], in_=hred[:, :], func=AF.Sigmoid)
                    nc.tensor.transpose(out=psT[:, :], in_=hwin[:, :], identity=ident[:, :])
                    htw = wsb.tile([HID, 128], F32, tag="htw")
                    nc.vector.tensor_copy(htw[:, :], psT[:, :])
                    # fused layer-2 projection of this window's own nodes
                    nc.tensor.matmul(out=psL[0:cn_w, :], lhsT=htw[:, 0:cn_w],
                                     rhs=W2aug[:, :], start=True, stop=True)
                    t2st = wsb.tile([128, T2W], U16, tag="t2st")
                    nc.vector.tensor_copy(t2st[0:cn_w, 0:HC2].bitcast(BF16),
                                          psL[0:cn_w, 0:HC2])
                    nc.vector.tensor_copy(t2st[0:cn_w, HC2:T2W].bitcast(F32),
                                          psL[0:cn_w, HC2:HC2 + HEADS])
                    if w < WSPLIT:
                        nc.sync.dma_start(out=t2cA[w * 128:w * 128 + cn_w, :],
                                          in_=t2st[0:cn_w, :])
                    else:
                        ob = (w - WSPLIT) * 128
                        nc.sync.dma_start(out=t2cB[ob:ob + cn_w, :],
                                          in_=t2st[0:cn_w, :])
                    nc.vector.tensor_copy(ad2sb[0:cn_w, w, :],
                                          psL[0:cn_w, HC2 + HEADS:HC2 + 2 * HEADS])
                    nc.vector.tensor_copy(skip2sb[0:cn_w, w, :],
                                          psL[0:cn_w, HC2 + 2 * HEADS:W2COL])
                    b0 += nb
                    if w == WSPLIT - 1 and 'D' in stages:
                        # chunk A complete on our core: allgather + expand it
                        # while the rest of stage C runs; stage E's A-side
                        # gathers unblock as soon as the expand lands
                        nc.gpsimd.collective_compute(
                            "AllGather", OP.bypass, replica_groups=[list(range(8))],
                            ins=[t2cA[:, :]], outs=[tab2cA[:, :]])
                        nc.sync.dma_start(out=table2A[:, 0:T2W], in_=tab2cA[:, :])

            if 'D' in stages:
                nc.gpsimd.collective_compute(
                    "AllGather", OP.bypass, replica_groups=[list(range(8))],
                    ins=[t2cB[:, :]], outs=[tab2cB[:, :]])
                nc.sync.dma_start(out=table2B[:, 0:T2W], in_=tab2cB[:, :])

            # ---- stage E: layer-2 edge softmax + aggregation ----
            NWIN_E = NWIN if 'E' in stages else 0
            with tc.tile_pool(name="w2ps", bufs=3, space="PSUM") as wps2, \
                 tc.tile_pool(name="ad2ps", bufs=3, space="PSUM") as aps2, \
                 tc.tile_pool(name="s2pool", bufs=4) as spl2, \
                 tc.tile_pool(name="st2pool", bufs=3) as stpl2, \
                 tc.tile_pool(name="w2sb", bufs=4) as w2sb:
                b0 = 0
                for w in range(NWIN_E):
                    BL = int(pp["nbw_low"][w])
                    BH = int(pp["nbw_high"][w])
                    nb = BL + BH
                    St2 = stpl2.tile([128, NBWmax * 128], BF16, tag="St2")
                    nc.scalar.dma_start(out=St2[:, 0:nb * 128],
                                        in_=Stdat_d[:, b0 * 128:(b0 + nb) * 128])
                    psAD2 = aps2.tile([128, NBWmax * HEADS], F32, space="PSUM")
                    for j in range(nb):
                        nc.tensor.matmul(out=psAD2[:, j * HEADS:(j + 1) * HEADS],
                                         lhsT=St2[:, j * 128:(j + 1) * 128],
                                         rhs=ad2sb[:, w, :], start=True, stop=True)
                    g2s = w2sb.tile([128, NBWmax, T2C], U16, tag="g2s")
                    if BL:
                        chunked_gather(nc, g2s, table2A[:, :],
                                       gidx_sb, b0, BL, T2C, gregs, qrr)
                    if BH:
                        chunked_gather(nc, g2s, table2B[:, :],
                                       gidx_sb, b0 + BL, BH, T2C, gregs, qrr, boff=BL)
                    ex2 = w2sb.tile([128, NBWmax, HEADS], F32, tag="ex2")
                    nc.vector.tensor_tensor(
                        out=ex2[:, 0:nb, :],
                        in0=g2s[:, 0:nb, HC2:16].bitcast(F32),
                        in1=psAD2[:, 0:nb * HEADS].rearrange("p (b h) -> p b h", h=HEADS),
                        op=OP.add)
                    ex2a = w2sb.tile([128, NBWmax, HEADS], BF16, tag="ex2a")
                    nc.scalar.activation(out=ex2a[:, 0:nb, :], in_=ex2[:, 0:nb, :],
                                         func=AF.Exp)
                    ex2b = w2sb.tile([128, NBWmax, HEADS], BF16, tag="ex2b")
                    nc.scalar.activation(out=ex2b[:, 0:nb, :], in_=ex2[:, 0:nb, :],
                                         func=AF.Exp, scale=float(neg_slope))
                    nc.vector.tensor_tensor(out=ex2b[:, 0:nb, :], in0=ex2a[:, 0:nb, :],
                                            in1=ex2b[:, 0:nb, :], op=OP.max)
                    R2 = w2sb.tile([128, NBWmax, R2COL], BF16, tag="R2")
                    nc.vector.tensor_tensor(
                        out=R2[:, 0:nb, 0:HC2].rearrange("p b (h c) -> p b h c", h=HEADS),
                        in0=g2s[:, 0:nb, 0:HC2].bitcast(BF16).rearrange("p b (h c) -> p b h c", h=HEADS),
                        in1=ex2b[:, 0:nb, :].unsqueeze(3).to_broadcast([128, nb, HEADS, OUT]),
                        op=OP.mult)
                    nc.vector.tensor_copy(R2[:, 0:nb, HC2:R2COL], ex2b[:, 0:nb, :])
                    S2 = spl2.tile([128, NBWmax, 128], BF16, tag="S2")
                    nc.vector.tensor_tensor(
                        out=S2[:, 0:nb, :],
                        in0=dstloc_sb[:, b0:b0 + nb].unsqueeze(2).to_broadcast([128, nb, 128]),
                        in1=iota_t[:, :].unsqueeze(1).to_broadcast([128, nb, 128]),
                        op=OP.is_equal)
                    psW2 = wps2.tile([128, R2COL], F32, space="PSUM")
                    for j in range(nb):
                        nc.tensor.matmul(out=psW2[:, :], lhsT=S2[:, j, :], rhs=R2[:, j, :],
                                         start=(j == 0), stop=(j == nb - 1))
                    rec2 = w2sb.tile([128, HEADS], F32, tag="rec2")
                    nc.vector.tensor_scalar(out=rec2[:, :], in0=psW2[:, HC2:R2COL],
                                            scalar1=1e-16, scalar2=float(HEADS),
                                            op0=OP.add, op1=OP.mult)
                    nc.vector.reciprocal(rec2[:, :], rec2[:, :])
                    og = w2sb.tile([128, HC2], F32, tag="og")
                    nc.vector.tensor_tensor(
                        out=og[:, :].rearrange("p (h c) -> p h c", h=HEADS),
                        in0=psW2[:, 0:HC2].rearrange("p (h c) -> p h c", h=HEADS),
                        in1=rec2[:, :].unsqueeze(2).to_broadcast([128, HEADS, OUT]),
                        op=OP.mult)
                    ored = w2sb.tile([128, OUT], F32, tag="ored")
                    nc.vector.tensor_reduce(
                        out=ored[:, :],
                        in_=og[:, :].rearrange("p (h c) -> p c h", h=HEADS),
                        axis=mybir.AxisListType.X, op=OP.add)
                    nc.vector.tensor_tensor(out=ored[:, :], in0=ored[:, :],
                                            in1=skip2sb[:, w, :], op=OP.add)
                    nc.vector.tensor_tensor(out=outsb[:, w, :], in0=ored[:, :],
                                            in1=bias2[:, :], op=OP.add)
                    b0 += nb

            # ---- final output DMA ----
            wf = NPC // 128 if 'E' in stages else 0
            rem = NPC % 128 if 'E' in stages else 0
            if wf:
                nc.sync.dma_start(
                    out=out_d[0:wf * 128, :].rearrange("(w p) c -> p w c", p=128),
                    in_=outsb[:, 0:wf, :])
            if rem:
                nc.sync.dma_start(out=out_d[wf * 128:NPC, :], in_=outsb[0:rem, wf, :])

    fix_library_reloads(nc)
    split_multi_waits(nc)
    return nc


def make_in_maps(pp, inputs, N, F_IN=128, HID=64, HEADS=4, OUT=2):
    NPC = pp["npc"]
    NB = pp["NB"]
    x = np.ascontiguousarray(np.asarray(inputs["x"], dtype=np.float32))
    xT = np.ascontiguousarray(x.T)
    f32 = lambda a, shp: np.ascontiguousarray(np.asarray(a, dtype=np.float32).reshape(shp))

    W1s = f32(inputs["W1s"], (F_IN, HEADS * HID))
    W1d = f32(inputs["W1d"], (F_IN, HEADS * HID))
    a1s = f32(inputs["a1s"], (HEADS, HID))
    a1d = f32(inputs["a1d"], (HEADS, HID))
    W2s = f32(inputs["W2s"], (HID, HEADS * OUT))
    W2d = f32(inputs["W2d"], (HID, HEADS * OUT))
    a2s = f32(inputs["a2s"], (HEADS, OUT))
    a2d = f32(inputs["a2d"], (HEADS, OUT))
    # host weight folding: a_src/a_dst projections as extra W columns
    fold1s = np.einsum('fhc,hc->fh', W1s.reshape(F_IN, HEADS, HID), a1s)
    fold1d = np.einsum('fhc,hc->fh', W1d.reshape(F_IN, HEADS, HID), a1d)
    fold2s = np.einsum('fhc,hc->fh', W2s.reshape(HID, HEADS, OUT), a2s)
    fold2d = np.einsum('fhc,hc->fh', W2d.reshape(HID, HEADS, OUT), a2d)
    W1aug = np.ascontiguousarray(np.concatenate([W1s, fold1s], axis=1))
    Wl1aug = np.ascontiguousarray(
        np.concatenate([f32(inputs["Wl1"], (F_IN, HID)), fold1d], axis=1))
    W2aug = np.ascontiguousarray(np.concatenate(
        [W2s, fold2s, fold2d, f32(inputs["Wl2"], (HID, OUT))], axis=1))
    bias1 = np.tile((f32(inputs["b1"], (1, HID)) + f32(inputs["bl1"], (1, HID))), (128, 1))
    bias2 = np.tile((f32(inputs["b2"], (1, OUT)) + f32(inputs["bl2"], (1, OUT))), (128, 1))

    import ml_dtypes
    bf = ml_dtypes.bfloat16
    # permute node columns into the chunked A/B table order so stage B's
    # sequential writes land at the right table1A/table1B rows
    ra, nb8 = RA, NPC - RA
    aidx = np.concatenate([np.arange(c * NPC, c * NPC + ra) for c in range(8)])
    bidx = np.concatenate([np.arange(c * NPC + ra, (c + 1) * NPC) for c in range(8)])
    xTperm = xT[:, np.concatenate([aidx, bidx])]
    common = {
        "xT": np.ascontiguousarray(xTperm.astype(bf)),
        "W1aug": np.ascontiguousarray(W1aug.astype(bf)),
        "Wl1aug": np.ascontiguousarray(Wl1aug.astype(bf)),
        "W2aug": W2aug,
        "bias1": np.ascontiguousarray(bias1),
        "bias2": np.ascontiguousarray(bias2),
    }
    in_maps = []
    for c in range(8):
        m = dict(common)
        m["xTown"] = np.ascontiguousarray(xT[:, c * NPC:(c + 1) * NPC].astype(bf))
        m["gidx"] = pp["gidx"][c]
        m["dstloc"] = pp["dstloc"][c]
        # transposed one-hot: St[d, b*128+p] = (dstloc[p, b] == d), bf16
        dl = pp["dstloc"][c]                       # [128, NB]
        pp_, bb_ = np.nonzero(dl >= 0)
        dv = dl[pp_, bb_].astype(np.int64)
        St = np.zeros((128, NB * 128), np.uint16)
        St[dv, bb_ * 128 + pp_] = 0x3F80           # 1.0 bf16
        m["Stdat"] = St.view(ml_dtypes.bfloat16)
        in_maps.append(m)
    return in_maps


_BUILD_CACHE = {}
LAST_RESULTS = None


def kernel(**inputs):
    """Full inputs in, full [N, 2] float32 output out."""
    global LAST_RESULTS
    trace = bool(inputs.pop("_trace", False))
    pp = preprocess(inputs["edge_index"], N_NODES)
    key = (pp["NB"], tuple(pp["nbw_low"]), tuple(pp["nbw_high"]))
    if key not in _BUILD_CACHE:
        _BUILD_CACHE[key] = build(pp, N_NODES)
    nc = _BUILD_CACHE[key]
    in_maps = make_in_maps(pp, inputs, N_NODES)
    res = run_bass_kernel_spmd(nc, in_maps, list(range(8)), trace=trace)
    LAST_RESULTS = res
    out = np.concatenate([res.results[c]["out"] for c in range(8)], axis=0)
    return out.astype(np.float32)


# revision 47
# speedup vs baseline: 4.0787x; 1.0119x over previous
"""Self-contained Trainium2 Bass kernel for the 2-layer GAT problem
(nn_GAT_26714696581831). 8-core SPMD: edges sorted by dst, 8 dst-range
shards; per-window one-hot matmul aggregation with dma_gather row fetches
spread across 4 SWDGE queues (4 Q7 core pairs generate descriptors in
parallel).

kernel(**inputs) takes the FULL unsharded inputs and returns the FULL
[50000, 2] output.
"""
import sys
sys.path.insert(0, '/opt/trn_rl_repo')
import numpy as np
import concourse.bass as bass
import concourse.mybir as mybir
import concourse.tile as tile
from concourse import library_config
from concourse.masks import make_identity
from concourse.bass_utils import run_bass_kernel_spmd

N_NODES = 50000
"""Workarounds for this walrus build, which rejects any instruction carrying
more than one sync-wait command: hoist extra waits onto same-engine NoOps
inserted immediately before the instruction."""


_ctr = [0]

def split_multi_waits(nc, max_waits=1):
    for fn in nc.m.functions:
        for bb in fn.blocks:
            insts = bb.instructions
            i = 0
            while i < len(insts):
                ins = insts[i]
                si = ins.sync_info
                if si is not None and si.on_wait and len(si.on_wait) > max_waits:
                    waits = list(si.on_wait)
                    keep = waits[-max_waits:]
                    hoist = waits[:-max_waits]
                    si.on_wait = keep
                    for w in hoist:
                        _ctr[0] += 1
                        n = mybir.InstNoOp(name=f"waitsplit-{_ctr[0]}", ins=[], outs=[])
                        n.engine = ins.engine
                        n.sync_info = mybir.SyncInfo(on_wait=[w], on_update=[])
                        insts.insert(i, n)
                        i += 1
                i += 1


def fix_library_reloads(nc):
    """bass_rust leaves InstPseudoReloadLibraryIndex.instr empty; this walrus
    rejects zero-length ISA instructions. Encode the 64-byte
    PSEUDO_LIBRARY_RELOAD_INDEX struct with the live ISA tables."""
    isa = nc.isa
    sn = 'NEURON_ISA_TPB_PSEUDO_LIBRARY_RELOAD_INDEX_STRUCT'
    e = isa.get_enum("NEURON_ISA_TPB_PSEUDO_OPCODE")
    val = e.NEURON_ISA_TPB_PSEUDO_OPCODE_PSEUDO_LIBRARY_RELOAD_INDEX.value
    for fn in nc.m.functions:
        for bb in fn.blocks:
            for ins in bb.instructions:
                if type(ins).__name__ == 'InstPseudoReloadLibraryIndex' and not ins.instr:
                    b = isa.asm({"header": {"opcode": 223, "inst_word_len": 16},
                                 "pseudo_opcode": val,
                                 "lib_index": ins.lib_index}, sn)
                    ins.instr = [int(x) for x in b]




WIN = 128                  # dst nodes per window
RA = 4096                  # chunk-A rows per core (32 windows); 8*RA = 32768
SPLIT = 32768              # total chunk-A rows = int16 positive limit


def preprocess(edge_index, n_nodes, ncores=8):
    src = np.asarray(edge_index[0], dtype=np.int64)
    dst = np.asarray(edge_index[1], dtype=np.int64)
    npc = n_nodes // ncores
    nwin = (npc + WIN - 1) // WIN
    ra = RA
    rb = npc - ra

    order = np.argsort(dst, kind="stable")
    src_s = src[order]
    dst_s = dst[order]

    # src ids remapped to the chunked table layout: chunk A = rows r<ra of
    # every core (table row c*ra+r), chunk B = the rest (row c*rb+(r-ra)).
    # Both index spaces fit int16; C (table1) and E (table2) share them.
    sc, sr = src_s // npc, src_s % npc
    is_a = sr < ra
    sidx = np.where(is_a, sc * ra + sr, sc * rb + (sr - ra))

    counts = np.bincount(dst_s // npc, minlength=ncores)
    core_slices = np.concatenate([[0], np.cumsum(counts)])

    nlow = np.zeros((ncores, nwin), dtype=np.int64)
    nhigh = np.zeros((ncores, nwin), dtype=np.int64)
    per_core_win_edges = []
    for c in range(ncores):
        s0, s1 = core_slices[c], core_slices[c + 1]
        csrc = sidx[s0:s1]
        cisa = is_a[s0:s1]
        cdst = dst_s[s0:s1]
        wloc = (cdst - c * npc) // WIN
        dloc = (cdst - c * npc) % WIN
        wins = []
        for w in range(nwin):
            m = wloc == w
            ws, wd, wa = csrc[m], dloc[m], cisa[m]
            wins.append((ws[wa], ws[~wa], wd[wa], wd[~wa]))
            nlow[c, w] = wa.sum()
            nhigh[c, w] = (~wa).sum()
        per_core_win_edges.append(wins)

    nbw_low = ((nlow.max(axis=0) + 127) // 128).astype(int)
    nbw_high = ((nhigh.max(axis=0) + 127) // 128).astype(int)
    for w in range(nwin):
        if nbw_low[w] + nbw_high[w] == 0:
            nbw_low[w] = 1
    NB = int(nbw_low.sum() + nbw_high.sum())

    gidx_lin = np.zeros((ncores, NB * 128), dtype=np.int16)
    srcidx_lin = np.zeros((ncores, NB * 128), dtype=np.int32)
    dstidx_lin = np.zeros((ncores, NB * 128), dtype=np.int32)
    dstloc_lin = np.full((ncores, NB * 128), -1, dtype=np.int16)

    for c in range(ncores):
        b0 = 0
        for w in range(nwin):
            slo, shi, dlo, dhi = per_core_win_edges[c][w]
            o = b0 * 128
            gidx_lin[c, o:o + len(slo)] = slo.astype(np.int16)
            srcidx_lin[c, o:o + len(slo)] = slo
            dstidx_lin[c, o:o + len(dlo)] = dlo + w * WIN + c * npc
            dstloc_lin[c, o:o + len(dlo)] = dlo.astype(np.int16)
            b0 += int(nbw_low[w])
            o = b0 * 128
            gidx_lin[c, o:o + len(shi)] = shi.astype(np.int16)
            srcidx_lin[c, o:o + len(shi)] = shi
            dstidx_lin[c, o:o + len(dhi)] = dhi + w * WIN + c * npc
            dstloc_lin[c, o:o + len(dhi)] = dhi.astype(np.int16)
            b0 += int(nbw_high[w])
        assert b0 == NB

    def wrap16(lin):  # [NC, NB*128] -> [NC, 128, NB*8] dma_gather layout
        x = lin.reshape(ncores, NB * 8, 16).transpose(0, 2, 1)
        return np.ascontiguousarray(np.tile(x, (1, 8, 1)))

    # dst-local indices for the a_dst gather (per-core local table, < 32768)
    adidx_lin = np.empty((ncores, NB * 128), dtype=np.int16)
    for c in range(ncores):
        loc = dstidx_lin[c].astype(np.int64) - c * npc
        loc[dstloc_lin[c] < 0] = 0          # pad slots -> row 0
        adidx_lin[c] = loc.astype(np.int16)

    def slotlay(lin, dtype):  # [NC, NB*128] -> [NC, 128, NB] ([p,b] = slot b*128+p)
        return np.ascontiguousarray(lin.reshape(ncores, NB, 128).transpose(0, 2, 1)).astype(dtype)

    return dict(
        NB=NB, nwin=nwin, npc=npc, ncores=ncores,
        nbw_low=nbw_low, nbw_high=nbw_high,
        gidx=wrap16(gidx_lin),
        adidx=wrap16(adidx_lin),
        srcidx=slotlay(srcidx_lin, np.int32),
        dstidx=slotlay(dstidx_lin, np.int32),
        dstloc=slotlay(dstloc_lin, np.int16),
    )




F32 = mybir.dt.float32
BF16 = mybir.dt.bfloat16
F8 = mybir.dt.float8e4
I16 = mybir.dt.int16
U16 = mybir.dt.uint16
AF = mybir.ActivationFunctionType
OP = mybir.AluOpType

SPLIT = 32768
GCHUNK = 8   # blocks per dma_gather call (1024 idx: single-packet-safe)
NQ = 4       # SWDGE queues: queue q's descriptors are generated by Q7 core
             # pair (2q, 2q+1); round-robin spreads desc-gen over all 8 cores


def chunked_gather(nc, out_tile, in_ap, idx_sb, b0, nblk, elem, regs, qrr, boff=0):
    """Issue dma_gather in <=GCHUNK-block chunks writing out_tile[:, boff+i...].

    Chunk sizes are balanced (11 -> 6+5, not 8+3) and queues strictly
    rotate so consecutive calls always hit different Q7 desc-gen core
    pairs - in-order instruction retirement then pipelines ~4 deep.
    """
    nchunks = (nblk + GCHUNK - 1) // GCHUNK
    base, rem = divmod(nblk, nchunks)
    done = 0
    for i in range(nchunks):
        step = base + (1 if i < rem else 0)
        n = step * 128
        if n not in regs:
            regs[n] = nc.gpsimd.to_reg(n)
        nc.gpsimd.dma_gather(
            out_tile[:, boff + done:boff + done + step, :], in_ap,
            idx_sb[:, (b0 + done) * 8:(b0 + done + step) * 8],
            n, regs[n], elem, queue_num=qrr[0] % NQ)
        qrr[0] += 1
        done += step


def build(pp, N, F_IN=128, HID=64, HEADS=4, OUT=2, neg_slope=0.2, stages='ABCDE'):
    NB = pp["NB"]
    NWIN = pp["nwin"]
    NPC = pp["npc"]
    HC1 = HEADS * HID          # 256
    HC2 = HEADS * OUT          # 8
    NBWmax = int(max(pp["nbw_low"][w] + pp["nbw_high"][w] for w in range(NWIN)))
    NCHUNK = (N + 127) // 128
    T1C = 256                  # u16 cols = 512B rows: 256 fp8 xs + 4 bf16 a_src
    L1COL = HC1 + HEADS        # 260: xs + a_src fold
    K1COL = HID + HEADS        # 68  (skip + W_ad fold)
    W2COL = HC2 + 2 * HEADS + OUT  # 18
    R2COL = HC2 + HEADS        # 12
    T2C = 128                  # u16 cols = 256B rows
    T2W = 16                   # used u16 cols of a table2 row (z2 bf16 + a2s f32)
    BBATCH = 16                # stage-B chunks per DMA batch
    WSPLIT = RA // 128         # C-windows per t2 chunk A (chunked allgather)
    ROWA = RA                  # own rows in chunk A (4096)
    ROWB = NPC - ROWA          # own rows in chunk B (2154)

    nc = bass.Bass("TRN2", target_bir_lowering=False, debug=False,
                   num_devices=8, num_swdge_queues=NQ)

    # ---- I/O ----
    xT = nc.dram_tensor("xT", [F_IN, N], BF16, kind="ExternalInput")
    xTown = nc.dram_tensor("xTown", [F_IN, NPC], BF16, kind="ExternalInput")
    W1aug_d = nc.dram_tensor("W1aug", [F_IN, L1COL], BF16, kind="ExternalInput")
    Wl1aug_d = nc.dram_tensor("Wl1aug", [F_IN, K1COL], BF16, kind="ExternalInput")
    W2aug_d = nc.dram_tensor("W2aug", [HID, W2COL], F32, kind="ExternalInput")
    bias1_d = nc.dram_tensor("bias1", [128, HID], F32, kind="ExternalInput")
    bias2_d = nc.dram_tensor("bias2", [128, OUT], F32, kind="ExternalInput")
    gidx_d = nc.dram_tensor("gidx", [128, NB * 8], I16, kind="ExternalInput")
    dstloc_d = nc.dram_tensor("dstloc", [128, NB], I16, kind="ExternalInput")
    # transposed one-hot dst matrices St[d, slot] (host-built, streamed):
    # fetch per-edge a_dst via a tiny matmul instead of a dma_gather
    Stdat_d = nc.dram_tensor("Stdat", [128, NB * 128], BF16, kind="ExternalInput")
    out_d = nc.dram_tensor("out", [NPC, OUT], F32, kind="ExternalOutput")

    # internal DRAM. All tables are split at the (core, r<RA) boundary:
    # chunk A = rows {c*NPC+r : r<RA} stored at c*RA+r (8*RA = 32768 rows),
    # chunk B = the rest. The same split chunks the layer-2 allgather, so
    # chunk-A tables complete mid-stage-C and stage E's A-side gathers can
    # start while C still runs.
    table1A = nc.dram_tensor("table1A", [8 * ROWA, T1C], U16)
    table1B = nc.dram_tensor("table1B", [8 * ROWB, T1C], U16)
    t2cA = nc.dram_tensor("t2cA", [ROWA, T2W], U16)
    t2cB = nc.dram_tensor("t2cB", [ROWB, T2W], U16)
    tab2cA = nc.dram_tensor("tab2cA", [8 * ROWA, T2W], U16, addr_space="Shared")
    tab2cB = nc.dram_tensor("tab2cB", [8 * ROWB, T2W], U16, addr_space="Shared")
    table2A = nc.dram_tensor("table2A", [8 * ROWA, T2C], U16)
    table2B = nc.dram_tensor("table2B", [8 * ROWB, T2C], U16)

    with tile.TileContext(nc) as tc:
        with tc.tile_pool(name="const", bufs=1) as cpool, \
             tc.tile_pool(name="resident", bufs=1) as rpool:

            # ---- constants (all weight folding/permutation done on host) ----
            W1aug = cpool.tile([F_IN, L1COL], BF16)
            nc.sync.dma_start(out=W1aug[:, :], in_=W1aug_d[:, :])
            Wl1aug = cpool.tile([F_IN, K1COL], BF16)
            nc.sync.dma_start(out=Wl1aug[:, :], in_=Wl1aug_d[:, :])
            W2aug = cpool.tile([HID, W2COL], F32)
            nc.sync.dma_start(out=W2aug[:, :], in_=W2aug_d[:, :])
            bias1 = cpool.tile([128, HID], F32)
            nc.sync.dma_start(out=bias1[:, :], in_=bias1_d[:, :])
            bias2 = cpool.tile([128, OUT], F32)
            nc.sync.dma_start(out=bias2[:, :], in_=bias2_d[:, :])

            iota_t = cpool.tile([128, 128], I16)
            nc.gpsimd.iota(iota_t[:, :], pattern=[[1, 128]], base=0, channel_multiplier=0)
            ident = cpool.tile([128, 128], F32)
            make_identity(nc, ident[:, :])

            gidx_sb = rpool.tile([128, NB * 8], I16)
            nc.sync.dma_start(out=gidx_sb[:, :], in_=gidx_d[:, :])
            dstloc_sb = rpool.tile([128, NB], I16)
            nc.sync.dma_start(out=dstloc_sb[:, :], in_=dstloc_d[:, :])

            # all standard-library gpsimd ops (iota/affine_select/memset) are
            # above; from here on the Q7 carveout holds the mlp library.
            nc.gpsimd.load_library(library_config.mlp)
            gregs = {}
            qrr = [0, 0, 0, 0]

            if 'C' in stages:
                skip2sb = rpool.tile([128, NWIN, OUT], F32)
                outsb = rpool.tile([128, NWIN, OUT], F32)
                ad2sb = rpool.tile([128, NWIN, HEADS], BF16)
                # zero-fill: the last ragged window leaves tail partitions
                # unwritten and 0 x NaN would poison the psAD2 contraction
                nc.vector.memset(ad2sb[:, :, :], 0.0)

            # ---- stage B: project all N nodes, table1 row = [xs | a_src] bf16 ----
            assert SPLIT % (128 * BBATCH) == 0
            NBAT = (NCHUNK + BBATCH - 1) // BBATCH if 'B' in stages else 0
            with tc.tile_pool(name="projps", bufs=4, space="PSUM") as ppp, \
                 tc.tile_pool(name="projsb", bufs=3) as psb:
                for i in range(NBAT):
                    o = i * 128 * BBATCH
                    cb = min(128 * BBATCH, N - o)          # rows this batch
                    nch = (cb + 127) // 128
                    tab, to = (table1A, o) if o < SPLIT else (table1B, o - SPLIT)
                    xb = psb.tile([F_IN, BBATCH * 128], BF16, tag="xb")
                    nc.sync.dma_start(out=xb[:, 0:cb], in_=xT[:, o:o + cb])
                    stq = psb.tile([128, BBATCH, T1C], U16, tag="stq")
                    for j in range(nch):
                        cn = min(128, cb - j * 128)
                        ps = ppp.tile([128, L1COL], F32, space="PSUM")
                        nc.tensor.matmul(out=ps[0:cn, :],
                                         lhsT=xb[:, j * 128:j * 128 + cn],
                                         rhs=W1aug[:, :], start=True, stop=True)
                        if j % 2 == 0:
                            nc.scalar.activation(out=stq[0:cn, j, 0:128].bitcast(F8),
                                                 in_=ps[0:cn, 0:HC1], func=AF.Copy)
                        else:
                            nc.vector.tensor_copy(stq[0:cn, j, 0:128].bitcast(F8),
                                                  ps[0:cn, 0:HC1])
                        nc.vector.tensor_copy(stq[0:cn, j, 128:132].bitcast(BF16),
                                              ps[0:cn, HC1:L1COL])
                    # full 512B-row writes: bigger aligned descriptors, no
                    # HBM read-modify-write (cols 132:256 are never read)
                    if cb == BBATCH * 128:
                        nc.scalar.dma_start(
                            out=tab[to:to + cb, :]
                                .rearrange("(b p) c -> p b c", p=128),
                            in_=stq[:, :, :])
                    else:
                        for j in range(nch):
                            cn = min(128, cb - j * 128)
                            oj = to + j * 128
                            nc.scalar.dma_start(
                                out=tab[oj:oj + cn, :],
                                in_=stq[0:cn, j, :])

            # ---- stage C: layer-1 edge softmax + aggregation per dst window,
            # with the layer-2 projection (old stage D) fused per window ----
            NWIN_C = NWIN if 'C' in stages else 0
            with tc.tile_pool(name="winps", bufs=2, space="PSUM") as wps, \
                 tc.tile_pool(name="klps", bufs=3, space="PSUM") as kps, \
                 tc.tile_pool(name="adps", bufs=3, space="PSUM") as aps, \
                 tc.tile_pool(name="gpool", bufs=6) as gpl, \
                 tc.tile_pool(name="stpool", bufs=3) as stpl, \
                 tc.tile_pool(name="spool", bufs=4) as spl, \
                 tc.tile_pool(name="winsb", bufs=3) as wsb:
                b0 = 0
                for w in range(NWIN_C):
                    BL = int(pp["nbw_low"][w])
                    BH = int(pp["nbw_high"][w])
                    nb = BL + BH
                    cn_w = min(128, NPC - w * 128)
                    # one PSUM tile per window: [0:68]=psK (skip+a_dst own),
                    # [68:86]=psL (fused layer-2 proj), [96:224]=psT (h transpose)
                    xo = wsb.tile([F_IN, 128], BF16, tag="xo")
                    nc.sync.dma_start(out=xo[:, 0:cn_w], in_=xTown[:, w * 128:w * 128 + cn_w])
                    psKL = kps.tile([128, 224], F32, space="PSUM")
                    psK = psKL[:, 0:K1COL]
                    psL = psKL[:, K1COL:K1COL + W2COL]
                    psT = psKL[0:HID, 96:224]
                    nc.tensor.matmul(out=psK[0:cn_w, :], lhsT=xo[:, 0:cn_w], rhs=Wl1aug[:, :],
                                     start=True, stop=True)
                    stK = wsb.tile([128, HEADS], BF16, tag="stK")
                    nc.vector.tensor_copy(stK[0:cn_w, :], psK[0:cn_w, HID:K1COL])
                    # one-hot dst matrices: S built on DVE, St streamed from host
                    S = spl.tile([128, NBWmax, 128], BF16, tag="S")
                    nc.vector.tensor_tensor(
                        out=S[:, 0:nb, :],
                        in0=dstloc_sb[:, b0:b0 + nb].unsqueeze(2).to_broadcast([128, nb, 128]),
                        in1=iota_t[:, :].unsqueeze(1).to_broadcast([128, nb, 128]),
                        op=OP.is_equal)
                    Stw = stpl.tile([128, NBWmax * 128], BF16, tag="Stw")
                    nc.scalar.dma_start(out=Stw[:, 0:nb * 128],
                                        in_=Stdat_d[:, b0 * 128:(b0 + nb) * 128])
                    # per-edge a_dst via St.T @ a_dst-window-table
                    psAD = aps.tile([128, NBWmax * HEADS], F32, space="PSUM")
                    for j in range(nb):
                        nc.tensor.matmul(out=psAD[:, j * HEADS:(j + 1) * HEADS],
                                         lhsT=Stw[:, j * 128:(j + 1) * 128],
                                         rhs=stK[:, :], start=True, stop=True)
                    # gathers
                    G = gpl.tile([128, NBWmax, T1C], U16, tag="G")
                    if BL:
                        chunked_gather(nc, G, table1A[:, :],
                                       gidx_sb, b0, BL, T1C, gregs, qrr)
                    if BH:
                        chunked_gather(nc, G, table1B[:, :],
                                       gidx_sb, b0 + BL, BH, T1C, gregs, qrr, boff=BL)
                    # e = a_src + a_dst; ex = exp(lrelu(e)) = max(exp(e), exp(0.2e))
                    ex = wsb.tile([128, NBWmax, HEADS], F32, tag="ex")
                    nc.vector.tensor_tensor(
                        out=ex[:, 0:nb, :],
                        in0=G[:, 0:nb, 128:132].bitcast(BF16),
                        in1=psAD[:, 0:nb * HEADS].rearrange("p (b h) -> p b h", h=HEADS),
                        op=OP.add)
                    exa = wsb.tile([128, NBWmax, HEADS], BF16, tag="exa")
                    nc.scalar.activation(out=exa[:, 0:nb, :], in_=ex[:, 0:nb, :],
                                         func=AF.Exp)
                    exb = wsb.tile([128, NBWmax, HEADS], BF16, tag="exb")
                    nc.scalar.activation(out=exb[:, 0:nb, :], in_=ex[:, 0:nb, :],
                                         func=AF.Exp, scale=float(neg_slope))
                    nc.vector.tensor_tensor(out=exb[:, 0:nb, :], in0=exa[:, 0:nb, :],
                                            in1=exb[:, 0:nb, :], op=OP.max)
                    Gp = gpl.tile([128, NBWmax, HC1 + HEADS], BF16, tag="Gp")
                    nc.vector.tensor_tensor(
                        out=Gp[:, 0:nb, 0:HC1].rearrange("p b (h c) -> p b h c", h=HEADS),
                        in0=G[:, 0:nb, 0:128].bitcast(F8).rearrange("p b (h c) -> p b h c", h=HEADS),
                        in1=exb[:, 0:nb, :].unsqueeze(3).to_broadcast([128, nb, HEADS, HID]),
                        op=OP.mult)
                    nc.vector.tensor_copy(Gp[:, 0:nb, HC1:HC1 + HEADS], exb[:, 0:nb, :])
                    psW = wps.tile([128, HC1 + HEADS], F32, space="PSUM")
                    for j in range(nb):
                        nc.tensor.matmul(out=psW[:, :], lhsT=S[:, j, :], rhs=Gp[:, j, :],
                                         start=(j == 0), stop=(j == nb - 1))
                    # extract: h = sigmoid(gat/4sum + skip + bias)
                    rec = wsb.tile([128, HEADS], F32, tag="rec")
                    nc.vector.tensor_scalar(out=rec[:, :], in0=psW[:, HC1:HC1 + HEADS],
                                            scalar1=1e-16, scalar2=float(HEADS),
                                            op0=OP.add, op1=OP.mult)
                    nc.vector.reciprocal(rec[:, :], rec[:, :])
                    gat = wsb.tile([128, HC1], F32, tag="gat")
                    nc.vector.tensor_tensor(
                        out=gat[:, :].rearrange("p (h c) -> p h c", h=HEADS),
                        in0=psW[:, 0:HC1].rearrange("p (h c) -> p h c", h=HEADS),
                        in1=rec[:, :].unsqueeze(2).to_broadcast([128, HEADS, HID]),
                        op=OP.mult)
                    hred = wsb.tile([128, HID], F32, tag="hred")
                    nc.vector.tensor_reduce(
                        out=hred[:, :],
                        in_=gat[:, :].rearrange("p (h c) -> p c h", h=HEADS),
                        axis=mybir.AxisListType.X, op=OP.add)
                    nc.vector.tensor_tensor(out=hred[:, :], in0=hred[:, :],
                                            in1=psK[:, 0:HID], op=OP.add)
                    nc.vector.tensor_tensor(out=hred[:, :], in0=hred[:, :],
                                            in1=bias1[:, :], op=OP.add)
                    hwin = wsb.tile([128, HID], F32, tag="hwin")
                    nc.scalar.activation(out=hwin[:, :], in_=hred[:, :], func=AF.Sigmoid)
                    nc.tensor.transpose(out=psT[:, :], in_=hwin[:, :], identity=ident[:, :])
                    htw = wsb.tile([HID, 128], F32, tag="htw")
                    nc.vector.tensor_copy(htw[:, :], psT[:, :])
                    # fused layer-2 projection of this window's own nodes
                    nc.tensor.matmul(out=psL[0:cn_w, :], lhsT=htw[:, 0:cn_w],
                                     rhs=W2aug[:, :], start=True, stop=True)
                    t2st = wsb.tile([128, T2W], U16, tag="t2st")
                    nc.vector.tensor_copy(t2st[0:cn_w, 0:HC2].bitcast(BF16),
                                          psL[0:cn_w, 0:HC2])
                    nc.vector.tensor_copy(t2st[0:cn_w, HC2:T2W].bitcast(F32),
                                          psL[0:cn_w, HC2:HC2 + HEADS])
                    if w < WSPLIT:
                        nc.sync.dma_start(out=t2cA[w * 128:w * 128 + cn_w, :],
                                          in_=t2st[0:cn_w, :])
                    else:
                        ob = (w - WSPLIT) * 128
                        nc.sync.dma_start(out=t2cB[ob:ob + cn_w, :],
                                          in_=t2st[0:cn_w, :])
                    nc.vector.tensor_copy(ad2sb[0:cn_w, w, :],
                                          psL[0:cn_w, HC2 + HEADS:HC2 + 2 * HEADS])
                    nc.vector.tensor_copy(skip2sb[0:cn_w, w, :],
                                          psL[0:cn_w, HC2 + 2 * HEADS:W2COL])
                    b0 += nb
                    if w == WSPLIT - 1 and 'D' in stages:
                        # chunk A complete on our core: allgather + expand it
                        # while the rest of stage C runs; stage E's A-side
                        # gathers unblock as soon as the expand lands
                        nc.gpsimd.collective_compute(
                            "AllGather", OP.bypass, replica_groups=[list(range(8))],
                            ins=[t2cA[:, :]], outs=[tab2cA[:, :]])
                        nc.sync.dma_start(out=table2A[:, 0:T2W], in_=tab2cA[:, :])

            if 'D' in stages:
                nc.gpsimd.collective_compute(
                    "AllGather", OP.bypass, replica_groups=[list(range(8))],
                    ins=[t2cB[:, :]], outs=[tab2cB[:, :]])
                nc.sync.dma_start(out=table2B[:, 0:T2W], in_=tab2cB[:, :])

            # ---- stage E: layer-2 edge softmax + aggregation ----
            NWIN_E = NWIN if 'E' in stages else 0
            with tc.tile_pool(name="w2ps", bufs=3, space="PSUM") as wps2, \
                 tc.tile_pool(name="ad2ps", bufs=3, space="PSUM") as aps2, \
                 tc.tile_pool(name="s2pool", bufs=4) as spl2, \
                 tc.tile_pool(name="st2pool", bufs=3) as stpl2, \
                 tc.tile_pool(name="g2pool", bufs=8) as g2pl, \
                 tc.tile_pool(name="w2sb", bufs=4) as w2sb:
                b0 = 0
                for w in range(NWIN_E):
                    BL = int(pp["nbw_low"][w])
                    BH = int(pp["nbw_high"][w])
                    nb = BL + BH
                    St2 = stpl2.tile([128, NBWmax * 128], BF16, tag="St2")
                    nc.scalar.dma_start(out=St2[:, 0:nb * 128],
                                        in_=Stdat_d[:, b0 * 128:(b0 + nb) * 128])
                    psAD2 = aps2.tile([128, NBWmax * HEADS], F32, space="PSUM")
                    for j in range(nb):
                        nc.tensor.matmul(out=psAD2[:, j * HEADS:(j + 1) * HEADS],
                                         lhsT=St2[:, j * 128:(j + 1) * 128],
                                         rhs=ad2sb[:, w, :], start=True, stop=True)
                    g2s = g2pl.tile([128, NBWmax, T2C], U16, tag="g2s")
                    if BL:
                        chunked_gather(nc, g2s, table2A[:, :],
                                       gidx_sb, b0, BL, T2C, gregs, qrr)
                    if BH:
                        chunked_gather(nc, g2s, table2B[:, :],
                                       gidx_sb, b0 + BL, BH, T2C, gregs, qrr, boff=BL)
                    ex2 = w2sb.tile([128, NBWmax, HEADS], F32, tag="ex2")
                    nc.vector.tensor_tensor(
                        out=ex2[:, 0:nb, :],
                        in0=g2s[:, 0:nb, HC2:16].bitcast(F32),
                        in1=psAD2[:, 0:nb * HEADS].rearrange("p (b h) -> p b h", h=HEADS),
                        op=OP.add)
                    ex2a = w2sb.tile([128, NBWmax, HEADS], BF16, tag="ex2a")
                    nc.scalar.activation(out=ex2a[:, 0:nb, :], in_=ex2[:, 0:nb, :],
                                         func=AF.Exp)
                    ex2b = w2sb.tile([128, NBWmax, HEADS], BF16, tag="ex2b")
                    nc.scalar.activation(out=ex2b[:, 0:nb, :], in_=ex2[:, 0:nb, :],
                                         func=AF.Exp, scale=float(neg_slope))
                    nc.vector.tensor_tensor(out=ex2b[:, 0:nb, :], in0=ex2a[:, 0:nb, :],
                                            in1=ex2b[:, 0:nb, :], op=OP.max)
                    R2 = w2sb.tile([128, NBWmax, R2COL], BF16, tag="R2")
                    nc.vector.tensor_tensor(
                        out=R2[:, 0:nb, 0:HC2].rearrange("p b (h c) -> p b h c", h=HEADS),
                        in0=g2s[:, 0:nb, 0:HC2].bitcast(BF16).rearrange("p b (h c) -> p b h c", h=HEADS),
                        in1=ex2b[:, 0:nb, :].unsqueeze(3).to_broadcast([128, nb, HEADS, OUT]),
                        op=OP.mult)
                    nc.vector.tensor_copy(R2[:, 0:nb, HC2:R2COL], ex2b[:, 0:nb, :])
                    S2 = spl2.tile([128, NBWmax, 128], BF16, tag="S2")
                    nc.vector.tensor_tensor(
                        out=S2[:, 0:nb, :],
                        in0=dstloc_sb[:, b0:b0 + nb].unsqueeze(2).to_broadcast([128, nb, 128]),
                        in1=iota_t[:, :].unsqueeze(1).to_broadcast([128, nb, 128]),
                        op=OP.is_equal)
                    psW2 = wps2.tile([128, R2COL], F32, space="PSUM")
                    for j in range(nb):
                        nc.tensor.matmul(out=psW2[:, :], lhsT=S2[:, j, :], rhs=R2[:, j, :],
                                         start=(j == 0), stop=(j == nb - 1))
                    rec2 = w2sb.tile([128, HEADS], F32, tag="rec2")
                    nc.vector.tensor_scalar(out=rec2[:, :], in0=psW2[:, HC2:R2COL],
                                            scalar1=1e-16, scalar2=float(HEADS),
                                            op0=OP.add, op1=OP.mult)
                    nc.vector.reciprocal(rec2[:, :], rec2[:, :])
                    og = w2sb.tile([128, HC2], F32, tag="og")
                    nc.vector.tensor_tensor(
                        out=og[:, :].rearrange("p (h c) -> p h c", h=HEADS),
                        in0=psW2[:, 0:HC2].rearrange("p (h c) -> p h c", h=HEADS),
                        in1=rec2[:, :].unsqueeze(2).to_broadcast([128, HEADS, OUT]),
                        op=OP.mult)
                    ored = w2sb.tile([128, OUT], F32, tag="ored")
                    nc.vector.tensor_reduce(
                        out=ored[:, :],
                        in_=og[:, :].rearrange("p (h c) -> p c h", h=HEADS),
                        axis=mybir.AxisListType.X, op=OP.add)
                    nc.vector.tensor_tensor(out=ored[:, :], in0=ored[:, :],
                                            in1=skip2sb[:, w, :], op=OP.add)
                    nc.vector.tensor_tensor(out=outsb[:, w, :], in0=ored[:, :],
                                            in1=bias2[:, :], op=OP.add)
                    b0 += nb

            # ---- final output DMA ----
            wf = NPC // 128 if 'E' in stages else 0
            rem = NPC % 128 if 'E' in stages else 0
            if wf:
                nc.sync.dma_start(
                    out=out_d[0:wf * 128, :].rearrange("(w p) c -> p w c", p=128),
                    in_=outsb[:, 0:wf, :])
            if rem:
                nc.sync.dma_start(out=out_d[wf * 128:NPC, :], in_=outsb[0:rem, wf, :])

    fix_library_reloads(nc)
    split_multi_waits(nc)
    return nc


def make_in_maps(pp, inputs, N, F_IN=128, HID=64, HEADS=4, OUT=2):
    NPC = pp["npc"]
    NB = pp["NB"]
    x = np.ascontiguousarray(np.asarray(inputs["x"], dtype=np.float32))
    xT = np.ascontiguousarray(x.T)
    f32 = lambda a, shp: np.ascontiguousarray(np.asarray(a, dtype=np.float32).reshape(shp))

    W1s = f32(inputs["W1s"], (F_IN, HEADS * HID))
    W1d = f32(inputs["W1d"], (F_IN, HEADS * HID))
    a1s = f32(inputs["a1s"], (HEADS, HID))
    a1d = f32(inputs["a1d"], (HEADS, HID))
    W2s = f32(inputs["W2s"], (HID, HEADS * OUT))
    W2d = f32(inputs["W2d"], (HID, HEADS * OUT))
    a2s = f32(inputs["a2s"], (HEADS, OUT))
    a2d = f32(inputs["a2d"], (HEADS, OUT))
    # host weight folding: a_src/a_dst projections as extra W columns
    fold1s = np.einsum('fhc,hc->fh', W1s.reshape(F_IN, HEADS, HID), a1s)
    fold1d = np.einsum('fhc,hc->fh', W1d.reshape(F_IN, HEADS, HID), a1d)
    fold2s = np.einsum('fhc,hc->fh', W2s.reshape(HID, HEADS, OUT), a2s)
    fold2d = np.einsum('fhc,hc->fh', W2d.reshape(HID, HEADS, OUT), a2d)
    W1aug = np.ascontiguousarray(np.concatenate([W1s, fold1s], axis=1))
    Wl1aug = np.ascontiguousarray(
        np.concatenate([f32(inputs["Wl1"], (F_IN, HID)), fold1d], axis=1))
    W2aug = np.ascontiguousarray(np.concatenate(
        [W2s, fold2s, fold2d, f32(inputs["Wl2"], (HID, OUT))], axis=1))
    bias1 = np.tile((f32(inputs["b1"], (1, HID)) + f32(inputs["bl1"], (1, HID))), (128, 1))
    bias2 = np.tile((f32(inputs["b2"], (1, OUT)) + f32(inputs["bl2"], (1, OUT))), (128, 1))

    import ml_dtypes
    bf = ml_dtypes.bfloat16
    # permute node columns into the chunked A/B table order so stage B's
    # sequential writes land at the right table1A/table1B rows
    ra, nb8 = RA, NPC - RA
    aidx = np.concatenate([np.arange(c * NPC, c * NPC + ra) for c in range(8)])
    bidx = np.concatenate([np.arange(c * NPC + ra, (c + 1) * NPC) for c in range(8)])
    xTperm = xT[:, np.concatenate([aidx, bidx])]
    common = {
        "xT": np.ascontiguousarray(xTperm.astype(bf)),
        "W1aug": np.ascontiguousarray(W1aug.astype(bf)),
        "Wl1aug": np.ascontiguousarray(Wl1aug.astype(bf)),
        "W2aug": W2aug,
        "bias1": np.ascontiguousarray(bias1),
        "bias2": np.ascontiguousarray(bias2),
    }
    in_maps = []
    for c in range(8):
        m = dict(common)
        m["xTown"] = np.ascontiguousarray(xT[:, c * NPC:(c + 1) * NPC].astype(bf))
        m["gidx"] = pp["gidx"][c]
        m["dstloc"] = pp["dstloc"][c]
        # transposed one-hot: St[d, b*128+p] = (dstloc[p, b] == d), bf16
        dl = pp["dstloc"][c]                       # [128, NB]
        pp_, bb_ = np.nonzero(dl >= 0)
        dv = dl[pp_, bb_].astype(np.int64)
        St = np.zeros((128, NB * 128), np.uint16)
        St[dv, bb_ * 128 + pp_] = 0x3F80           # 1.0 bf16
        m["Stdat"] = St.view(ml_dtypes.bfloat16)
        in_maps.append(m)
    return in_maps


_BUILD_CACHE = {}
LAST_RESULTS = None


def kernel(**inputs):
    """Full inputs in, full [N, 2] float32 output out."""
    global LAST_RESULTS
    trace = bool(inputs.pop("_trace", False))
    pp = preprocess(inputs["edge_index"], N_NODES)
    key = (pp["NB"], tuple(pp["nbw_low"]), tuple(pp["nbw_high"]))
    if key not in _BUILD_CACHE:
        _BUILD_CACHE[key] = build(pp, N_NODES)
    nc = _BUILD_CACHE[key]
    in_maps = make_in_maps(pp, inputs, N_NODES)
    res = run_bass_kernel_spmd(nc, in_maps, list(range(8)), trace=trace)
    LAST_RESULTS = res
    out = np.concatenate([res.results[c]["out"] for c in range(8)], axis=0)
    return out.astype(np.float32)
